# revision 30
# baseline (speedup 1.0000x reference)
"""Trainium2 Bass kernel for nn_Block_87351044866235 (sparse_attention).

Data-parallel over batch: 8 samples -> 8 NeuronCores. Channel-major
layout [C, H*W] on chip; depthwise convs as diagonal bf16 matmuls on
TensorE; 1x1 convs as bf16 matmuls; LN stats via ones-matmuls; q/k gram
via PE transposes + bf16 matmuls; dynamic-k gate mean via a scalar
AllReduce. Activation spills are bf16; v stays SBUF-resident.
"""
import sys, os

for _p in ("/opt/trn_rl_repo", "/root/.axon_site/_ro/trn_rl_repo"):
    if os.path.isdir(_p) and _p not in sys.path:
        sys.path.append(_p)

import numpy as np
import ml_dtypes
import concourse.bass as bass
import concourse.bacc as bacc
import concourse.tile as tile
from concourse import mybir
from concourse import bass_utils

try:
    from concourse import tile_utils as _tu
    _tu.max_sbuf_usage = 208 * 1024
except Exception:
    pass

dt = mybir.dt
Alu = mybir.AluOpType
Act = mybir.ActivationFunctionType
AX = mybir.AxisListType.X

EMBED, PDIM, HEADS, HID = 192, 96, 8, 256
CPH = PDIM // HEADS  # 12
SLOP = 8
RC = 3    # conv output rows per chunk
BR = 12   # rows per band

F32, F32R, BF16 = dt.float32, dt.float32r, dt.bfloat16
FP8 = dt.float8e4
BF16NP = ml_dtypes.bfloat16
FP8NP = ml_dtypes.float8_e4m3
FP8DW = False      # fp8 DoubleRow for the qkv_dw depthwise conv
FP8DW_C = False    # fp8 DoubleRow for the pair/dw3 FFN depthwise convs
DR = mybir.MatmulPerfMode.DoubleRow

# tap pairs for 3x3 depthwise as fp8 DoubleRow (2 taps per pass)
QPAIRS = [(0, 1), (2, 3), (4, 5), (6, 7), (8, None)]


def _ceil(a, b):
    return (a + b - 1) // b


# ----------------------------------------------------------------------------
# host-side weight prep: everything 2D [partitions, free]
# ----------------------------------------------------------------------------

def _prep_weights(p):
    w = {}
    f32 = lambda a: (np.ascontiguousarray(a, np.float32), F32)
    bfw = lambda a: (np.ascontiguousarray(np.asarray(a, np.float32)
                                          .astype(BF16NP)), BF16)
    eps_bn = 1e-5

    w["ident"] = f32(np.eye(128, dtype=np.float32))
    w["identb"] = bfw(np.eye(128, dtype=np.float32))

    # pos depthwise diag: [96, (t*2+cg)*96]
    pw = p["pos_w"][:, 0]  # [192,3,3]
    pos_d = np.zeros((96, 18 * 96), np.float32)
    for t in range(9):
        dy, dx = t // 3 - 1, t % 3 - 1
        for cg in range(2):
            pos_d[:, (t * 2 + cg) * 96:(t * 2 + cg + 1) * 96] = \
                np.diag(pw[cg * 96:(cg + 1) * 96, dy + 1, dx + 1])
    w["pos_diag"] = bfw(pos_d)
    w["pos_b"] = f32(p["pos_b"].reshape(2, 96).T)  # [96, 2]

    g1v, b1v = p["ln1_g"], p["ln1_b"]
    qw = p["qkv_w"][:, :, 0, 0]  # [288, 96]
    qw_eff = qw * g1v[None, :96]
    w["qkv_wT"] = bfw(np.concatenate(
        [qw_eff[j * 96:(j + 1) * 96].T for j in range(3)], axis=1))  # [96, 3*96]
    w["qkv_bias"] = f32((qw @ b1v[:96]).reshape(3, 96).T)  # [96, 3]

    qdw = p["qkv_dw_w"][:, 0]  # [288,3,3]
    qdw_d = np.zeros((96, 27 * 96), np.float32)
    for t in range(9):
        dy, dx = t // 3 - 1, t % 3 - 1
        for j in range(3):
            qdw_d[:, (t * 3 + j) * 96:(t * 3 + j + 1) * 96] = \
                np.diag(qdw[j * 96:(j + 1) * 96, dy + 1, dx + 1])
    w["qdw_diag"] = bfw(qdw_d)
    f8w = lambda a: (np.ascontiguousarray(
        np.clip(np.asarray(a, np.float32), -240, 240).astype(FP8NP)), FP8)
    qdw8 = np.zeros((96, len(QPAIRS) * 3 * 192), np.float32)
    for pi, (ta, tb) in enumerate(QPAIRS):
        for j in range(3):
            o = (pi * 3 + j) * 192
            qdw8[:, o:o + 96] = np.diag(qdw[j * 96:(j + 1) * 96,
                                            ta // 3, ta % 3])
            if tb is not None:
                qdw8[:, o + 96:o + 192] = np.diag(qdw[j * 96:(j + 1) * 96,
                                                      tb // 3, tb % 3])
    w["qdw8"] = f8w(qdw8)

    gw1 = p["gate_w1"][:, :, 0, 0]  # [96, 192]
    gw1_eff = gw1 * g1v[None, :]
    w["gate_w1T"] = bfw(np.concatenate(
        [gw1_eff[:, cg * 96:(cg + 1) * 96].T for cg in range(2)], axis=1))  # [96, 192]
    w["gate_b1"] = f32((p["gate_b1"] + gw1 @ b1v).reshape(96, 1))
    w["gate_w2T"] = bfw(p["gate_w2"][:, :, 0, 0].T.copy())  # [96,1]
    w["gate_b2"] = f32(p["gate_b2"].reshape(1, 1))

    pj = p["proj_w"][:, :, 0, 0]
    pj1, pj2 = pj[:, :96], pj[:, 96:] * g1v[None, 96:]
    w["proj1T"] = bfw(np.concatenate(
        [pj1[cg * 96:(cg + 1) * 96].T for cg in range(2)], axis=1))  # [96, 192]
    w["proj2T"] = bfw(np.concatenate(
        [pj2[cg * 96:(cg + 1) * 96].T for cg in range(2)], axis=1))
    w["proj_bias"] = f32((pj[:, 96:] @ b1v[96:]).reshape(2, 96).T)  # [96, 2]

    attn_scale = float(p["attn1"][0] + p["attn2"][0] + p["attn3"][0] + p["attn4"][0])
    w["_attn_scale"] = (attn_scale, None)
    w["tempvec"] = f32(np.repeat(p["temperature"].reshape(HEADS), CPH).reshape(96, 1))

    g2v, b2v = p["ln2_g"], p["ln2_b"]
    f1 = p["fc1_w"][:, :, 0, 0]  # [256, 192]
    f1_eff = f1 * g2v[None, :]
    fc1 = np.zeros((96, 4 * 128), np.float32)
    for mg in range(2):
        for cg in range(2):
            fc1[:, (mg * 2 + cg) * 128:(mg * 2 + cg + 1) * 128] = \
                f1_eff[mg * 128:(mg + 1) * 128, cg * 96:(cg + 1) * 96].T
    w["fc1T"] = bfw(fc1)
    w["fc1_bias"] = f32((f1 @ b2v).reshape(2, 128).T)  # [128, 2]

    s1 = p["bn1_g"] / np.sqrt(p["bn1_v"] + eps_bn)
    t1 = p["bn1_b"] - p["bn1_m"] * s1
    s2 = p["bn2_g"] / np.sqrt(p["bn2_v"] + eps_bn)
    t2 = p["bn2_b"] - p["bn2_m"] * s2
    s3 = p["bn3_g"] / np.sqrt(p["bn3_v"] + eps_bn)
    t3 = p["bn3_b"] - p["bn3_m"] * s3

    dw1w, dw2w, dw3w = p["dw1_w"][:, 0], p["dw2_w"][:, 0], p["dw3_w"][:, 0]
    dw1b, dw2b, dw3b = p["dw1_b"], p["dw2_b"], p["dw3_b"]
    s1g = [s1[i * 64:(i + 1) * 64] for i in range(4)]
    t1g = [t1[i * 64:(i + 1) * 64] for i in range(4)]

    pair_d = np.zeros((128, 25 * 128), np.float32)
    for t in range(25):
        dy, dx = t // 5 - 2, t % 5 - 2
        blk = np.zeros((128, 128), np.float32)
        d2 = dw2w[:, dy + 2, dx + 2] * s1g[2]
        if dy == 0 and dx == 0:
            d2 = d2 + s1g[2]
        blk[64:, 64:] = np.diag(d2)
        if -1 <= dy <= 1 and -1 <= dx <= 1:
            d1 = dw1w[:, dy + 1, dx + 1] * s1g[1]
            if dy == 0 and dx == 0:
                d1 = d1 + s1g[1]
            blk[:64, :64] = np.diag(d1)
        pair_d[:, t * 128:(t + 1) * 128] = blk
    w["pair_diag"] = bfw(pair_d)
    f8w = lambda a: (np.ascontiguousarray(
        np.clip(np.asarray(a, np.float32), -240, 240).astype(FP8NP)), FP8)
    npair8 = 13
    pair8 = np.zeros((128, npair8 * 256), np.float32)
    for pi in range(npair8):
        ta, tb = 2 * pi, 2 * pi + 1
        pair8[:, pi * 256:pi * 256 + 128] = pair_d[:, ta * 128:(ta + 1) * 128]
        if tb < 25:
            pair8[:, pi * 256 + 128:pi * 256 + 256] = \
                pair_d[:, tb * 128:(tb + 1) * 128]
    w["pair8"] = f8w(pair8)
    bc1 = t1g[1] * dw1w.sum((1, 2)) + dw1b + t1g[1]
    bc2 = t1g[2] * dw2w.sum((1, 2)) + dw2b + t1g[2]
    w["pair_bias"] = f32(np.concatenate([bc1, bc2]).reshape(128, 1))

    # rows 64:128 of v0t2 hold the same data stored shifted +1, so a read at
    # AP offset (dy, dxa) yields tap (dy, dxa-1) for those rows.
    dw3_passes = []
    for dy in range(-3, 4):
        for dxa in (-2, 0, 2):
            dw3_passes.append((dy, dxa, True))
        dw3_passes.append((dy, 3, False))
    dw3_d = np.zeros((128, len(dw3_passes) * 64), np.float32)
    for i, (dy, dxa, hasb) in enumerate(dw3_passes):
        wa = dw3w[:, dy + 3, dxa + 3] * s1g[3]
        if dy == 0 and dxa == 0:
            wa = wa + s1g[3]
        dw3_d[:64, i * 64:(i + 1) * 64] = np.diag(wa)
        if hasb:
            wb = dw3w[:, dy + 3, dxa - 1 + 3] * s1g[3]
            if dy == 0 and dxa - 1 == 0:
                wb = wb + s1g[3]
            dw3_d[64:, i * 64:(i + 1) * 64] = np.diag(wb)
    w["dw3_diag"] = bfw(dw3_d)
    ndw38 = len(dw3_passes) // 2
    dw38 = np.zeros((128, ndw38 * 128), np.float32)
    for pi in range(ndw38):
        dw38[:, pi * 128:pi * 128 + 64] = dw3_d[:, (2 * pi) * 64:(2 * pi + 1) * 64]
        dw38[:, pi * 128 + 64:pi * 128 + 128] = \
            dw3_d[:, (2 * pi + 1) * 64:(2 * pi + 2) * 64]
    w["dw38"] = f8w(dw38)
    w["_dw3_passes"] = (dw3_passes, None)
    w["dw3_bias"] = f32((t1g[3] * dw3w.sum((1, 2)) + dw3b + t1g[3]).reshape(64, 1))

    d0w, d0b = p["dw0_w"][:, 0, 0, 0], p["dw0_b"]
    w["g0_scale"] = f32(((d0w + 1.0) * s1g[0]).reshape(64, 1))
    w["g0_bias"] = f32(((d0w + 1.0) * t1g[0] + d0b).reshape(64, 1))

    f2 = p["fc2_w"][:, :, 0, 0]  # [192, 256]
    f2a = f2 * s2[None, :]
    f2b = f2 * (t2 * s1)[None, :]
    cstv = f2 @ (t2 * t1)
    fc2a = np.zeros((128, 4 * 96), np.float32)
    for cg in range(2):
        for kg in range(2):
            fc2a[:, (cg * 2 + kg) * 96:(cg * 2 + kg + 1) * 96] = \
                f2a[cg * 96:(cg + 1) * 96, kg * 128:(kg + 1) * 128].T
    w["fc2aT"] = bfw(fc2a)
    w["_fc2b_zero"] = (bool(np.all(f2b == 0.0)), None)
    w["fc2bT_g0"] = bfw(np.concatenate(
        [f2b[cg * 96:(cg + 1) * 96, 0:64].T for cg in range(2)], axis=1))    # [64, 192]
    w["fc2bT_g12"] = bfw(np.concatenate(
        [f2b[cg * 96:(cg + 1) * 96, 64:192].T for cg in range(2)], axis=1))  # [128, 192]
    w["fc2bT_g3"] = bfw(np.concatenate(
        [f2b[cg * 96:(cg + 1) * 96, 192:256].T for cg in range(2)], axis=1))  # [64, 192]
    w["s3v"] = f32(np.stack([s3[:96], s3[96:]], axis=1))          # [96, 2]
    ob = s3 * cstv + t3
    w["out_bias"] = f32(np.stack([ob[:96], ob[96:]], axis=1))     # [96, 2]

    sg = np.where(s1 == 0, 1.0, s1)
    padv = -t1 / sg
    w["padv1"] = f32(np.concatenate([padv[64:128], padv[128:192]]).reshape(128, 1))
    w["padv2"] = f32(np.concatenate([padv[192:256], padv[192:256]]).reshape(128, 1))
    w["s1a"] = f32(s1[:128].reshape(128, 1))
    w["s1b"] = f32(s1[128:].reshape(128, 1))
    w["t1a"] = f32(t1[:128].reshape(128, 1))
    w["t1b"] = f32(t1[128:].reshape(128, 1))

    w["ones_st"] = bfw(np.full((96, 128), 1.0 / EMBED, np.float32))
    w["epsv"] = f32(np.full((128, 1), 1e-6, np.float32))
    vm = np.zeros((96, 96), np.float32)
    for h in range(HEADS):
        vm[h * CPH:(h + 1) * CPH, h * CPH:(h + 1) * CPH] = 1.0
    w["vmask"] = f32(vm)
    return w


WSPEC = {
    "ident": ([128, 128], F32), "identb": ([128, 128], BF16),
    "pos_diag": ([96, 18 * 96], BF16),
    "pos_b": ([96, 2], F32), "qkv_wT": ([96, 3 * 96], BF16),
    "qkv_bias": ([96, 3], F32), "qdw_diag": ([96, 27 * 96], BF16),
    "qdw8": ([96, 5 * 3 * 192], FP8), "pair8": ([128, 13 * 256], FP8),
    "dw38": ([128, 14 * 128], FP8),
    "gate_w1T": ([96, 192], BF16), "gate_b1": ([96, 1], F32),
    "gate_w2T": ([96, 1], BF16), "gate_b2": ([1, 1], F32),
    "proj1T": ([96, 192], BF16), "proj2T": ([96, 192], BF16),
    "proj_bias": ([96, 2], F32), "tempvec": ([96, 1], F32),
    "fc1T": ([96, 4 * 128], BF16), "fc1_bias": ([128, 2], F32),
    "pair_diag": ([128, 25 * 128], BF16), "pair_bias": ([128, 1], F32),
    "dw3_diag": ([128, 28 * 64], BF16), "dw3_bias": ([64, 1], F32),
    "g0_scale": ([64, 1], F32), "g0_bias": ([64, 1], F32),
    "fc2aT": ([128, 4 * 96], BF16), "fc2bT_g0": ([64, 192], BF16),
    "fc2bT_g12": ([128, 192], BF16), "fc2bT_g3": ([64, 192], BF16),
    "s3v": ([96, 2], F32), "out_bias": ([96, 2], F32),
    "padv1": ([128, 1], F32),
    "padv2": ([128, 1], F32),
    "s1a": ([128, 1], F32), "s1b": ([128, 1], F32),
    "t1a": ([128, 1], F32), "t1b": ([128, 1], F32),
    "ones_st": ([96, 128], BF16),
    "epsv": ([128, 1], F32),
    "vmask": ([96, 96], F32),
}


# ----------------------------------------------------------------------------
# device kernel
# ----------------------------------------------------------------------------

def build(nc, H, W, n_cores, attn_scale, dw3_passes, fc2b_zero=False):
    S = H * W
    Wp1 = W + 2
    P1B = (BR + 2) * Wp1 + 2 * SLOP   # band buffer (pad1)
    Wp3, Hp3 = W + 6, H + 6
    P3 = Hp3 * Wp3 + 2 * SLOP
    NCH = _ceil(H, RC)
    NB = _ceil(H, BR)
    NSC = _ceil(S, 512)
    GCH = 512 // W                    # gate chunk rows (512 cols)
    NGC_PER_BAND = _ceil(BR, GCH)

    # x and out are channel-major [EMBED, S]; host transposes NHWC<->CM
    x_t = nc.dram_tensor("x", [EMBED, S], BF16, kind="ExternalInput")
    out_t = nc.dram_tensor("out", [EMBED, S], F32, kind="ExternalOutput")
    wt = {k: nc.dram_tensor("w_" + k, shp, d, kind="ExternalInput")
          for k, (shp, d) in WSPEC.items()}

    def pd3(r):
        return SLOP + r * Wp3

    def dr_rhs(base2d, delta, n):
        # [P, 2, n] view with an overlapping middle dim of stride `delta`
        ap = [list(p) for p in base2d.ap]
        return bass.AP(base2d.tensor, base2d.offset,
                       [ap[0], [delta, 2], [1, n]])

    with tile.TileContext(nc) as tc:
        C_ONLY_W = ['pair8', 'dw38',
                    'fc1T', 'fc1_bias', 'pair_diag', 'pair_bias', 'dw3_diag',
                    'dw3_bias', 'g0_scale', 'g0_bias', 'fc2aT', 'fc2bT_g0',
                    'fc2bT_g12', 'fc2bT_g3', 's3v', 'out_bias', 's1a', 's1b',
                    't1a', 't1b', 'padv1', 'padv2']
        PERS_W = ['ones_st', 'epsv']
        with (
            tc.tile_pool(name="dram", bufs=1, space="DRAM") as dram,
            tc.tile_pool(name="persist", bufs=1) as pers,
        ):
            ws = {}

            def _load_w(pool, names):
                for k in names:
                    shp, d = WSPEC[k]
                    tl = pool.tile(shp, d, tag="w_" + k, name="w_" + k)
                    nc.sync.dma_start(out=tl[:], in_=wt[k][:])
                    ws[k] = tl

            yn1_sp = dram.tile([96, S], BF16)
            yn2_sp = dram.tile([96, S], BF16)
            xc_sp = [dram.tile([96, S], BF16, name=f"xc_sp{i}") for i in range(2)]
            xcp_sp = [dram.tile([96, S], BF16, name=f"xcp_sp{i}") for i in range(2)]
            zn_sp = [dram.tile([96, S], BF16, name=f"zn_sp{i}") for i in range(2)]
            cc_in = dram.tile([1, 1], F32)
            cc_out = dram.tile([1, 1], F32)

            gsum = pers.tile([1, NB * NGC_PER_BAND + 8], F32)
            nc.vector.memset(gsum[:], 0.0)
            dynk = pers.tile([96, 1], F32)
            probsT = pers.tile([96, 96], BF16)
            # persistent copies of LN helpers (used in phases A, B5 and C)
            ones_p = pers.tile([96, 128], BF16, name="p_ones")
            nc.sync.dma_start(out=ones_p[:], in_=wt["ones_st"][:])
            eps_p = pers.tile([128, 1], F32, name="p_eps")
            nc.sync.dma_start(out=eps_p[:], in_=wt["epsv"][:])
            # big memsets are pathologically slow; keep one zeroed band tile
            # and clear band buffers with fast engine copies instead
            zt = pers.tile([96, (BR + 2) * (W + 2) + 2 * SLOP], BF16, name="p_zero")
            nc.vector.memset(zt[:], 0.0)

            # ================= PHASE A =================
            _wpab_cm = tc.tile_pool(name="wpAB", bufs=1)
            wpab = _wpab_cm.__enter__()
            _load_w(wpab, [k for k in WSPEC
                           if k not in C_ONLY_W and k not in PERS_W])
            ident = ws["ident"]
            identb = ws["identb"]
            # v stays SBUF-resident through phase B5
            vres = wpab.tile([96, S], BF16, name="vres")
            with (
                tc.tile_pool(name="pa_band", bufs=2) as pab,
                tc.tile_pool(name="pa_rot", bufs=3) as par,
                tc.tile_pool(name="pa_ps", bufs=2, space="PSUM") as paps,
            ):
                for b in range(NB):
                    r0, r1 = b * BR, min((b + 1) * BR, H)
                    ylo, yhi = max(r0 - 1, 0), min(r1 + 1, H)
                    nr = yhi - ylo
                    boff = SLOP + (ylo - (r0 - 1)) * Wp1 + 1
                    xband = [pab.tile([96, P1B], BF16, tag=f"xb{cg}",
                                      name=f"xb{cg}") for cg in range(2)]
                    for cg in range(2):
                        # pad cells must be zero; buffers rotate with bufs=2 so
                        # zero each physical buffer once, then re-zero only the
                        # bottom halo row slot for the final band
                        if b < 2:
                            nc.scalar.copy(xband[cg][:], zt[:])
                        elif b == NB - 1:
                            ze = min(SLOP + (nr + 1) * Wp1 + SLOP, P1B)
                            zs = SLOP + nr * Wp1
                            nc.scalar.copy(xband[cg][:, zs:ze], zt[:, zs:ze])
                        dst = xband[cg][:, boff:boff + nr * Wp1] \
                            .rearrange("p (r w) -> p r w", w=Wp1)[:, :, 0:W]
                        src = x_t[cg * 96:(cg + 1) * 96, ylo * W:yhi * W] \
                            .rearrange("p (r w) -> p r w", w=W)
                        eng = nc.sync if cg == 0 else nc.scalar
                        eng.dma_start(out=dst, in_=src)
                    for c0 in range(r0, r1, RC):
                        nr_c = min(RC, H - c0)
                        N = nr_c * Wp1
                        NN = nr_c * W
                        sb0 = SLOP + (c0 - r0 + 1) * Wp1
                        xc_ch = [par.tile([96, RC * W], BF16, tag=f"xc{cg}",
                                          name=f"xc{cg}") for cg in range(2)]
                        xsq = [par.tile([96, RC * W], BF16, tag=f"xq{cg}",
                                        name=f"xq{cg}") for cg in range(2)]
                        for cg in range(2):
                            ps = paps.tile([96, RC * Wp1], F32, tag="posps")
                            for t in range(9):
                                dy, dx = t // 3 - 1, t % 3 - 1
                                o = sb0 + dy * Wp1 + dx
                                nc.tensor.matmul(
                                    ps[:, :N],
                                    ws["pos_diag"][:, (t * 2 + cg) * 96:(t * 2 + cg + 1) * 96],
                                    xband[cg][:, o:o + N],
                                    start=(t == 0), stop=(t == 8))
                            ps_int = ps[:, :N].rearrange("p (r w) -> p r w", w=Wp1)[:, :, 1:1 + W]
                            xb_int = xband[cg][:, sb0:sb0 + N] \
                                .rearrange("p (r w) -> p r w", w=Wp1)[:, :, 1:1 + W]
                            xcv = xc_ch[cg][:, :NN].rearrange("p (r w) -> p r w", w=W)
                            nc.vector.scalar_tensor_tensor(
                                out=xcv, in0=ps_int, scalar=ws["pos_b"][:, cg:cg + 1],
                                in1=xb_int, op0=Alu.add, op1=Alu.add)
                            nc.scalar.square(xsq[cg][:, :NN], xc_ch[cg][:, :NN])
                        mu_ps = paps.tile([128, RC * W], F32, tag="mups")
                        m2_ps = paps.tile([128, RC * W], F32, tag="m2ps")
                        for cg in range(2):
                            nc.tensor.matmul(mu_ps[:, :NN], ones_p, xc_ch[cg][:, :NN],
                                             start=(cg == 0), stop=(cg == 1))
                            nc.tensor.matmul(m2_ps[:, :NN], ones_p, xsq[cg][:, :NN],
                                             start=(cg == 0), stop=(cg == 1))
                        tmp = par.tile([128, RC * W], F32, tag="musq")
                        nc.scalar.square(tmp[:, :NN], mu_ps[:, :NN])
                        nc.vector.tensor_tensor(out=tmp[:, :NN], in0=m2_ps[:, :NN],
                                                in1=tmp[:, :NN], op=Alu.subtract)
                        nc.scalar.activation(tmp[:, :NN], tmp[:, :NN], Act.Sqrt,
                                             bias=eps_p)
                        rstd = par.tile([128, RC * W], F32, tag="rstd")
                        nc.vector.reciprocal(rstd[:, :NN], tmp[:, :NN])
                        for cg in range(2):
                            tdf = par.tile([96, RC * W], F32, tag=f"td{cg}")
                            nc.vector.tensor_tensor(out=tdf[:, :NN], in0=xc_ch[cg][:, :NN],
                                                    in1=mu_ps[:96, :NN], op=Alu.subtract)
                            ynch = par.tile([96, RC * W], BF16, tag=f"yn{cg}")
                            nc.vector.tensor_tensor(out=ynch[:, :NN], in0=tdf[:, :NN],
                                                    in1=rstd[:96, :NN], op=Alu.mult)
                            sp = yn1_sp if cg == 0 else yn2_sp
                            nc.sync.dma_start(out=sp[:, c0 * W:c0 * W + NN],
                                              in_=ynch[:, :NN])
                            nc.scalar.dma_start(out=xc_sp[cg][:, c0 * W:c0 * W + NN],
                                                in_=xc_ch[cg][:, :NN])

            # ================= PHASE B =================
            with (
                tc.tile_pool(name="pb_band", bufs=1) as pbb,
                tc.tile_pool(name="pb_rot", bufs=3) as pbr,
                tc.tile_pool(name="gram_ps", bufs=1, space="PSUM") as gpsp,
            ):
                g1_ps = gpsp.tile([96, 192], F32)
                g2_ps = gpsp.tile([96, 96], F32)
                with (
                    tc.tile_pool(name="pb_psg", bufs=1, space="PSUM") as pbpsg,
                    tc.tile_pool(name="pb_ps", bufs=2, space="PSUM") as pbps,
                ):
                    for b in range(NB):
                        r0, r1 = b * BR, min((b + 1) * BR, H)
                        ylo, yhi = max(r0 - 1, 0), min(r1 + 1, H)
                        ynb = [pbb.tile([96, (BR + 2) * W], BF16, tag=f"ynb{cg}",
                                        name=f"ynb{cg}") for cg in range(2)]
                        for cg in range(2):
                            sp = yn1_sp if cg == 0 else yn2_sp
                            eng = nc.sync if cg == 0 else nc.scalar
                            eng.dma_start(
                                out=ynb[cg][:, (ylo - r0 + 1) * W:(yhi - r0 + 1) * W],
                                in_=sp[:, ylo * W:yhi * W])
                        # gate (512-col chunks over rows [r0, r1))
                        for gi in range(NGC_PER_BAND):
                            gr0 = r0 + gi * GCH
                            if gr0 >= r1:
                                break
                            ngr = min(GCH, r1 - gr0)
                            NG = ngr * W
                            yo = (gr0 - r0 + 1) * W
                            gps = pbpsg.tile([96, 512], F32, tag="gps")
                            for cg in range(2):
                                nc.tensor.matmul(gps[:, :NG],
                                                 ws["gate_w1T"][:, cg * 96:(cg + 1) * 96],
                                                 ynb[cg][:, yo:yo + NG],
                                                 start=(cg == 0), stop=(cg == 1))
                            g1s = pbr.tile([96, 512], BF16, tag="g1s")
                            nc.scalar.activation(g1s[:, :NG], gps[:, :NG], Act.Relu,
                                                 bias=ws["gate_b1"])
                            g2ps = pbpsg.tile([96, 512], F32, tag="gps")
                            nc.tensor.matmul(g2ps[0:1, :NG], ws["gate_w2T"], g1s[:, :NG],
                                             start=True, stop=True)
                            sgt = pbr.tile([1, 512], F32, tag="sgt")
                            idx = b * NGC_PER_BAND + gi
                            nc.scalar.activation(sgt[:, :NG], g2ps[0:1, :NG], Act.Sigmoid,
                                                 bias=ws["gate_b2"],
                                                 accum_out=gsum[0:1, idx:idx + 1])
                        # qkv0 band
                        qkv0 = [pbb.tile([96, P1B], FP8 if FP8DW else BF16,
                                         tag=f"qk0{j}", name=f"qk0{j}")
                                for j in range(3)]
                        nrq = yhi - ylo
                        for j in range(3):
                            # single physical buffer (bufs=1): zero fully on the
                            # first band; re-zero only the stale bottom slots on
                            # the final band
                            if b == 0:
                                nc.scalar.copy(qkv0[j][:], zt[:])
                            elif b == NB - 1:
                                ze = min(SLOP + (nrq + 1) * Wp1 + SLOP, P1B)
                                zs = SLOP + nrq * Wp1
                                nc.scalar.copy(qkv0[j][:, zs:ze], zt[:, zs:ze])
                        for rr in range(ylo, yhi, 4):
                            nrw = min(4, yhi - rr)
                            NQ = nrw * W
                            for j in range(3):
                                qps = pbps.tile([96, 4 * W], F32, tag="qps")
                                nc.tensor.matmul(qps[:, :NQ],
                                                 ws["qkv_wT"][:, j * 96:(j + 1) * 96],
                                                 ynb[0][:, (rr - r0 + 1) * W:(rr - r0 + 1) * W + NQ],
                                                 start=True, stop=True)
                                dst = SLOP + (rr - r0 + 1) * Wp1 + 1
                                dview = qkv0[j][:, dst:dst + nrw * Wp1] \
                                    .rearrange("p (r w) -> p r w", w=Wp1)[:, :, 0:W]
                                nc.scalar.activation(
                                    dview, qps[:, :NQ].rearrange("p (r w) -> p r w", w=W),
                                    Act.Identity, bias=ws["qkv_bias"][:, j:j + 1])
                        # depthwise; q/k transposed into qkband via PE
                        qkband = pbr.tile([W, BR * 192], BF16, tag="qkband")
                        for c0 in range(r0, r1, RC):
                            nr_c = min(RC, H - c0)
                            N = nr_c * Wp1
                            NN = nr_c * W
                            sb0 = SLOP + (c0 - r0 + 1) * Wp1
                            qk = {}
                            for j in range(3):
                                ps = pbps.tile([96, RC * Wp1], F32, tag="dwps")
                                if FP8DW:
                                    toff = lambda t: (t // 3 - 1) * Wp1 + (t % 3 - 1)
                                    for pi, (ta, tb) in enumerate(QPAIRS):
                                        oa = sb0 + toff(ta)
                                        delta = (toff(tb) - toff(ta)) \
                                            if tb is not None else 1
                                        lhsT = ws["qdw8"][:, (pi * 3 + j) * 192:
                                                          (pi * 3 + j + 1) * 192] \
                                            .rearrange("p (two m) -> p two m", two=2)
                                        nc.tensor.matmul(
                                            ps[:, :N], lhsT,
                                            dr_rhs(qkv0[j][:, oa:oa + N], delta, N),
                                            start=(pi == 0),
                                            stop=(pi == len(QPAIRS) - 1),
                                            perf_mode=DR)
                                else:
                                    for t in range(9):
                                        dy, dx = t // 3 - 1, t % 3 - 1
                                        o = sb0 + dy * Wp1 + dx
                                        nc.tensor.matmul(
                                            ps[:, :N],
                                            ws["qdw_diag"][:, (t * 3 + j) * 96:(t * 3 + j + 1) * 96],
                                            qkv0[j][:, o:o + N],
                                            start=(t == 0), stop=(t == 8))
                                ps_int = ps[:, :N].rearrange("p (r w) -> p r w", w=Wp1)[:, :, 1:1 + W]
                                if j == 2:
                                    nc.scalar.copy(
                                        vres[:, c0 * W:c0 * W + NN]
                                        .rearrange("p (r w) -> p r w", w=W), ps_int)
                                else:
                                    qb = pbr.tile([96, RC * W], BF16, tag=f"qb{j}")
                                    nc.scalar.copy(
                                        qb[:, :NN].rearrange("p (r w) -> p r w", w=W),
                                        ps_int)
                                    qk[j] = qb
                            for rr in range(c0, c0 + nr_c):
                                ro = (rr - r0) * 192
                                rl = (rr - c0) * W
                                for j in range(2):
                                    tps = pbpsg.tile([128, 96], BF16, tag="tps")
                                    nc.tensor.transpose(tps[:], qk[j][:, rl:rl + W],
                                                        identb[:96, :96])
                                    nc.scalar.copy(
                                        qkband[:, ro + j * 96:ro + (j + 1) * 96],
                                        tps[:])
                        for rr in range(r0, r1):
                            ro = (rr - r0) * 192
                            nc.tensor.matmul(g1_ps[:], qkband[:, ro:ro + 96],
                                             qkband[:, ro:ro + 192],
                                             start=(rr == 0), stop=(rr == H - 1))
                            nc.tensor.matmul(g2_ps[:], qkband[:, ro + 96:ro + 192],
                                             qkband[:, ro + 96:ro + 192],
                                             start=(rr == 0), stop=(rr == H - 1))

                # ---- gate mean -> AllReduce -> dynk ----
                gred = pers.tile([1, 1], F32)
                nc.vector.reduce_sum(gred[:], gsum[0:1, 0:NB * NGC_PER_BAND], axis=AX)
                gsc = pers.tile([1, 1], F32)
                nc.vector.tensor_scalar_mul(gsc[:], gred[:], float(CPH) / (n_cores * S))
                nc.sync.dma_start(out=cc_in[:], in_=gsc[:])
                nc.gpsimd.collective_compute(
                    "AllReduce", Alu.add, replica_groups=[list(range(n_cores))],
                    ins=[cc_in.opt()], outs=[cc_out.opt()])
                nc.sync.dma_start(out=dynk[:], in_=cc_out[:].partition_broadcast(96))

                # ---- attn block ----
                with (
                    tc.tile_pool(name="at_ps", bufs=2, space="PSUM") as atps,
                    tc.tile_pool(name="at_sb", bufs=1) as ab,
                ):
                    g1sb = ab.tile([96, 192], F32)
                    nc.scalar.copy(g1sb[:], g1_ps[:])
                    g2sb = ab.tile([96, 96], F32)
                    nc.scalar.copy(g2sb[:], g2_ps[:])
                    idm = ident[:96, :96]
                    tq = ab.tile([96, 96], F32)
                    nc.vector.tensor_tensor(out=tq[:], in0=g1sb[:, 0:96], in1=idm,
                                            op=Alu.mult)
                    nq2 = ab.tile([96, 1], F32)
                    nc.vector.reduce_sum(nq2[:], tq[:], axis=AX)
                    tk = ab.tile([96, 96], F32)
                    nc.vector.tensor_tensor(out=tk[:], in0=g2sb[:], in1=idm,
                                            op=Alu.mult)
                    nk2 = ab.tile([96, 1], F32)
                    nc.vector.reduce_sum(nk2[:], tk[:], axis=AX)

                    def rsqrt_clamped(nm, src):
                        sq = ab.tile([96, 1], F32, tag=nm + "sq")
                        nc.scalar.sqrt(sq[:], src[:])
                        cl = ab.tile([96, 1], F32, tag=nm + "cl")
                        nc.vector.tensor_scalar_max(cl[:], sq[:], 1e-12)
                        rvv = ab.tile([96, 1], F32, tag=nm)
                        nc.vector.reciprocal(rvv[:], cl[:])
                        return rvv

                    rq = rsqrt_clamped("rq", nq2)
                    rk = rsqrt_clamped("rk", nk2)
                    rqt = ab.tile([96, 1], F32)
                    nc.vector.tensor_tensor(out=rqt[:], in0=rq[:], in1=ws["tempvec"][:],
                                            op=Alu.mult)
                    asr = ab.tile([96, 96], F32)
                    nc.vector.tensor_scalar_mul(asr[:], g1sb[:, 96:192], rqt[:])
                    as_ps = atps.tile([96, 96], F32, tag="atp")
                    nc.tensor.transpose(as_ps[:], asr[:], ident[:96, :96])
                    ast = ab.tile([96, 96], F32)
                    nc.vector.tensor_scalar_mul(ast[:], as_ps[:], rk[:])
                    as2_ps = atps.tile([96, 96], F32, tag="atp")
                    nc.tensor.transpose(as2_ps[:], ast[:], ident[:96, :96])
                    as2 = ab.tile([96, 96], F32)
                    nc.scalar.copy(as2[:], as2_ps[:])
                    # mask off-head-block entries to -60
                    t60 = ab.tile([96, 96], F32)
                    nc.vector.tensor_scalar_add(t60[:], as2[:], 60.0)
                    amf = ab.tile([96, 96], F32)
                    nc.vector.tensor_tensor(out=amf[:], in0=t60[:], in1=ws["vmask"][:],
                                            op=Alu.mult)
                    nc.vector.tensor_scalar_add(amf[:], amf[:], -60.0)
                    # rank+1 over full row via pairwise is_ge
                    rnk3 = ab.tile([96, 96 * 96], F32)
                    a_i = amf[:].unsqueeze(1).broadcast_to([96, 96, 96])
                    a_d = amf[:].unsqueeze(2).broadcast_to([96, 96, 96])
                    rvw = rnk3[:].rearrange("p (i d) -> p i d", d=96)
                    nc.vector.tensor_tensor(out=rvw, in0=a_i, in1=a_d, op=Alu.is_ge)
                    rank1 = ab.tile([96, 96], F32)
                    nc.vector.reduce_sum(rank1[:].unsqueeze(2), rvw, axis=AX)
                    sel = ab.tile([96, 96], F32)
                    nc.vector.tensor_tensor(out=sel[:], in0=rank1[:],
                                            in1=dynk[:].broadcast_to([96, 96]), op=Alu.is_le)
                    am = ab.tile([96, 96], F32)
                    t60b = ab.tile([96, 96], F32)
                    nc.vector.tensor_scalar_add(t60b[:], amf[:], 60.0)
                    nc.vector.tensor_tensor(out=am[:], in0=t60b[:], in1=sel[:], op=Alu.mult)
                    nc.vector.tensor_scalar_add(am[:], am[:], -60.0)
                    mx = ab.tile([96, 1], F32)
                    nc.vector.reduce_max(mx[:], am[:], axis=AX)
                    nmx = ab.tile([96, 1], F32)
                    nc.vector.tensor_scalar_mul(nmx[:], mx[:], -1.0)
                    ex = ab.tile([96, 96], F32)
                    nc.scalar.activation(ex[:], am[:], Act.Exp, bias=nmx[:])
                    sme = ab.tile([96, 1], F32)
                    nc.vector.reduce_sum(sme[:], ex[:], axis=AX)
                    rsm = ab.tile([96, 1], F32)
                    nc.vector.reciprocal(rsm[:], sme[:])
                    probs = ab.tile([96, 96], F32)
                    nc.vector.tensor_scalar_mul(probs[:], ex[:], rsm[:])
                    pt_ps = atps.tile([96, 96], F32, tag="atp2")
                    nc.tensor.transpose(pt_ps[:], probs[:], ident[:96, :96])
                    nc.scalar.copy(probsT[:], pt_ps[:])

            # ================= PHASE B5 =================
            with (
                tc.tile_pool(name="b5_rot", bufs=3) as b5r,
                tc.tile_pool(name="b5_ps", bufs=1, space="PSUM") as b5ps,
            ):
                for ci in range(NSC):
                    o0 = ci * 512
                    NN = min(512, S - o0)
                    av_ps = b5ps.tile([96, 512], F32, tag="avps")
                    nc.tensor.matmul(av_ps[:, :NN], probsT[:], vres[:, o0:o0 + NN],
                                     start=True, stop=True)
                    avs = b5r.tile([96, 512], BF16, tag="avs")
                    nc.scalar.activation(avs[:, :NN], av_ps[:, :NN], Act.Copy,
                                         scale=attn_scale)
                    x2ch = b5r.tile([96, 512], BF16, tag="x2ch")
                    nc.sync.dma_start(out=x2ch[:, :NN], in_=yn2_sp[:, o0:o0 + NN])
                    xpch = [b5r.tile([96, 512], BF16, tag=f"xp{cg}", name=f"xp{cg}")
                            for cg in range(2)]
                    xsq = [b5r.tile([96, 512], BF16, tag=f"xs{cg}", name=f"xs{cg}")
                           for cg in range(2)]
                    for cg in range(2):
                        xcch = b5r.tile([96, 512], BF16, tag=f"xcc{cg}")
                        nc.scalar.dma_start(out=xcch[:, :NN], in_=xc_sp[cg][:, o0:o0 + NN])
                        pj_ps = b5ps.tile([96, 512], F32, tag=f"pjps{cg}")
                        nc.tensor.matmul(pj_ps[:, :NN],
                                         ws["proj1T"][:, cg * 96:(cg + 1) * 96],
                                         avs[:, :NN], start=True, stop=False)
                        nc.tensor.matmul(pj_ps[:, :NN],
                                         ws["proj2T"][:, cg * 96:(cg + 1) * 96],
                                         x2ch[:, :NN], start=False, stop=True)
                        nc.vector.scalar_tensor_tensor(
                            out=xpch[cg][:, :NN], in0=pj_ps[:, :NN],
                            scalar=ws["proj_bias"][:, cg:cg + 1], in1=xcch[:, :NN],
                            op0=Alu.add, op1=Alu.add)
                        nc.sync.dma_start(out=xcp_sp[cg][:, o0:o0 + NN],
                                          in_=xpch[cg][:, :NN])
                        nc.scalar.square(xsq[cg][:, :NN], xpch[cg][:, :NN])
                    mu_ps = b5ps.tile([128, 512], F32, tag="mu2ps")
                    m2_ps = b5ps.tile([128, 512], F32, tag="m22ps")
                    for cg in range(2):
                        nc.tensor.matmul(mu_ps[:, :NN], ones_p, xpch[cg][:, :NN],
                                         start=(cg == 0), stop=(cg == 1))
                        nc.tensor.matmul(m2_ps[:, :NN], ones_p, xsq[cg][:, :NN],
                                         start=(cg == 0), stop=(cg == 1))
                    tmp = b5r.tile([128, 512], F32, tag="musq2")
                    nc.scalar.square(tmp[:, :NN], mu_ps[:, :NN])
                    nc.vector.tensor_tensor(out=tmp[:, :NN], in0=m2_ps[:, :NN],
                                            in1=tmp[:, :NN], op=Alu.subtract)
                    nc.scalar.activation(tmp[:, :NN], tmp[:, :NN], Act.Sqrt,
                                         bias=eps_p)
                    rstd = b5r.tile([128, 512], F32, tag="rstd2")
                    nc.vector.reciprocal(rstd[:, :NN], tmp[:, :NN])
                    for cg in range(2):
                        td2 = b5r.tile([96, 512], F32, tag=f"td2{cg}")
                        nc.vector.tensor_tensor(out=td2[:, :NN], in0=xpch[cg][:, :NN],
                                                in1=mu_ps[:96, :NN], op=Alu.subtract)
                        znt = b5r.tile([96, 512], BF16, tag=f"znt{cg}")
                        nc.vector.tensor_tensor(out=znt[:, :NN], in0=td2[:, :NN],
                                                in1=rstd[:96, :NN], op=Alu.mult)
                        nc.scalar.dma_start(out=zn_sp[cg][:, o0:o0 + NN],
                                            in_=znt[:, :NN])

            _wpab_cm.__exit__(None, None, None)
            # ================= PHASE C =================
            _wpc_cm = tc.tile_pool(name="wpC", bufs=1)
            wpc = _wpc_cm.__enter__()
            _load_w(wpc, C_ONLY_W)
            with tc.tile_pool(name="c_v0", bufs=1) as cv0:
                v0t1 = cv0.tile([128, P3], BF16)
                v0t2 = cv0.tile([128, P3], BF16)
                vgug = cv0.tile([128, P3], BF16)   # rows 0:64 = v-gelu0, 64:128 = u-gelu0
                with (
                    tc.tile_pool(name="c1_rot", bufs=2) as c1r,
                    tc.tile_pool(name="c1_ps", bufs=2, space="PSUM") as c1ps,
                ):
                    # pad cells must hold -t1/s1 so the bn-folded depthwise
                    # reads zeros in v0_bn space at image borders. Only the pad
                    # regions need initialization (interior is overwritten):
                    # top rows, bottom rows, and left/right columns per row.
                    def _pad_init(tl, padw, lcols, rcols):
                        for a, bnd in ((0, pd3(3)), (pd3(H + 3), P3)):
                            nc.vector.memset(tl[:, a:bnd], 0.0)
                            nc.vector.tensor_scalar_add(tl[:, a:bnd], tl[:, a:bnd],
                                                        padw)
                        for (p0, p1, c0_, c1_) in (lcols + rcols):
                            vv = tl[p0:p1, pd3(3):pd3(3 + H)] \
                                .rearrange("p (r w) -> p r w", w=Wp3)[:, :, c0_:c1_]
                            nc.vector.memset(vv, 0.0)
                            nc.vector.tensor_scalar_add(vv, vv, padw[p0:p1])

                    _pad_init(v0t1, ws["padv1"],
                              [(0, 128, 0, 3)], [(0, 128, 3 + W, Wp3)])
                    # v0t2 rows 64:128 are stored shifted +1 (interior written
                    # at cols [4, 4+W)), so their pads are cols [0,4) and
                    # [4+W, Wp3)
                    _pad_init(v0t2, ws["padv2"],
                              [(0, 64, 0, 3), (64, 128, 0, 4)],
                              [(0, 64, 3 + W, Wp3), (64, 128, 4 + W, Wp3)])
                    for ci in range(NCH):
                        c0 = ci * RC
                        nr_c = min(RC, H - c0)
                        NN = nr_c * W
                        o0 = c0 * W
                        d0 = pd3(3 + c0) + 3

                        def v0view(tl, lo, hi, shift=0):
                            return tl[lo:hi, d0 + shift:d0 + shift + nr_c * Wp3] \
                                .rearrange("p (r w) -> p r w", w=Wp3)[:, :, 0:W]

                        znch = [c1r.tile([96, RC * W], BF16, tag=f"cz{cg}",
                                         name=f"cz{cg}") for cg in range(2)]
                        for cg in range(2):
                            eng = nc.sync if cg == 0 else nc.scalar
                            eng.dma_start(out=znch[cg][:, :NN],
                                          in_=zn_sp[cg][:, o0:o0 + NN])
                        for mg in range(2):
                            fps = c1ps.tile([128, RC * W], F32, tag="fps")
                            for cg in range(2):
                                nc.tensor.matmul(
                                    fps[:, :NN],
                                    ws["fc1T"][:, (mg * 2 + cg) * 128:(mg * 2 + cg + 1) * 128],
                                    znch[cg][:, :NN], start=(cg == 0), stop=(cg == 1))
                            fv = lambda lo, hi: fps[lo:hi, :NN] \
                                .rearrange("p (r w) -> p r w", w=W)
                            if mg == 0:
                                nc.scalar.activation(
                                    v0view(vgug, 0, 64), fv(0, 64), Act.Gelu,
                                    bias=ws["fc1_bias"][0:64, 0:1])
                                nc.scalar.activation(
                                    v0view(vgug, 64, 128), v0view(vgug, 0, 64),
                                    Act.Gelu, bias=ws["g0_bias"], scale=ws["g0_scale"])
                                nc.scalar.activation(
                                    v0view(v0t1, 0, 64), fv(64, 128), Act.Gelu,
                                    bias=ws["fc1_bias"][64:128, 0:1])
                            else:
                                nc.scalar.activation(
                                    v0view(v0t1, 64, 128), fv(0, 64), Act.Gelu,
                                    bias=ws["fc1_bias"][0:64, 1:2])
                                nc.scalar.activation(
                                    v0view(v0t2, 0, 64), fv(64, 128), Act.Gelu,
                                    bias=ws["fc1_bias"][64:128, 1:2])
                                nc.scalar.activation(
                                    v0view(v0t2, 64, 128, shift=1), fv(64, 128),
                                    Act.Gelu, bias=ws["fc1_bias"][64:128, 1:2])

                if FP8DW_C:
                    v0t18 = cv0.tile([128, P3], FP8, name="v0t18")
                    v0t28 = cv0.tile([128, P3], FP8, name="v0t28")
                    nc.scalar.copy(v0t18[:], v0t1[:])
                    nc.vector.tensor_copy(out=v0t28[:], in_=v0t2[:])
                with (
                    tc.tile_pool(name="c2_rot", bufs=2) as c2r,
                    tc.tile_pool(name="c2_ps", bufs=2, space="PSUM") as c2ps,
                ):
                    for ci in range(NCH):
                        c0 = ci * RC
                        nr_c = min(RC, H - c0)
                        N = nr_c * Wp3
                        NN = nr_c * W
                        sb0 = pd3(3 + c0)
                        ps_a = c2ps.tile([128, RC * Wp3], F32, tag="psa")
                        if FP8DW_C:
                            poff = lambda t: (t // 5 - 2) * Wp3 + (t % 5 - 2)
                            for pi in range(13):
                                ta, tb = 2 * pi, 2 * pi + 1
                                oa = sb0 + poff(ta)
                                delta = (poff(tb) - poff(ta)) if tb < 25 else 1
                                lhsT = ws["pair8"][:, pi * 256:(pi + 1) * 256] \
                                    .rearrange("p (two m) -> p two m", two=2)
                                nc.tensor.matmul(
                                    ps_a[:, :N], lhsT,
                                    dr_rhs(v0t18[:, oa:oa + N], delta, N),
                                    start=(pi == 0), stop=(pi == 12),
                                    perf_mode=DR)
                        else:
                            for t in range(25):
                                dy, dx = t // 5 - 2, t % 5 - 2
                                o = sb0 + dy * Wp3 + dx
                                nc.tensor.matmul(ps_a[:, :N],
                                                 ws["pair_diag"][:, t * 128:(t + 1) * 128],
                                                 v0t1[:, o:o + N],
                                                 start=(t == 0), stop=(t == 24))
                        ps_b = c2ps.tile([64, RC * Wp3], F32, tag="psb")
                        if FP8DW_C:
                            nmeta = len(dw3_passes) // 2
                            for pi in range(nmeta):
                                (dya, dxaa, _), (dyb, dxab, _) = \
                                    dw3_passes[2 * pi], dw3_passes[2 * pi + 1]
                                oa = sb0 + dya * Wp3 + dxaa
                                delta = (dyb * Wp3 + dxab) - (dya * Wp3 + dxaa)
                                lhsT = ws["dw38"][:, pi * 128:(pi + 1) * 128] \
                                    .rearrange("p (two m) -> p two m", two=2)
                                nc.tensor.matmul(
                                    ps_b[:, :N], lhsT,
                                    dr_rhs(v0t28[:, oa:oa + N], delta, N),
                                    start=(pi == 0), stop=(pi == nmeta - 1),
                                    perf_mode=DR)
                        else:
                            for i, (dy, dxa, hasb) in enumerate(dw3_passes):
                                o = sb0 + dy * Wp3 + dxa
                                nc.tensor.matmul(ps_b[:, :N],
                                                 ws["dw3_diag"][:, i * 64:(i + 1) * 64],
                                                 v0t2[:, o:o + N],
                                                 start=(i == 0), stop=(i == len(dw3_passes) - 1))

                        def inner(ap_flat, lo, hi):
                            # interior view of a PSUM chunk (starts at free 0)
                            return ap_flat[lo:hi, :N].rearrange(
                                "p (r w) -> p r w", w=Wp3)[:, :, 3:3 + W]

                        def inner_v0(tl, lo, hi):
                            # interior view of the padded v0 buffers at this chunk
                            return tl[lo:hi, sb0:sb0 + N].rearrange(
                                "p (r w) -> p r w", w=Wp3)[:, :, 3:3 + W]

                        ug_a = c2r.tile([128, RC * W], BF16, tag="uga")
                        ug_b = c2r.tile([128, RC * W], BF16, tag="ugb")
                        vb_a = c2r.tile([128, RC * W], BF16, tag="vba")
                        vb_b = c2r.tile([128, RC * W], BF16, tag="vbb")
                        uv = lambda tl, lo, hi: tl[lo:hi, :NN] \
                            .rearrange("p (r w) -> p r w", w=W)
                        nc.scalar.copy(uv(ug_a, 0, 64), inner_v0(vgug, 64, 128))
                        nc.scalar.activation(uv(ug_a, 64, 128), inner(ps_a, 0, 64),
                                             Act.Gelu, bias=ws["pair_bias"][0:64])
                        nc.scalar.activation(uv(ug_b, 0, 64), inner(ps_a, 64, 128),
                                             Act.Gelu, bias=ws["pair_bias"][64:128])
                        nc.scalar.activation(uv(ug_b, 64, 128), inner(ps_b, 0, 64),
                                             Act.Gelu, bias=ws["dw3_bias"])
                        nc.gpsimd.tensor_scalar(out=uv(vb_a, 0, 64),
                                                in0=inner_v0(vgug, 0, 64),
                                                scalar1=ws["s1a"][0:64],
                                                scalar2=ws["t1a"][0:64],
                                                op0=Alu.mult, op1=Alu.add)
                        nc.gpsimd.tensor_scalar(out=uv(vb_a, 64, 128),
                                                in0=inner_v0(v0t1, 0, 64),
                                                scalar1=ws["s1a"][64:128],
                                                scalar2=ws["t1a"][64:128],
                                                op0=Alu.mult, op1=Alu.add)
                        nc.gpsimd.tensor_scalar(out=uv(vb_b, 0, 64),
                                                in0=inner_v0(v0t1, 64, 128),
                                                scalar1=ws["s1b"][0:64],
                                                scalar2=ws["t1b"][0:64],
                                                op0=Alu.mult, op1=Alu.add)
                        nc.gpsimd.tensor_scalar(out=uv(vb_b, 64, 128),
                                                in0=inner_v0(v0t2, 0, 64),
                                                scalar1=ws["s1b"][64:128],
                                                scalar2=ws["t1b"][64:128],
                                                op0=Alu.mult, op1=Alu.add)
                        z1a = c2r.tile([128, RC * W], BF16, tag="z1a")
                        z1b = c2r.tile([128, RC * W], BF16, tag="z1b")
                        nc.gpsimd.tensor_tensor(out=z1a[:, :NN], in0=ug_a[:, :NN],
                                                in1=vb_a[:, :NN], op=Alu.mult)
                        nc.vector.tensor_tensor(out=z1b[:, :NN], in0=ug_b[:, :NN],
                                                in1=vb_b[:, :NN], op=Alu.mult)
                        for cg in range(2):
                            ops = c2ps.tile([96, RC * W], F32, tag=f"ops{cg}")
                            nc.tensor.matmul(ops[:, :NN],
                                             ws["fc2aT"][:, (cg * 2) * 96:(cg * 2 + 1) * 96],
                                             z1a[:, :NN], start=True, stop=False)
                            nc.tensor.matmul(ops[:, :NN],
                                             ws["fc2aT"][:, (cg * 2 + 1) * 96:(cg * 2 + 2) * 96],
                                             z1b[:, :NN], start=False, stop=fc2b_zero)
                            if not fc2b_zero:
                                opsv = ops[:, :NN].rearrange("p (r w) -> p r w", w=W)
                                nc.tensor.matmul(opsv,
                                                 ws["fc2bT_g0"][:, cg * 96:(cg + 1) * 96],
                                                 inner_v0(vgug, 0, 64), start=False, stop=False)
                                nc.tensor.matmul(opsv,
                                                 ws["fc2bT_g12"][:, cg * 96:(cg + 1) * 96],
                                                 inner_v0(v0t1, 0, 128), start=False, stop=False)
                                nc.tensor.matmul(opsv,
                                                 ws["fc2bT_g3"][:, cg * 96:(cg + 1) * 96],
                                                 inner_v0(v0t2, 0, 64), start=False, stop=True)
                            xrch = c2r.tile([96, RC * W], BF16, tag=f"xr{cg}", bufs=1)
                            eng = nc.sync if cg == 0 else nc.scalar
                            eng.dma_start(out=xrch[:, :NN],
                                          in_=xcp_sp[cg][:, c0 * W:c0 * W + NN])
                            ob = c2r.tile([96, RC * W], F32, tag=f"ob{cg}", bufs=1)
                            nc.vector.tensor_scalar(out=ob[:, :NN], in0=ops[:, :NN],
                                                    scalar1=ws["s3v"][:, cg:cg + 1],
                                                    scalar2=ws["out_bias"][:, cg:cg + 1],
                                                    op0=Alu.mult, op1=Alu.add)
                            oc = c2r.tile([96, RC * W], F32, tag=f"oc{cg}", bufs=1)
                            nc.gpsimd.tensor_tensor(out=oc[:, :NN], in0=ob[:, :NN],
                                                    in1=xrch[:, :NN], op=Alu.add)
                            eng2 = nc.sync if cg == 0 else nc.scalar
                            eng2.dma_start(
                                out=out_t[cg * 96:(cg + 1) * 96,
                                          c0 * W:c0 * W + NN],
                                in_=oc[:, :NN])
            _wpc_cm.__exit__(None, None, None)
    return out_t.name


# ----------------------------------------------------------------------------
# host entry
# ----------------------------------------------------------------------------

_CACHE = {}
_FC2B_ZERO = {}


def make_program(H, W, n_cores, attn_scale, dw3_passes, fc2b_zero=None):
    if fc2b_zero is None:
        fc2b_zero = _FC2B_ZERO.get("v", False)
    key = (H, W, n_cores, round(attn_scale, 9), fc2b_zero)
    if key in _CACHE:
        return _CACHE[key]
    nc = bacc.Bacc("TRN2", target_bir_lowering=False, debug=False, num_devices=n_cores)
    out_name = build(nc, H, W, n_cores, attn_scale, dw3_passes, fc2b_zero=fc2b_zero)
    nc.compile()
    _CACHE[key] = (nc, out_name)
    return nc, out_name


def make_in_maps(inputs):
    x = np.asarray(inputs["x"], np.float32)
    B = x.shape[0]
    C = x.shape[-1]
    wdict = _prep_weights({k: np.asarray(v) for k, v in inputs.items()})
    _FC2B_ZERO["v"] = wdict["_fc2b_zero"][0]
    base = {}
    for k, (shp, d) in WSPEC.items():
        base["w_" + k] = wdict[k][0].reshape(shp)
    in_maps = []
    for b in range(B):
        m = dict(base)
        # channel-major [C, H*W] on device, bf16
        m["x"] = np.ascontiguousarray(x[b].reshape(-1, C).T).astype(BF16NP)
        in_maps.append(m)
    return in_maps, wdict


def kernel(**inputs):
    x = np.asarray(inputs["x"], np.float32)
    B, H, W, C = x.shape
    in_maps, wdict = make_in_maps(inputs)
    nc, out_name = make_program(H, W, B, wdict["_attn_scale"][0],
                                wdict["_dw3_passes"][0])
    res = bass_utils.run_bass_kernel_spmd(nc, in_maps, core_ids=list(range(B)))
    return np.stack([np.asarray(res.results[b][out_name], np.float32)
                     .reshape(C, H * W).T.reshape(H, W, C) for b in range(B)])


# revision 31
# speedup vs baseline: 1.2012x; 1.2012x over previous
"""Trainium2 Bass kernel for nn_Block_87351044866235 (sparse_attention).

Data-parallel over batch: 8 samples -> 8 NeuronCores. Channel-major
layout [C, H*W] on chip; depthwise convs as diagonal bf16 matmuls on
TensorE; 1x1 convs as bf16 matmuls; LN stats via ones-matmuls; q/k gram
via PE transposes + bf16 matmuls; dynamic-k gate mean via a scalar
AllReduce. Activation spills are bf16; v stays SBUF-resident.
"""
import sys, os

for _p in ("/opt/trn_rl_repo", "/root/.axon_site/_ro/trn_rl_repo"):
    if os.path.isdir(_p) and _p not in sys.path:
        sys.path.append(_p)

import numpy as np
import ml_dtypes
import concourse.bass as bass
import concourse.bacc as bacc
import concourse.tile as tile
from concourse import mybir
from concourse import bass_utils

try:
    from concourse import tile_utils as _tu
    _tu.max_sbuf_usage = 208 * 1024
except Exception:
    pass

dt = mybir.dt
Alu = mybir.AluOpType
Act = mybir.ActivationFunctionType
AX = mybir.AxisListType.X

EMBED, PDIM, HEADS, HID = 192, 96, 8, 256
CPH = PDIM // HEADS  # 12
SLOP = 8
RC = 3    # conv output rows per chunk
BR = 12   # rows per band

F32, F32R, BF16 = dt.float32, dt.float32r, dt.bfloat16
FP8 = dt.float8e4
BF16NP = ml_dtypes.bfloat16
FP8NP = ml_dtypes.float8_e4m3
FP8DW = False      # fp8 DoubleRow for the qkv_dw depthwise conv
FP8DW_C = False    # fp8 DoubleRow for the pair/dw3 FFN depthwise convs
DR = mybir.MatmulPerfMode.DoubleRow

# tap pairs for 3x3 depthwise as fp8 DoubleRow (2 taps per pass)
QPAIRS = [(0, 1), (2, 3), (4, 5), (6, 7), (8, None)]


def _ceil(a, b):
    return (a + b - 1) // b


# ----------------------------------------------------------------------------
# host-side weight prep: everything 2D [partitions, free]
# ----------------------------------------------------------------------------

def _prep_weights(p):
    w = {}
    f32 = lambda a: (np.ascontiguousarray(a, np.float32), F32)
    bfw = lambda a: (np.ascontiguousarray(np.asarray(a, np.float32)
                                          .astype(BF16NP)), BF16)
    eps_bn = 1e-5

    w["ident"] = f32(np.eye(128, dtype=np.float32))
    w["identb"] = bfw(np.eye(128, dtype=np.float32))

    # pos depthwise diag: [96, (t*2+cg)*96]
    pw = p["pos_w"][:, 0]  # [192,3,3]
    pos_d = np.zeros((96, 18 * 96), np.float32)
    for t in range(9):
        dy, dx = t // 3 - 1, t % 3 - 1
        for cg in range(2):
            pos_d[:, (t * 2 + cg) * 96:(t * 2 + cg + 1) * 96] = \
                np.diag(pw[cg * 96:(cg + 1) * 96, dy + 1, dx + 1])
    w["pos_diag"] = bfw(pos_d)
    w["pos_b"] = f32(p["pos_b"].reshape(2, 96).T)  # [96, 2]

    g1v, b1v = p["ln1_g"], p["ln1_b"]
    qw = p["qkv_w"][:, :, 0, 0]  # [288, 96]
    qw_eff = qw * g1v[None, :96]
    w["qkv_wT"] = bfw(np.concatenate(
        [qw_eff[j * 96:(j + 1) * 96].T for j in range(3)], axis=1))  # [96, 3*96]
    w["qkv_bias"] = f32((qw @ b1v[:96]).reshape(3, 96).T)  # [96, 3]

    qdw = p["qkv_dw_w"][:, 0]  # [288,3,3]
    qdw_d = np.zeros((96, 27 * 96), np.float32)
    for t in range(9):
        dy, dx = t // 3 - 1, t % 3 - 1
        for j in range(3):
            qdw_d[:, (t * 3 + j) * 96:(t * 3 + j + 1) * 96] = \
                np.diag(qdw[j * 96:(j + 1) * 96, dy + 1, dx + 1])
    w["qdw_diag"] = bfw(qdw_d)
    f8w = lambda a: (np.ascontiguousarray(
        np.clip(np.asarray(a, np.float32), -240, 240).astype(FP8NP)), FP8)
    qdw8 = np.zeros((96, len(QPAIRS) * 3 * 192), np.float32)
    for pi, (ta, tb) in enumerate(QPAIRS):
        for j in range(3):
            o = (pi * 3 + j) * 192
            qdw8[:, o:o + 96] = np.diag(qdw[j * 96:(j + 1) * 96,
                                            ta // 3, ta % 3])
            if tb is not None:
                qdw8[:, o + 96:o + 192] = np.diag(qdw[j * 96:(j + 1) * 96,
                                                      tb // 3, tb % 3])
    w["qdw8"] = f8w(qdw8)

    gw1 = p["gate_w1"][:, :, 0, 0]  # [96, 192]
    gw1_eff = gw1 * g1v[None, :]
    w["gate_w1T"] = bfw(np.concatenate(
        [gw1_eff[:, cg * 96:(cg + 1) * 96].T for cg in range(2)], axis=1))  # [96, 192]
    w["gate_b1"] = f32((p["gate_b1"] + gw1 @ b1v).reshape(96, 1))
    w["gate_w2T"] = bfw(p["gate_w2"][:, :, 0, 0].T.copy())  # [96,1]
    w["gate_b2"] = f32(p["gate_b2"].reshape(1, 1))

    pj = p["proj_w"][:, :, 0, 0]
    pj1, pj2 = pj[:, :96], pj[:, 96:] * g1v[None, 96:]
    w["proj1T"] = bfw(np.concatenate(
        [pj1[cg * 96:(cg + 1) * 96].T for cg in range(2)], axis=1))  # [96, 192]
    w["proj2T"] = bfw(np.concatenate(
        [pj2[cg * 96:(cg + 1) * 96].T for cg in range(2)], axis=1))
    w["proj_bias"] = f32((pj[:, 96:] @ b1v[96:]).reshape(2, 96).T)  # [96, 2]

    attn_scale = float(p["attn1"][0] + p["attn2"][0] + p["attn3"][0] + p["attn4"][0])
    w["_attn_scale"] = (attn_scale, None)
    w["tempvec"] = f32(np.repeat(p["temperature"].reshape(HEADS), CPH).reshape(96, 1))

    g2v, b2v = p["ln2_g"], p["ln2_b"]
    f1 = p["fc1_w"][:, :, 0, 0]  # [256, 192]
    f1_eff = f1 * g2v[None, :]
    fc1 = np.zeros((96, 4 * 128), np.float32)
    for mg in range(2):
        for cg in range(2):
            fc1[:, (mg * 2 + cg) * 128:(mg * 2 + cg + 1) * 128] = \
                f1_eff[mg * 128:(mg + 1) * 128, cg * 96:(cg + 1) * 96].T
    w["fc1T"] = bfw(fc1)
    w["fc1_bias"] = f32((f1 @ b2v).reshape(2, 128).T)  # [128, 2]

    s1 = p["bn1_g"] / np.sqrt(p["bn1_v"] + eps_bn)
    t1 = p["bn1_b"] - p["bn1_m"] * s1
    s2 = p["bn2_g"] / np.sqrt(p["bn2_v"] + eps_bn)
    t2 = p["bn2_b"] - p["bn2_m"] * s2
    s3 = p["bn3_g"] / np.sqrt(p["bn3_v"] + eps_bn)
    t3 = p["bn3_b"] - p["bn3_m"] * s3

    dw1w, dw2w, dw3w = p["dw1_w"][:, 0], p["dw2_w"][:, 0], p["dw3_w"][:, 0]
    dw1b, dw2b, dw3b = p["dw1_b"], p["dw2_b"], p["dw3_b"]
    s1g = [s1[i * 64:(i + 1) * 64] for i in range(4)]
    t1g = [t1[i * 64:(i + 1) * 64] for i in range(4)]

    pair_d = np.zeros((128, 25 * 128), np.float32)
    for t in range(25):
        dy, dx = t // 5 - 2, t % 5 - 2
        blk = np.zeros((128, 128), np.float32)
        d2 = dw2w[:, dy + 2, dx + 2] * s1g[2]
        if dy == 0 and dx == 0:
            d2 = d2 + s1g[2]
        blk[64:, 64:] = np.diag(d2)
        if -1 <= dy <= 1 and -1 <= dx <= 1:
            d1 = dw1w[:, dy + 1, dx + 1] * s1g[1]
            if dy == 0 and dx == 0:
                d1 = d1 + s1g[1]
            blk[:64, :64] = np.diag(d1)
        pair_d[:, t * 128:(t + 1) * 128] = blk
    w["pair_diag"] = bfw(pair_d)
    f8w = lambda a: (np.ascontiguousarray(
        np.clip(np.asarray(a, np.float32), -240, 240).astype(FP8NP)), FP8)
    npair8 = 13
    pair8 = np.zeros((128, npair8 * 256), np.float32)
    for pi in range(npair8):
        ta, tb = 2 * pi, 2 * pi + 1
        pair8[:, pi * 256:pi * 256 + 128] = pair_d[:, ta * 128:(ta + 1) * 128]
        if tb < 25:
            pair8[:, pi * 256 + 128:pi * 256 + 256] = \
                pair_d[:, tb * 128:(tb + 1) * 128]
    w["pair8"] = f8w(pair8)
    bc1 = t1g[1] * dw1w.sum((1, 2)) + dw1b + t1g[1]
    bc2 = t1g[2] * dw2w.sum((1, 2)) + dw2b + t1g[2]
    w["pair_bias"] = f32(np.concatenate([bc1, bc2]).reshape(128, 1))

    # rows 64:128 of v0t2 hold the same data stored shifted +1, so a read at
    # AP offset (dy, dxa) yields tap (dy, dxa-1) for those rows.
    dw3_passes = []
    for dy in range(-3, 4):
        for dxa in (-2, 0, 2):
            dw3_passes.append((dy, dxa, True))
        dw3_passes.append((dy, 3, False))
    dw3_d = np.zeros((128, len(dw3_passes) * 64), np.float32)
    for i, (dy, dxa, hasb) in enumerate(dw3_passes):
        wa = dw3w[:, dy + 3, dxa + 3] * s1g[3]
        if dy == 0 and dxa == 0:
            wa = wa + s1g[3]
        dw3_d[:64, i * 64:(i + 1) * 64] = np.diag(wa)
        if hasb:
            wb = dw3w[:, dy + 3, dxa - 1 + 3] * s1g[3]
            if dy == 0 and dxa - 1 == 0:
                wb = wb + s1g[3]
            dw3_d[64:, i * 64:(i + 1) * 64] = np.diag(wb)
    w["dw3_diag"] = bfw(dw3_d)
    ndw38 = len(dw3_passes) // 2
    dw38 = np.zeros((128, ndw38 * 128), np.float32)
    for pi in range(ndw38):
        dw38[:, pi * 128:pi * 128 + 64] = dw3_d[:, (2 * pi) * 64:(2 * pi + 1) * 64]
        dw38[:, pi * 128 + 64:pi * 128 + 128] = \
            dw3_d[:, (2 * pi + 1) * 64:(2 * pi + 2) * 64]
    w["dw38"] = f8w(dw38)
    w["_dw3_passes"] = (dw3_passes, None)
    w["dw3_bias"] = f32((t1g[3] * dw3w.sum((1, 2)) + dw3b + t1g[3]).reshape(64, 1))

    d0w, d0b = p["dw0_w"][:, 0, 0, 0], p["dw0_b"]
    w["g0_scale"] = f32(((d0w + 1.0) * s1g[0]).reshape(64, 1))
    w["g0_bias"] = f32(((d0w + 1.0) * t1g[0] + d0b).reshape(64, 1))

    f2 = p["fc2_w"][:, :, 0, 0]  # [192, 256]
    f2a = f2 * s2[None, :]
    f2b = f2 * (t2 * s1)[None, :]
    cstv = f2 @ (t2 * t1)
    fc2a = np.zeros((128, 4 * 96), np.float32)
    for cg in range(2):
        for kg in range(2):
            fc2a[:, (cg * 2 + kg) * 96:(cg * 2 + kg + 1) * 96] = \
                f2a[cg * 96:(cg + 1) * 96, kg * 128:(kg + 1) * 128].T
    w["fc2aT"] = bfw(fc2a)
    w["_fc2b_zero"] = (bool(np.all(f2b == 0.0)), None)
    w["fc2bT_g0"] = bfw(np.concatenate(
        [f2b[cg * 96:(cg + 1) * 96, 0:64].T for cg in range(2)], axis=1))    # [64, 192]
    w["fc2bT_g12"] = bfw(np.concatenate(
        [f2b[cg * 96:(cg + 1) * 96, 64:192].T for cg in range(2)], axis=1))  # [128, 192]
    w["fc2bT_g3"] = bfw(np.concatenate(
        [f2b[cg * 96:(cg + 1) * 96, 192:256].T for cg in range(2)], axis=1))  # [64, 192]
    w["s3v"] = f32(np.stack([s3[:96], s3[96:]], axis=1))          # [96, 2]
    ob = s3 * cstv + t3
    w["out_bias"] = f32(np.stack([ob[:96], ob[96:]], axis=1))     # [96, 2]

    sg = np.where(s1 == 0, 1.0, s1)
    padv = -t1 / sg
    w["padv1"] = f32(np.concatenate([padv[64:128], padv[128:192]]).reshape(128, 1))
    w["padv2"] = f32(np.concatenate([padv[192:256], padv[192:256]]).reshape(128, 1))
    w["s1a"] = f32(s1[:128].reshape(128, 1))
    w["s1b"] = f32(s1[128:].reshape(128, 1))
    w["t1a"] = f32(t1[:128].reshape(128, 1))
    w["t1b"] = f32(t1[128:].reshape(128, 1))

    w["ones_st"] = bfw(np.full((96, 128), 1.0 / EMBED, np.float32))
    w["epsv"] = f32(np.full((128, 1), 1e-6, np.float32))
    vm = np.zeros((96, 96), np.float32)
    for h in range(HEADS):
        vm[h * CPH:(h + 1) * CPH, h * CPH:(h + 1) * CPH] = 1.0
    w["vmask"] = f32(vm)
    return w


WSPEC = {
    "ident": ([128, 128], F32), "identb": ([128, 128], BF16),
    "pos_diag": ([96, 18 * 96], BF16),
    "pos_b": ([96, 2], F32), "qkv_wT": ([96, 3 * 96], BF16),
    "qkv_bias": ([96, 3], F32), "qdw_diag": ([96, 27 * 96], BF16),
    "qdw8": ([96, 5 * 3 * 192], FP8), "pair8": ([128, 13 * 256], FP8),
    "dw38": ([128, 14 * 128], FP8),
    "gate_w1T": ([96, 192], BF16), "gate_b1": ([96, 1], F32),
    "gate_w2T": ([96, 1], BF16), "gate_b2": ([1, 1], F32),
    "proj1T": ([96, 192], BF16), "proj2T": ([96, 192], BF16),
    "proj_bias": ([96, 2], F32), "tempvec": ([96, 1], F32),
    "fc1T": ([96, 4 * 128], BF16), "fc1_bias": ([128, 2], F32),
    "pair_diag": ([128, 25 * 128], BF16), "pair_bias": ([128, 1], F32),
    "dw3_diag": ([128, 28 * 64], BF16), "dw3_bias": ([64, 1], F32),
    "g0_scale": ([64, 1], F32), "g0_bias": ([64, 1], F32),
    "fc2aT": ([128, 4 * 96], BF16), "fc2bT_g0": ([64, 192], BF16),
    "fc2bT_g12": ([128, 192], BF16), "fc2bT_g3": ([64, 192], BF16),
    "s3v": ([96, 2], F32), "out_bias": ([96, 2], F32),
    "padv1": ([128, 1], F32),
    "padv2": ([128, 1], F32),
    "s1a": ([128, 1], F32), "s1b": ([128, 1], F32),
    "t1a": ([128, 1], F32), "t1b": ([128, 1], F32),
    "ones_st": ([96, 128], BF16),
    "epsv": ([128, 1], F32),
    "vmask": ([96, 96], F32),
}


# ----------------------------------------------------------------------------
# device kernel
# ----------------------------------------------------------------------------

def build(nc, H, W, n_cores, attn_scale, dw3_passes, fc2b_zero=False):
    S = H * W
    Wp1 = W + 2
    P1B = (BR + 2) * Wp1 + 2 * SLOP   # band buffer (pad1)
    Wp3, Hp3 = W + 6, H + 6
    P3 = Hp3 * Wp3 + 2 * SLOP
    NCH = _ceil(H, RC)
    NB = _ceil(H, BR)
    NSC = _ceil(S, 512)
    GCH = 512 // W                    # gate chunk rows (512 cols)
    NGC_PER_BAND = _ceil(BR, GCH)

    # x and out are channel-major [EMBED, S]; host transposes NHWC<->CM
    x_t = nc.dram_tensor("x", [EMBED, S], BF16, kind="ExternalInput")
    out_t = nc.dram_tensor("out", [EMBED, S], F32, kind="ExternalOutput")
    wt = {k: nc.dram_tensor("w_" + k, shp, d, kind="ExternalInput")
          for k, (shp, d) in WSPEC.items()}

    def pd3(r):
        return SLOP + r * Wp3

    def dr_rhs(base2d, delta, n):
        # [P, 2, n] view with an overlapping middle dim of stride `delta`
        ap = [list(p) for p in base2d.ap]
        return bass.AP(base2d.tensor, base2d.offset,
                       [ap[0], [delta, 2], [1, n]])

    with tile.TileContext(nc) as tc:
        C_ONLY_W = ['pair8', 'dw38',
                    'fc1T', 'fc1_bias', 'pair_diag', 'pair_bias', 'dw3_diag',
                    'dw3_bias', 'g0_scale', 'g0_bias', 'fc2aT', 'fc2bT_g0',
                    'fc2bT_g12', 'fc2bT_g3', 's3v', 'out_bias', 's1a', 's1b',
                    't1a', 't1b', 'padv1', 'padv2']
        PERS_W = ['ones_st', 'epsv']
        with (
            tc.tile_pool(name="dram", bufs=1, space="DRAM") as dram,
            tc.tile_pool(name="persist", bufs=1) as pers,
        ):
            ws = {}

            def _load_w(pool, names):
                for k in names:
                    shp, d = WSPEC[k]
                    tl = pool.tile(shp, d, tag="w_" + k, name="w_" + k)
                    nc.sync.dma_start(out=tl[:], in_=wt[k][:])
                    ws[k] = tl

            yn1_sp = dram.tile([96, S], BF16)
            yn2_sp = dram.tile([96, S], BF16)
            xc_sp = [dram.tile([96, S], BF16, name=f"xc_sp{i}") for i in range(2)]
            xcp_sp = [dram.tile([96, S], BF16, name=f"xcp_sp{i}") for i in range(2)]
            zn_sp = [dram.tile([96, S], BF16, name=f"zn_sp{i}") for i in range(2)]
            cc_in = dram.tile([1, 1], F32)
            cc_out = dram.tile([1, 1], F32)

            gsum = pers.tile([1, NB * NGC_PER_BAND + 8], F32)
            nc.vector.memset(gsum[:], 0.0)
            dynk = pers.tile([96, 1], F32)
            probsT = pers.tile([96, 96], BF16)
            # persistent copies of LN helpers (used in phases A, B5 and C)
            ones_p = pers.tile([96, 128], BF16, name="p_ones")
            nc.sync.dma_start(out=ones_p[:], in_=wt["ones_st"][:])
            eps_p = pers.tile([128, 1], F32, name="p_eps")
            nc.sync.dma_start(out=eps_p[:], in_=wt["epsv"][:])
            # big memsets are pathologically slow; keep one zeroed band tile
            # and clear band buffers with fast engine copies instead
            zt = pers.tile([96, (BR + 2) * (W + 2) + 2 * SLOP], BF16, name="p_zero")
            nc.vector.memset(zt[:], 0.0)

            # ================= PHASE A =================
            _wpab_cm = tc.tile_pool(name="wpAB", bufs=1)
            wpab = _wpab_cm.__enter__()
            _load_w(wpab, [k for k in WSPEC
                           if k not in C_ONLY_W and k not in PERS_W])
            ident = ws["ident"]
            identb = ws["identb"]
            # v stays SBUF-resident through phase B5
            vres = wpab.tile([96, S], BF16, name="vres")
            with (
                tc.tile_pool(name="pa_band", bufs=2) as pab,
                tc.tile_pool(name="pa_rot", bufs=3) as par,
                tc.tile_pool(name="pa_ps", bufs=2, space="PSUM") as paps,
            ):
                for b in range(NB):
                    r0, r1 = b * BR, min((b + 1) * BR, H)
                    ylo, yhi = max(r0 - 1, 0), min(r1 + 1, H)
                    nr = yhi - ylo
                    boff = SLOP + (ylo - (r0 - 1)) * Wp1 + 1
                    xband = [pab.tile([96, P1B], BF16, tag=f"xb{cg}",
                                      name=f"xb{cg}") for cg in range(2)]
                    for cg in range(2):
                        # pad cells must be zero; buffers rotate with bufs=2 so
                        # zero each physical buffer once, then re-zero only the
                        # bottom halo row slot for the final band
                        if b < 2:
                            nc.scalar.copy(xband[cg][:], zt[:])
                        elif b == NB - 1:
                            ze = min(SLOP + (nr + 1) * Wp1 + SLOP, P1B)
                            zs = SLOP + nr * Wp1
                            nc.scalar.copy(xband[cg][:, zs:ze], zt[:, zs:ze])
                        dst = xband[cg][:, boff:boff + nr * Wp1] \
                            .rearrange("p (r w) -> p r w", w=Wp1)[:, :, 0:W]
                        src = x_t[cg * 96:(cg + 1) * 96, ylo * W:yhi * W] \
                            .rearrange("p (r w) -> p r w", w=W)
                        eng = nc.sync if cg == 0 else nc.scalar
                        eng.dma_start(out=dst, in_=src)
                    for c0 in range(r0, r1, RC):
                        nr_c = min(RC, H - c0)
                        N = nr_c * Wp1
                        NN = nr_c * W
                        sb0 = SLOP + (c0 - r0 + 1) * Wp1
                        xc_ch = [par.tile([96, RC * W], BF16, tag=f"xc{cg}",
                                          name=f"xc{cg}") for cg in range(2)]
                        xsq = [par.tile([96, RC * W], BF16, tag=f"xq{cg}",
                                        name=f"xq{cg}") for cg in range(2)]
                        for cg in range(2):
                            ps = paps.tile([96, RC * Wp1], F32, tag="posps")
                            for t in range(9):
                                dy, dx = t // 3 - 1, t % 3 - 1
                                o = sb0 + dy * Wp1 + dx
                                nc.tensor.matmul(
                                    ps[:, :N],
                                    ws["pos_diag"][:, (t * 2 + cg) * 96:(t * 2 + cg + 1) * 96],
                                    xband[cg][:, o:o + N],
                                    start=(t == 0), stop=(t == 8))
                            ps_int = ps[:, :N].rearrange("p (r w) -> p r w", w=Wp1)[:, :, 1:1 + W]
                            xb_int = xband[cg][:, sb0:sb0 + N] \
                                .rearrange("p (r w) -> p r w", w=Wp1)[:, :, 1:1 + W]
                            xcv = xc_ch[cg][:, :NN].rearrange("p (r w) -> p r w", w=W)
                            nc.vector.scalar_tensor_tensor(
                                out=xcv, in0=ps_int, scalar=ws["pos_b"][:, cg:cg + 1],
                                in1=xb_int, op0=Alu.add, op1=Alu.add)
                            nc.scalar.square(xsq[cg][:, :NN], xc_ch[cg][:, :NN])
                        mu_ps = paps.tile([128, RC * W], F32, tag="mups")
                        m2_ps = paps.tile([128, RC * W], F32, tag="m2ps")
                        for cg in range(2):
                            nc.tensor.matmul(mu_ps[:, :NN], ones_p, xc_ch[cg][:, :NN],
                                             start=(cg == 0), stop=(cg == 1))
                            nc.tensor.matmul(m2_ps[:, :NN], ones_p, xsq[cg][:, :NN],
                                             start=(cg == 0), stop=(cg == 1))
                        tmp = par.tile([128, RC * W], F32, tag="musq")
                        nc.scalar.square(tmp[:, :NN], mu_ps[:, :NN])
                        nc.vector.tensor_tensor(out=tmp[:, :NN], in0=m2_ps[:, :NN],
                                                in1=tmp[:, :NN], op=Alu.subtract)
                        nc.scalar.activation(tmp[:, :NN], tmp[:, :NN], Act.Sqrt,
                                             bias=eps_p)
                        rstd = par.tile([128, RC * W], F32, tag="rstd")
                        nc.vector.reciprocal(rstd[:, :NN], tmp[:, :NN])
                        for cg in range(2):
                            tdf = par.tile([96, RC * W], F32, tag=f"td{cg}")
                            nc.vector.tensor_tensor(out=tdf[:, :NN], in0=xc_ch[cg][:, :NN],
                                                    in1=mu_ps[:96, :NN], op=Alu.subtract)
                            ynch = par.tile([96, RC * W], BF16, tag=f"yn{cg}")
                            nc.vector.tensor_tensor(out=ynch[:, :NN], in0=tdf[:, :NN],
                                                    in1=rstd[:96, :NN], op=Alu.mult)
                            sp = yn1_sp if cg == 0 else yn2_sp
                            nc.sync.dma_start(out=sp[:, c0 * W:c0 * W + NN],
                                              in_=ynch[:, :NN])
                            nc.scalar.dma_start(out=xc_sp[cg][:, c0 * W:c0 * W + NN],
                                                in_=xc_ch[cg][:, :NN])

            # ================= PHASE B =================
            with (
                tc.tile_pool(name="pb_band", bufs=1) as pbb,
                tc.tile_pool(name="pb_rot", bufs=3) as pbr,
                tc.tile_pool(name="gram_ps", bufs=1, space="PSUM") as gpsp,
            ):
                g1_ps = gpsp.tile([96, 192], F32)
                g2_ps = gpsp.tile([96, 96], F32)
                with (
                    tc.tile_pool(name="pb_psg", bufs=1, space="PSUM") as pbpsg,
                    tc.tile_pool(name="pb_ps", bufs=2, space="PSUM") as pbps,
                ):
                    for b in range(NB):
                        r0, r1 = b * BR, min((b + 1) * BR, H)
                        ylo, yhi = max(r0 - 1, 0), min(r1 + 1, H)
                        ynb = [pbb.tile([96, (BR + 2) * W], BF16, tag=f"ynb{cg}",
                                        name=f"ynb{cg}") for cg in range(2)]
                        for cg in range(2):
                            sp = yn1_sp if cg == 0 else yn2_sp
                            eng = nc.sync if cg == 0 else nc.scalar
                            eng.dma_start(
                                out=ynb[cg][:, (ylo - r0 + 1) * W:(yhi - r0 + 1) * W],
                                in_=sp[:, ylo * W:yhi * W])
                        # gate (512-col chunks over rows [r0, r1))
                        for gi in range(NGC_PER_BAND):
                            gr0 = r0 + gi * GCH
                            if gr0 >= r1:
                                break
                            ngr = min(GCH, r1 - gr0)
                            NG = ngr * W
                            yo = (gr0 - r0 + 1) * W
                            gps = pbpsg.tile([96, 512], F32, tag="gps")
                            for cg in range(2):
                                nc.tensor.matmul(gps[:, :NG],
                                                 ws["gate_w1T"][:, cg * 96:(cg + 1) * 96],
                                                 ynb[cg][:, yo:yo + NG],
                                                 start=(cg == 0), stop=(cg == 1))
                            g1s = pbr.tile([96, 512], BF16, tag="g1s")
                            nc.scalar.activation(g1s[:, :NG], gps[:, :NG], Act.Relu,
                                                 bias=ws["gate_b1"])
                            g2ps = pbpsg.tile([96, 512], F32, tag="gps")
                            nc.tensor.matmul(g2ps[0:1, :NG], ws["gate_w2T"], g1s[:, :NG],
                                             start=True, stop=True)
                            sgt = pbr.tile([1, 512], F32, tag="sgt")
                            idx = b * NGC_PER_BAND + gi
                            nc.scalar.activation(sgt[:, :NG], g2ps[0:1, :NG], Act.Sigmoid,
                                                 bias=ws["gate_b2"],
                                                 accum_out=gsum[0:1, idx:idx + 1])
                        # qkv0 band
                        qkv0 = [pbb.tile([96, P1B], FP8 if FP8DW else BF16,
                                         tag=f"qk0{j}", name=f"qk0{j}")
                                for j in range(3)]
                        nrq = yhi - ylo
                        for j in range(3):
                            # single physical buffer (bufs=1): zero fully on the
                            # first band; re-zero only the stale bottom slots on
                            # the final band
                            if b == 0:
                                nc.scalar.copy(qkv0[j][:], zt[:])
                            elif b == NB - 1:
                                ze = min(SLOP + (nrq + 1) * Wp1 + SLOP, P1B)
                                zs = SLOP + nrq * Wp1
                                nc.scalar.copy(qkv0[j][:, zs:ze], zt[:, zs:ze])
                        for rr in range(ylo, yhi, 2):
                            nrw = min(2, yhi - rr)
                            NQ = nrw * W
                            for j in range(3):
                                qps = pbps.tile([96, 2 * W], F32, tag="qps")
                                nc.tensor.matmul(qps[:, :NQ],
                                                 ws["qkv_wT"][:, j * 96:(j + 1) * 96],
                                                 ynb[0][:, (rr - r0 + 1) * W:(rr - r0 + 1) * W + NQ],
                                                 start=True, stop=True)
                                dst = SLOP + (rr - r0 + 1) * Wp1 + 1
                                dview = qkv0[j][:, dst:dst + nrw * Wp1] \
                                    .rearrange("p (r w) -> p r w", w=Wp1)[:, :, 0:W]
                                nc.scalar.activation(
                                    dview, qps[:, :NQ].rearrange("p (r w) -> p r w", w=W),
                                    Act.Identity, bias=ws["qkv_bias"][:, j:j + 1])
                        # depthwise; q/k transposed into qkband via PE
                        qkband = pbr.tile([W, BR * 192], BF16, tag="qkband")
                        for c0 in range(r0, r1, RC):
                            nr_c = min(RC, H - c0)
                            N = nr_c * Wp1
                            NN = nr_c * W
                            sb0 = SLOP + (c0 - r0 + 1) * Wp1
                            qk = {}
                            for j in range(3):
                                ps = pbps.tile([96, RC * Wp1], F32, tag="dwps")
                                if FP8DW:
                                    toff = lambda t: (t // 3 - 1) * Wp1 + (t % 3 - 1)
                                    for pi, (ta, tb) in enumerate(QPAIRS):
                                        oa = sb0 + toff(ta)
                                        delta = (toff(tb) - toff(ta)) \
                                            if tb is not None else 1
                                        lhsT = ws["qdw8"][:, (pi * 3 + j) * 192:
                                                          (pi * 3 + j + 1) * 192] \
                                            .rearrange("p (two m) -> p two m", two=2)
                                        nc.tensor.matmul(
                                            ps[:, :N], lhsT,
                                            dr_rhs(qkv0[j][:, oa:oa + N], delta, N),
                                            start=(pi == 0),
                                            stop=(pi == len(QPAIRS) - 1),
                                            perf_mode=DR)
                                else:
                                    for t in range(9):
                                        dy, dx = t // 3 - 1, t % 3 - 1
                                        o = sb0 + dy * Wp1 + dx
                                        nc.tensor.matmul(
                                            ps[:, :N],
                                            ws["qdw_diag"][:, (t * 3 + j) * 96:(t * 3 + j + 1) * 96],
                                            qkv0[j][:, o:o + N],
                                            start=(t == 0), stop=(t == 8))
                                ps_int = ps[:, :N].rearrange("p (r w) -> p r w", w=Wp1)[:, :, 1:1 + W]
                                if j == 2:
                                    nc.scalar.copy(
                                        vres[:, c0 * W:c0 * W + NN]
                                        .rearrange("p (r w) -> p r w", w=W), ps_int)
                                else:
                                    qb = pbr.tile([96, RC * W], BF16, tag=f"qb{j}")
                                    nc.scalar.copy(
                                        qb[:, :NN].rearrange("p (r w) -> p r w", w=W),
                                        ps_int)
                                    qk[j] = qb
                            for rr in range(c0, c0 + nr_c):
                                ro = (rr - r0) * 192
                                rl = (rr - c0) * W
                                for j in range(2):
                                    tps = pbpsg.tile([128, 96], BF16, tag="tps")
                                    nc.tensor.transpose(tps[:], qk[j][:, rl:rl + W],
                                                        identb[:96, :96])
                                    nc.scalar.copy(
                                        qkband[:, ro + j * 96:ro + (j + 1) * 96],
                                        tps[:])
                        for rr in range(r0, r1):
                            ro = (rr - r0) * 192
                            nc.tensor.matmul(g1_ps[:], qkband[:, ro:ro + 96],
                                             qkband[:, ro:ro + 192],
                                             start=(rr == 0), stop=(rr == H - 1))
                            nc.tensor.matmul(g2_ps[:], qkband[:, ro + 96:ro + 192],
                                             qkband[:, ro + 96:ro + 192],
                                             start=(rr == 0), stop=(rr == H - 1))

                # ---- gate mean -> AllReduce -> dynk ----
                gred = pers.tile([1, 1], F32)
                nc.vector.reduce_sum(gred[:], gsum[0:1, 0:NB * NGC_PER_BAND], axis=AX)
                gsc = pers.tile([1, 1], F32)
                nc.vector.tensor_scalar_mul(gsc[:], gred[:], float(CPH) / (n_cores * S))
                nc.sync.dma_start(out=cc_in[:], in_=gsc[:])
                nc.gpsimd.collective_compute(
                    "AllReduce", Alu.add, replica_groups=[list(range(n_cores))],
                    ins=[cc_in.opt()], outs=[cc_out.opt()])
                nc.sync.dma_start(out=dynk[:], in_=cc_out[:].partition_broadcast(96))

                # ---- attn block ----
                with (
                    tc.tile_pool(name="at_ps", bufs=2, space="PSUM") as atps,
                    tc.tile_pool(name="at_sb", bufs=1) as ab,
                ):
                    g1sb = ab.tile([96, 192], F32)
                    nc.scalar.copy(g1sb[:], g1_ps[:])
                    g2sb = ab.tile([96, 96], F32)
                    nc.scalar.copy(g2sb[:], g2_ps[:])
                    idm = ident[:96, :96]
                    tq = ab.tile([96, 96], F32)
                    nc.vector.tensor_tensor(out=tq[:], in0=g1sb[:, 0:96], in1=idm,
                                            op=Alu.mult)
                    nq2 = ab.tile([96, 1], F32)
                    nc.vector.reduce_sum(nq2[:], tq[:], axis=AX)
                    tk = ab.tile([96, 96], F32)
                    nc.vector.tensor_tensor(out=tk[:], in0=g2sb[:], in1=idm,
                                            op=Alu.mult)
                    nk2 = ab.tile([96, 1], F32)
                    nc.vector.reduce_sum(nk2[:], tk[:], axis=AX)

                    def rsqrt_clamped(nm, src):
                        sq = ab.tile([96, 1], F32, tag=nm + "sq")
                        nc.scalar.sqrt(sq[:], src[:])
                        cl = ab.tile([96, 1], F32, tag=nm + "cl")
                        nc.vector.tensor_scalar_max(cl[:], sq[:], 1e-12)
                        rvv = ab.tile([96, 1], F32, tag=nm)
                        nc.vector.reciprocal(rvv[:], cl[:])
                        return rvv

                    rq = rsqrt_clamped("rq", nq2)
                    rk = rsqrt_clamped("rk", nk2)
                    rqt = ab.tile([96, 1], F32)
                    nc.vector.tensor_tensor(out=rqt[:], in0=rq[:], in1=ws["tempvec"][:],
                                            op=Alu.mult)
                    asr = ab.tile([96, 96], F32)
                    nc.vector.tensor_scalar_mul(asr[:], g1sb[:, 96:192], rqt[:])
                    as_ps = atps.tile([96, 96], F32, tag="atp")
                    nc.tensor.transpose(as_ps[:], asr[:], ident[:96, :96])
                    ast = ab.tile([96, 96], F32)
                    nc.vector.tensor_scalar_mul(ast[:], as_ps[:], rk[:])
                    as2_ps = atps.tile([96, 96], F32, tag="atp")
                    nc.tensor.transpose(as2_ps[:], ast[:], ident[:96, :96])
                    as2 = ab.tile([96, 96], F32)
                    nc.scalar.copy(as2[:], as2_ps[:])
                    # mask off-head-block entries to -60
                    t60 = ab.tile([96, 96], F32)
                    nc.vector.tensor_scalar_add(t60[:], as2[:], 60.0)
                    amf = ab.tile([96, 96], F32)
                    nc.vector.tensor_tensor(out=amf[:], in0=t60[:], in1=ws["vmask"][:],
                                            op=Alu.mult)
                    nc.vector.tensor_scalar_add(amf[:], amf[:], -60.0)
                    # rank+1 over full row via pairwise is_ge
                    rnk3 = ab.tile([96, 96 * 96], F32)
                    a_i = amf[:].unsqueeze(1).broadcast_to([96, 96, 96])
                    a_d = amf[:].unsqueeze(2).broadcast_to([96, 96, 96])
                    rvw = rnk3[:].rearrange("p (i d) -> p i d", d=96)
                    nc.vector.tensor_tensor(out=rvw, in0=a_i, in1=a_d, op=Alu.is_ge)
                    rank1 = ab.tile([96, 96], F32)
                    nc.vector.reduce_sum(rank1[:].unsqueeze(2), rvw, axis=AX)
                    sel = ab.tile([96, 96], F32)
                    nc.vector.tensor_tensor(out=sel[:], in0=rank1[:],
                                            in1=dynk[:].broadcast_to([96, 96]), op=Alu.is_le)
                    am = ab.tile([96, 96], F32)
                    t60b = ab.tile([96, 96], F32)
                    nc.vector.tensor_scalar_add(t60b[:], amf[:], 60.0)
                    nc.vector.tensor_tensor(out=am[:], in0=t60b[:], in1=sel[:], op=Alu.mult)
                    nc.vector.tensor_scalar_add(am[:], am[:], -60.0)
                    mx = ab.tile([96, 1], F32)
                    nc.vector.reduce_max(mx[:], am[:], axis=AX)
                    nmx = ab.tile([96, 1], F32)
                    nc.vector.tensor_scalar_mul(nmx[:], mx[:], -1.0)
                    ex = ab.tile([96, 96], F32)
                    nc.scalar.activation(ex[:], am[:], Act.Exp, bias=nmx[:])
                    sme = ab.tile([96, 1], F32)
                    nc.vector.reduce_sum(sme[:], ex[:], axis=AX)
                    rsm = ab.tile([96, 1], F32)
                    nc.vector.reciprocal(rsm[:], sme[:])
                    probs = ab.tile([96, 96], F32)
                    nc.vector.tensor_scalar_mul(probs[:], ex[:], rsm[:])
                    pt_ps = atps.tile([96, 96], F32, tag="atp2")
                    nc.tensor.transpose(pt_ps[:], probs[:], ident[:96, :96])
                    nc.scalar.copy(probsT[:], pt_ps[:])

            # ================= PHASE B5 =================
            with (
                tc.tile_pool(name="b5_rot", bufs=3) as b5r,
                tc.tile_pool(name="b5_ps", bufs=1, space="PSUM") as b5ps,
            ):
                for ci in range(NSC):
                    o0 = ci * 512
                    NN = min(512, S - o0)
                    av_ps = b5ps.tile([96, 512], F32, tag="avps")
                    nc.tensor.matmul(av_ps[:, :NN], probsT[:], vres[:, o0:o0 + NN],
                                     start=True, stop=True)
                    avs = b5r.tile([96, 512], BF16, tag="avs")
                    nc.scalar.activation(avs[:, :NN], av_ps[:, :NN], Act.Copy,
                                         scale=attn_scale)
                    x2ch = b5r.tile([96, 512], BF16, tag="x2ch")
                    nc.sync.dma_start(out=x2ch[:, :NN], in_=yn2_sp[:, o0:o0 + NN])
                    xpch = [b5r.tile([96, 512], BF16, tag=f"xp{cg}", name=f"xp{cg}")
                            for cg in range(2)]
                    xsq = [b5r.tile([96, 512], BF16, tag=f"xs{cg}", name=f"xs{cg}")
                           for cg in range(2)]
                    for cg in range(2):
                        xcch = b5r.tile([96, 512], BF16, tag=f"xcc{cg}")
                        nc.scalar.dma_start(out=xcch[:, :NN], in_=xc_sp[cg][:, o0:o0 + NN])
                        pj_ps = b5ps.tile([96, 512], F32, tag=f"pjps{cg}")
                        nc.tensor.matmul(pj_ps[:, :NN],
                                         ws["proj1T"][:, cg * 96:(cg + 1) * 96],
                                         avs[:, :NN], start=True, stop=False)
                        nc.tensor.matmul(pj_ps[:, :NN],
                                         ws["proj2T"][:, cg * 96:(cg + 1) * 96],
                                         x2ch[:, :NN], start=False, stop=True)
                        nc.vector.scalar_tensor_tensor(
                            out=xpch[cg][:, :NN], in0=pj_ps[:, :NN],
                            scalar=ws["proj_bias"][:, cg:cg + 1], in1=xcch[:, :NN],
                            op0=Alu.add, op1=Alu.add)
                        nc.sync.dma_start(out=xcp_sp[cg][:, o0:o0 + NN],
                                          in_=xpch[cg][:, :NN])
                        nc.scalar.square(xsq[cg][:, :NN], xpch[cg][:, :NN])
                    mu_ps = b5ps.tile([128, 512], F32, tag="mu2ps")
                    m2_ps = b5ps.tile([128, 512], F32, tag="m22ps")
                    for cg in range(2):
                        nc.tensor.matmul(mu_ps[:, :NN], ones_p, xpch[cg][:, :NN],
                                         start=(cg == 0), stop=(cg == 1))
                        nc.tensor.matmul(m2_ps[:, :NN], ones_p, xsq[cg][:, :NN],
                                         start=(cg == 0), stop=(cg == 1))
                    tmp = b5r.tile([128, 512], F32, tag="musq2")
                    nc.scalar.square(tmp[:, :NN], mu_ps[:, :NN])
                    nc.vector.tensor_tensor(out=tmp[:, :NN], in0=m2_ps[:, :NN],
                                            in1=tmp[:, :NN], op=Alu.subtract)
                    nc.scalar.activation(tmp[:, :NN], tmp[:, :NN], Act.Sqrt,
                                         bias=eps_p)
                    rstd = b5r.tile([128, 512], F32, tag="rstd2")
                    nc.vector.reciprocal(rstd[:, :NN], tmp[:, :NN])
                    for cg in range(2):
                        td2 = b5r.tile([96, 512], F32, tag=f"td2{cg}")
                        nc.vector.tensor_tensor(out=td2[:, :NN], in0=xpch[cg][:, :NN],
                                                in1=mu_ps[:96, :NN], op=Alu.subtract)
                        znt = b5r.tile([96, 512], BF16, tag=f"znt{cg}")
                        nc.vector.tensor_tensor(out=znt[:, :NN], in0=td2[:, :NN],
                                                in1=rstd[:96, :NN], op=Alu.mult)
                        nc.scalar.dma_start(out=zn_sp[cg][:, o0:o0 + NN],
                                            in_=znt[:, :NN])

            _wpab_cm.__exit__(None, None, None)
            # ================= PHASE C =================
            _wpc_cm = tc.tile_pool(name="wpC", bufs=1)
            wpc = _wpc_cm.__enter__()
            _load_w(wpc, C_ONLY_W)
            with tc.tile_pool(name="c_v0", bufs=1) as cv0:
                v0t1 = cv0.tile([128, P3], BF16)
                v0t2 = cv0.tile([128, P3], BF16)
                vgug = cv0.tile([128, P3], BF16)   # rows 0:64 = v-gelu0, 64:128 = u-gelu0
                with (
                    tc.tile_pool(name="c1_rot", bufs=2) as c1r,
                    tc.tile_pool(name="c1_ps", bufs=2, space="PSUM") as c1ps,
                ):
                    # pad cells must hold -t1/s1 so the bn-folded depthwise
                    # reads zeros in v0_bn space at image borders. Only the pad
                    # regions need initialization (interior is overwritten):
                    # top rows, bottom rows, and left/right columns per row.
                    def _pad_init(tl, padw, lcols, rcols):
                        for a, bnd in ((0, pd3(3)), (pd3(H + 3), P3)):
                            nc.vector.memset(tl[:, a:bnd], 0.0)
                            nc.vector.tensor_scalar_add(tl[:, a:bnd], tl[:, a:bnd],
                                                        padw)
                        for (p0, p1, c0_, c1_) in (lcols + rcols):
                            vv = tl[p0:p1, pd3(3):pd3(3 + H)] \
                                .rearrange("p (r w) -> p r w", w=Wp3)[:, :, c0_:c1_]
                            nc.vector.memset(vv, 0.0)
                            nc.vector.tensor_scalar_add(vv, vv, padw[p0:p1])

                    _pad_init(v0t1, ws["padv1"],
                              [(0, 128, 0, 3)], [(0, 128, 3 + W, Wp3)])
                    # v0t2 rows 64:128 are stored shifted +1 (interior written
                    # at cols [4, 4+W)), so their pads are cols [0,4) and
                    # [4+W, Wp3)
                    _pad_init(v0t2, ws["padv2"],
                              [(0, 64, 0, 3), (64, 128, 0, 4)],
                              [(0, 64, 3 + W, Wp3), (64, 128, 4 + W, Wp3)])
                    for ci in range(NCH):
                        c0 = ci * RC
                        nr_c = min(RC, H - c0)
                        NN = nr_c * W
                        o0 = c0 * W
                        d0 = pd3(3 + c0) + 3

                        def v0view(tl, lo, hi, shift=0):
                            return tl[lo:hi, d0 + shift:d0 + shift + nr_c * Wp3] \
                                .rearrange("p (r w) -> p r w", w=Wp3)[:, :, 0:W]

                        znch = [c1r.tile([96, RC * W], BF16, tag=f"cz{cg}",
                                         name=f"cz{cg}") for cg in range(2)]
                        for cg in range(2):
                            eng = nc.sync if cg == 0 else nc.scalar
                            eng.dma_start(out=znch[cg][:, :NN],
                                          in_=zn_sp[cg][:, o0:o0 + NN])
                        for mg in range(2):
                            fps = c1ps.tile([128, RC * W], F32, tag="fps")
                            for cg in range(2):
                                nc.tensor.matmul(
                                    fps[:, :NN],
                                    ws["fc1T"][:, (mg * 2 + cg) * 128:(mg * 2 + cg + 1) * 128],
                                    znch[cg][:, :NN], start=(cg == 0), stop=(cg == 1))
                            fv = lambda lo, hi: fps[lo:hi, :NN] \
                                .rearrange("p (r w) -> p r w", w=W)
                            if mg == 0:
                                nc.scalar.activation(
                                    v0view(vgug, 0, 64), fv(0, 64), Act.Gelu,
                                    bias=ws["fc1_bias"][0:64, 0:1])
                                nc.scalar.activation(
                                    v0view(vgug, 64, 128), v0view(vgug, 0, 64),
                                    Act.Gelu, bias=ws["g0_bias"], scale=ws["g0_scale"])
                                nc.scalar.activation(
                                    v0view(v0t1, 0, 64), fv(64, 128), Act.Gelu,
                                    bias=ws["fc1_bias"][64:128, 0:1])
                            else:
                                nc.scalar.activation(
                                    v0view(v0t1, 64, 128), fv(0, 64), Act.Gelu,
                                    bias=ws["fc1_bias"][0:64, 1:2])
                                nc.scalar.activation(
                                    v0view(v0t2, 0, 64), fv(64, 128), Act.Gelu,
                                    bias=ws["fc1_bias"][64:128, 1:2])
                                nc.scalar.activation(
                                    v0view(v0t2, 64, 128, shift=1), fv(64, 128),
                                    Act.Gelu, bias=ws["fc1_bias"][64:128, 1:2])

                if FP8DW_C:
                    v0t18 = cv0.tile([128, P3], FP8, name="v0t18")
                    v0t28 = cv0.tile([128, P3], FP8, name="v0t28")
                    nc.scalar.copy(v0t18[:], v0t1[:])
                    nc.vector.tensor_copy(out=v0t28[:], in_=v0t2[:])
                with (
                    tc.tile_pool(name="c2_rot", bufs=2) as c2r,
                    tc.tile_pool(name="c2_ps", bufs=2, space="PSUM") as c2ps,
                ):
                    for ci in range(NCH):
                        c0 = ci * RC
                        nr_c = min(RC, H - c0)
                        N = nr_c * Wp3
                        NN = nr_c * W
                        sb0 = pd3(3 + c0)
                        ps_a = c2ps.tile([128, RC * Wp3], F32, tag="psa")
                        if FP8DW_C:
                            poff = lambda t: (t // 5 - 2) * Wp3 + (t % 5 - 2)
                            for pi in range(13):
                                ta, tb = 2 * pi, 2 * pi + 1
                                oa = sb0 + poff(ta)
                                delta = (poff(tb) - poff(ta)) if tb < 25 else 1
                                lhsT = ws["pair8"][:, pi * 256:(pi + 1) * 256] \
                                    .rearrange("p (two m) -> p two m", two=2)
                                nc.tensor.matmul(
                                    ps_a[:, :N], lhsT,
                                    dr_rhs(v0t18[:, oa:oa + N], delta, N),
                                    start=(pi == 0), stop=(pi == 12),
                                    perf_mode=DR)
                        else:
                            for t in range(25):
                                dy, dx = t // 5 - 2, t % 5 - 2
                                o = sb0 + dy * Wp3 + dx
                                nc.tensor.matmul(ps_a[:, :N],
                                                 ws["pair_diag"][:, t * 128:(t + 1) * 128],
                                                 v0t1[:, o:o + N],
                                                 start=(t == 0), stop=(t == 24))
                        ps_b = c2ps.tile([64, RC * Wp3], F32, tag="psb")
                        if FP8DW_C:
                            nmeta = len(dw3_passes) // 2
                            for pi in range(nmeta):
                                (dya, dxaa, _), (dyb, dxab, _) = \
                                    dw3_passes[2 * pi], dw3_passes[2 * pi + 1]
                                oa = sb0 + dya * Wp3 + dxaa
                                delta = (dyb * Wp3 + dxab) - (dya * Wp3 + dxaa)
                                lhsT = ws["dw38"][:, pi * 128:(pi + 1) * 128] \
                                    .rearrange("p (two m) -> p two m", two=2)
                                nc.tensor.matmul(
                                    ps_b[:, :N], lhsT,
                                    dr_rhs(v0t28[:, oa:oa + N], delta, N),
                                    start=(pi == 0), stop=(pi == nmeta - 1),
                                    perf_mode=DR)
                        else:
                            for i, (dy, dxa, hasb) in enumerate(dw3_passes):
                                o = sb0 + dy * Wp3 + dxa
                                nc.tensor.matmul(ps_b[:, :N],
                                                 ws["dw3_diag"][:, i * 64:(i + 1) * 64],
                                                 v0t2[:, o:o + N],
                                                 start=(i == 0), stop=(i == len(dw3_passes) - 1))

                        def inner(ap_flat, lo, hi):
                            # interior view of a PSUM chunk (starts at free 0)
                            return ap_flat[lo:hi, :N].rearrange(
                                "p (r w) -> p r w", w=Wp3)[:, :, 3:3 + W]

                        def inner_v0(tl, lo, hi):
                            # interior view of the padded v0 buffers at this chunk
                            return tl[lo:hi, sb0:sb0 + N].rearrange(
                                "p (r w) -> p r w", w=Wp3)[:, :, 3:3 + W]

                        ug_a = c2r.tile([128, RC * W], BF16, tag="uga")
                        ug_b = c2r.tile([128, RC * W], BF16, tag="ugb")
                        vb_a = c2r.tile([128, RC * W], BF16, tag="vba")
                        vb_b = c2r.tile([128, RC * W], BF16, tag="vbb")
                        uv = lambda tl, lo, hi: tl[lo:hi, :NN] \
                            .rearrange("p (r w) -> p r w", w=W)
                        nc.scalar.copy(uv(ug_a, 0, 64), inner_v0(vgug, 64, 128))
                        nc.scalar.activation(uv(ug_a, 64, 128), inner(ps_a, 0, 64),
                                             Act.Gelu, bias=ws["pair_bias"][0:64])
                        nc.scalar.activation(uv(ug_b, 0, 64), inner(ps_a, 64, 128),
                                             Act.Gelu, bias=ws["pair_bias"][64:128])
                        nc.scalar.activation(uv(ug_b, 64, 128), inner(ps_b, 0, 64),
                                             Act.Gelu, bias=ws["dw3_bias"])
                        nc.gpsimd.tensor_scalar(out=uv(vb_a, 0, 64),
                                                in0=inner_v0(vgug, 0, 64),
                                                scalar1=ws["s1a"][0:64],
                                                scalar2=ws["t1a"][0:64],
                                                op0=Alu.mult, op1=Alu.add)
                        nc.gpsimd.tensor_scalar(out=uv(vb_a, 64, 128),
                                                in0=inner_v0(v0t1, 0, 64),
                                                scalar1=ws["s1a"][64:128],
                                                scalar2=ws["t1a"][64:128],
                                                op0=Alu.mult, op1=Alu.add)
                        nc.gpsimd.tensor_scalar(out=uv(vb_b, 0, 64),
                                                in0=inner_v0(v0t1, 64, 128),
                                                scalar1=ws["s1b"][0:64],
                                                scalar2=ws["t1b"][0:64],
                                                op0=Alu.mult, op1=Alu.add)
                        nc.gpsimd.tensor_scalar(out=uv(vb_b, 64, 128),
                                                in0=inner_v0(v0t2, 0, 64),
                                                scalar1=ws["s1b"][64:128],
                                                scalar2=ws["t1b"][64:128],
                                                op0=Alu.mult, op1=Alu.add)
                        z1a = c2r.tile([128, RC * W], BF16, tag="z1a")
                        z1b = c2r.tile([128, RC * W], BF16, tag="z1b")
                        nc.gpsimd.tensor_tensor(out=z1a[:, :NN], in0=ug_a[:, :NN],
                                                in1=vb_a[:, :NN], op=Alu.mult)
                        nc.vector.tensor_tensor(out=z1b[:, :NN], in0=ug_b[:, :NN],
                                                in1=vb_b[:, :NN], op=Alu.mult)
                        for cg in range(2):
                            ops = c2ps.tile([96, RC * W], F32, tag=f"ops{cg}")
                            nc.tensor.matmul(ops[:, :NN],
                                             ws["fc2aT"][:, (cg * 2) * 96:(cg * 2 + 1) * 96],
                                             z1a[:, :NN], start=True, stop=False)
                            nc.tensor.matmul(ops[:, :NN],
                                             ws["fc2aT"][:, (cg * 2 + 1) * 96:(cg * 2 + 2) * 96],
                                             z1b[:, :NN], start=False, stop=fc2b_zero)
                            if not fc2b_zero:
                                opsv = ops[:, :NN].rearrange("p (r w) -> p r w", w=W)
                                nc.tensor.matmul(opsv,
                                                 ws["fc2bT_g0"][:, cg * 96:(cg + 1) * 96],
                                                 inner_v0(vgug, 0, 64), start=False, stop=False)
                                nc.tensor.matmul(opsv,
                                                 ws["fc2bT_g12"][:, cg * 96:(cg + 1) * 96],
                                                 inner_v0(v0t1, 0, 128), start=False, stop=False)
                                nc.tensor.matmul(opsv,
                                                 ws["fc2bT_g3"][:, cg * 96:(cg + 1) * 96],
                                                 inner_v0(v0t2, 0, 64), start=False, stop=True)
                            xrch = c2r.tile([96, RC * W], BF16, tag=f"xr{cg}", bufs=1)
                            eng = nc.sync if cg == 0 else nc.scalar
                            eng.dma_start(out=xrch[:, :NN],
                                          in_=xcp_sp[cg][:, c0 * W:c0 * W + NN])
                            ob = c2r.tile([96, RC * W], F32, tag=f"ob{cg}", bufs=1)
                            nc.vector.tensor_scalar(out=ob[:, :NN], in0=ops[:, :NN],
                                                    scalar1=ws["s3v"][:, cg:cg + 1],
                                                    scalar2=ws["out_bias"][:, cg:cg + 1],
                                                    op0=Alu.mult, op1=Alu.add)
                            oc = c2r.tile([96, RC * W], F32, tag=f"oc{cg}", bufs=1)
                            nc.gpsimd.tensor_tensor(out=oc[:, :NN], in0=ob[:, :NN],
                                                    in1=xrch[:, :NN], op=Alu.add)
                            eng2 = nc.sync if cg == 0 else nc.scalar
                            eng2.dma_start(
                                out=out_t[cg * 96:(cg + 1) * 96,
                                          c0 * W:c0 * W + NN],
                                in_=oc[:, :NN])
            _wpc_cm.__exit__(None, None, None)
    return out_t.name


# ----------------------------------------------------------------------------
# host entry
# ----------------------------------------------------------------------------

_CACHE = {}
_FC2B_ZERO = {}


def make_program(H, W, n_cores, attn_scale, dw3_passes, fc2b_zero=None):
    if fc2b_zero is None:
        fc2b_zero = _FC2B_ZERO.get("v", False)
    key = (H, W, n_cores, round(attn_scale, 9), fc2b_zero)
    if key in _CACHE:
        return _CACHE[key]
    nc = bacc.Bacc("TRN2", target_bir_lowering=False, debug=False, num_devices=n_cores)
    out_name = build(nc, H, W, n_cores, attn_scale, dw3_passes, fc2b_zero=fc2b_zero)
    nc.compile()
    _CACHE[key] = (nc, out_name)
    return nc, out_name


def make_in_maps(inputs):
    x = np.asarray(inputs["x"], np.float32)
    B = x.shape[0]
    C = x.shape[-1]
    wdict = _prep_weights({k: np.asarray(v) for k, v in inputs.items()})
    _FC2B_ZERO["v"] = wdict["_fc2b_zero"][0]
    base = {}
    for k, (shp, d) in WSPEC.items():
        base["w_" + k] = wdict[k][0].reshape(shp)
    in_maps = []
    for b in range(B):
        m = dict(base)
        # channel-major [C, H*W] on device, bf16
        m["x"] = np.ascontiguousarray(x[b].reshape(-1, C).T).astype(BF16NP)
        in_maps.append(m)
    return in_maps, wdict


def kernel(**inputs):
    x = np.asarray(inputs["x"], np.float32)
    B, H, W, C = x.shape
    in_maps, wdict = make_in_maps(inputs)
    nc, out_name = make_program(H, W, B, wdict["_attn_scale"][0],
                                wdict["_dw3_passes"][0])
    res = bass_utils.run_bass_kernel_spmd(nc, in_maps, core_ids=list(range(B)))
    return np.stack([np.asarray(res.results[b][out_name], np.float32)
                     .reshape(C, H * W).T.reshape(H, W, C) for b in range(B)])


# revision 32
# speedup vs baseline: 1.2061x; 1.0040x over previous
"""Trainium2 Bass kernel for nn_Block_87351044866235 (sparse_attention).

Data-parallel over batch: 8 samples -> 8 NeuronCores. Channel-major
layout [C, H*W] on chip; depthwise convs as diagonal bf16 matmuls on
TensorE; 1x1 convs as bf16 matmuls; LN stats via ones-matmuls; q/k gram
via PE transposes + bf16 matmuls; dynamic-k gate mean via a scalar
AllReduce. Activation spills are bf16; v stays SBUF-resident.
"""
import sys, os

for _p in ("/opt/trn_rl_repo", "/root/.axon_site/_ro/trn_rl_repo"):
    if os.path.isdir(_p) and _p not in sys.path:
        sys.path.append(_p)

import numpy as np
import ml_dtypes
import concourse.bass as bass
import concourse.bacc as bacc
import concourse.tile as tile
from concourse import mybir
from concourse import bass_utils

try:
    from concourse import tile_utils as _tu
    _tu.max_sbuf_usage = 208 * 1024
except Exception:
    pass

dt = mybir.dt
Alu = mybir.AluOpType
Act = mybir.ActivationFunctionType
AX = mybir.AxisListType.X

EMBED, PDIM, HEADS, HID = 192, 96, 8, 256
CPH = PDIM // HEADS  # 12
SLOP = 8
RC = 3    # conv output rows per chunk
BR = 12   # rows per band

F32, F32R, BF16 = dt.float32, dt.float32r, dt.bfloat16
FP8 = dt.float8e4
BF16NP = ml_dtypes.bfloat16
FP8NP = ml_dtypes.float8_e4m3
FP8DW = False      # fp8 DoubleRow for the qkv_dw depthwise conv
FP8DW_C = False    # fp8 DoubleRow for the pair/dw3 FFN depthwise convs
DR = mybir.MatmulPerfMode.DoubleRow

# tap pairs for 3x3 depthwise as fp8 DoubleRow (2 taps per pass)
QPAIRS = [(0, 1), (2, 3), (4, 5), (6, 7), (8, None)]


def _ceil(a, b):
    return (a + b - 1) // b


# ----------------------------------------------------------------------------
# host-side weight prep: everything 2D [partitions, free]
# ----------------------------------------------------------------------------

def _prep_weights(p):
    w = {}
    f32 = lambda a: (np.ascontiguousarray(a, np.float32), F32)
    bfw = lambda a: (np.ascontiguousarray(np.asarray(a, np.float32)
                                          .astype(BF16NP)), BF16)
    eps_bn = 1e-5

    w["ident"] = f32(np.eye(128, dtype=np.float32))
    w["identb"] = bfw(np.eye(128, dtype=np.float32))

    # pos depthwise diag: [96, (t*2+cg)*96]
    pw = p["pos_w"][:, 0]  # [192,3,3]
    pos_d = np.zeros((96, 18 * 96), np.float32)
    for t in range(9):
        dy, dx = t // 3 - 1, t % 3 - 1
        for cg in range(2):
            pos_d[:, (t * 2 + cg) * 96:(t * 2 + cg + 1) * 96] = \
                np.diag(pw[cg * 96:(cg + 1) * 96, dy + 1, dx + 1])
    w["pos_diag"] = bfw(pos_d)
    w["pos_b"] = f32(p["pos_b"].reshape(2, 96).T)  # [96, 2]

    g1v, b1v = p["ln1_g"], p["ln1_b"]
    qw = p["qkv_w"][:, :, 0, 0]  # [288, 96]
    qw_eff = qw * g1v[None, :96]
    w["qkv_wT"] = bfw(np.concatenate(
        [qw_eff[j * 96:(j + 1) * 96].T for j in range(3)], axis=1))  # [96, 3*96]
    w["qkv_bias"] = f32((qw @ b1v[:96]).reshape(3, 96).T)  # [96, 3]

    qdw = p["qkv_dw_w"][:, 0]  # [288,3,3]
    qdw_d = np.zeros((96, 27 * 96), np.float32)
    for t in range(9):
        dy, dx = t // 3 - 1, t % 3 - 1
        for j in range(3):
            qdw_d[:, (t * 3 + j) * 96:(t * 3 + j + 1) * 96] = \
                np.diag(qdw[j * 96:(j + 1) * 96, dy + 1, dx + 1])
    w["qdw_diag"] = bfw(qdw_d)
    f8w = lambda a: (np.ascontiguousarray(
        np.clip(np.asarray(a, np.float32), -240, 240).astype(FP8NP)), FP8)
    qdw8 = np.zeros((96, len(QPAIRS) * 3 * 192), np.float32)
    for pi, (ta, tb) in enumerate(QPAIRS):
        for j in range(3):
            o = (pi * 3 + j) * 192
            qdw8[:, o:o + 96] = np.diag(qdw[j * 96:(j + 1) * 96,
                                            ta // 3, ta % 3])
            if tb is not None:
                qdw8[:, o + 96:o + 192] = np.diag(qdw[j * 96:(j + 1) * 96,
                                                      tb // 3, tb % 3])
    w["qdw8"] = f8w(qdw8)

    gw1 = p["gate_w1"][:, :, 0, 0]  # [96, 192]
    gw1_eff = gw1 * g1v[None, :]
    w["gate_w1T"] = bfw(np.concatenate(
        [gw1_eff[:, cg * 96:(cg + 1) * 96].T for cg in range(2)], axis=1))  # [96, 192]
    w["gate_b1"] = f32((p["gate_b1"] + gw1 @ b1v).reshape(96, 1))
    w["gate_w2T"] = bfw(p["gate_w2"][:, :, 0, 0].T.copy())  # [96,1]
    w["gate_b2"] = f32(p["gate_b2"].reshape(1, 1))

    pj = p["proj_w"][:, :, 0, 0]
    pj1, pj2 = pj[:, :96], pj[:, 96:] * g1v[None, 96:]
    w["proj1T"] = bfw(np.concatenate(
        [pj1[cg * 96:(cg + 1) * 96].T for cg in range(2)], axis=1))  # [96, 192]
    w["proj2T"] = bfw(np.concatenate(
        [pj2[cg * 96:(cg + 1) * 96].T for cg in range(2)], axis=1))
    w["proj_bias"] = f32((pj[:, 96:] @ b1v[96:]).reshape(2, 96).T)  # [96, 2]

    attn_scale = float(p["attn1"][0] + p["attn2"][0] + p["attn3"][0] + p["attn4"][0])
    w["_attn_scale"] = (attn_scale, None)
    w["tempvec"] = f32(np.repeat(p["temperature"].reshape(HEADS), CPH).reshape(96, 1))

    g2v, b2v = p["ln2_g"], p["ln2_b"]
    f1 = p["fc1_w"][:, :, 0, 0]  # [256, 192]
    f1_eff = f1 * g2v[None, :]
    fc1 = np.zeros((96, 4 * 128), np.float32)
    for mg in range(2):
        for cg in range(2):
            fc1[:, (mg * 2 + cg) * 128:(mg * 2 + cg + 1) * 128] = \
                f1_eff[mg * 128:(mg + 1) * 128, cg * 96:(cg + 1) * 96].T
    w["fc1T"] = bfw(fc1)
    w["fc1_bias"] = f32((f1 @ b2v).reshape(2, 128).T)  # [128, 2]

    s1 = p["bn1_g"] / np.sqrt(p["bn1_v"] + eps_bn)
    t1 = p["bn1_b"] - p["bn1_m"] * s1
    s2 = p["bn2_g"] / np.sqrt(p["bn2_v"] + eps_bn)
    t2 = p["bn2_b"] - p["bn2_m"] * s2
    s3 = p["bn3_g"] / np.sqrt(p["bn3_v"] + eps_bn)
    t3 = p["bn3_b"] - p["bn3_m"] * s3

    dw1w, dw2w, dw3w = p["dw1_w"][:, 0], p["dw2_w"][:, 0], p["dw3_w"][:, 0]
    dw1b, dw2b, dw3b = p["dw1_b"], p["dw2_b"], p["dw3_b"]
    s1g = [s1[i * 64:(i + 1) * 64] for i in range(4)]
    t1g = [t1[i * 64:(i + 1) * 64] for i in range(4)]

    pair_d = np.zeros((128, 25 * 128), np.float32)
    for t in range(25):
        dy, dx = t // 5 - 2, t % 5 - 2
        blk = np.zeros((128, 128), np.float32)
        d2 = dw2w[:, dy + 2, dx + 2] * s1g[2]
        if dy == 0 and dx == 0:
            d2 = d2 + s1g[2]
        blk[64:, 64:] = np.diag(d2)
        if -1 <= dy <= 1 and -1 <= dx <= 1:
            d1 = dw1w[:, dy + 1, dx + 1] * s1g[1]
            if dy == 0 and dx == 0:
                d1 = d1 + s1g[1]
            blk[:64, :64] = np.diag(d1)
        pair_d[:, t * 128:(t + 1) * 128] = blk
    w["pair_diag"] = bfw(pair_d)
    f8w = lambda a: (np.ascontiguousarray(
        np.clip(np.asarray(a, np.float32), -240, 240).astype(FP8NP)), FP8)
    npair8 = 13
    pair8 = np.zeros((128, npair8 * 256), np.float32)
    for pi in range(npair8):
        ta, tb = 2 * pi, 2 * pi + 1
        pair8[:, pi * 256:pi * 256 + 128] = pair_d[:, ta * 128:(ta + 1) * 128]
        if tb < 25:
            pair8[:, pi * 256 + 128:pi * 256 + 256] = \
                pair_d[:, tb * 128:(tb + 1) * 128]
    w["pair8"] = f8w(pair8)
    bc1 = t1g[1] * dw1w.sum((1, 2)) + dw1b + t1g[1]
    bc2 = t1g[2] * dw2w.sum((1, 2)) + dw2b + t1g[2]
    w["pair_bias"] = f32(np.concatenate([bc1, bc2]).reshape(128, 1))

    # rows 64:128 of v0t2 hold the same data stored shifted +1, so a read at
    # AP offset (dy, dxa) yields tap (dy, dxa-1) for those rows.
    dw3_passes = []
    for dy in range(-3, 4):
        for dxa in (-2, 0, 2):
            dw3_passes.append((dy, dxa, True))
        dw3_passes.append((dy, 3, False))
    dw3_d = np.zeros((128, len(dw3_passes) * 64), np.float32)
    for i, (dy, dxa, hasb) in enumerate(dw3_passes):
        wa = dw3w[:, dy + 3, dxa + 3] * s1g[3]
        if dy == 0 and dxa == 0:
            wa = wa + s1g[3]
        dw3_d[:64, i * 64:(i + 1) * 64] = np.diag(wa)
        if hasb:
            wb = dw3w[:, dy + 3, dxa - 1 + 3] * s1g[3]
            if dy == 0 and dxa - 1 == 0:
                wb = wb + s1g[3]
            dw3_d[64:, i * 64:(i + 1) * 64] = np.diag(wb)
    w["dw3_diag"] = bfw(dw3_d)
    ndw38 = len(dw3_passes) // 2
    dw38 = np.zeros((128, ndw38 * 128), np.float32)
    for pi in range(ndw38):
        dw38[:, pi * 128:pi * 128 + 64] = dw3_d[:, (2 * pi) * 64:(2 * pi + 1) * 64]
        dw38[:, pi * 128 + 64:pi * 128 + 128] = \
            dw3_d[:, (2 * pi + 1) * 64:(2 * pi + 2) * 64]
    w["dw38"] = f8w(dw38)
    w["_dw3_passes"] = (dw3_passes, None)
    w["dw3_bias"] = f32((t1g[3] * dw3w.sum((1, 2)) + dw3b + t1g[3]).reshape(64, 1))

    d0w, d0b = p["dw0_w"][:, 0, 0, 0], p["dw0_b"]
    w["g0_scale"] = f32(((d0w + 1.0) * s1g[0]).reshape(64, 1))
    w["g0_bias"] = f32(((d0w + 1.0) * t1g[0] + d0b).reshape(64, 1))

    f2 = p["fc2_w"][:, :, 0, 0]  # [192, 256]
    f2a = f2 * s2[None, :]
    f2b = f2 * (t2 * s1)[None, :]
    cstv = f2 @ (t2 * t1)
    fc2a = np.zeros((128, 4 * 96), np.float32)
    for cg in range(2):
        for kg in range(2):
            fc2a[:, (cg * 2 + kg) * 96:(cg * 2 + kg + 1) * 96] = \
                f2a[cg * 96:(cg + 1) * 96, kg * 128:(kg + 1) * 128].T
    w["fc2aT"] = bfw(fc2a)
    w["_fc2b_zero"] = (bool(np.all(f2b == 0.0)), None)
    w["fc2bT_g0"] = bfw(np.concatenate(
        [f2b[cg * 96:(cg + 1) * 96, 0:64].T for cg in range(2)], axis=1))    # [64, 192]
    w["fc2bT_g12"] = bfw(np.concatenate(
        [f2b[cg * 96:(cg + 1) * 96, 64:192].T for cg in range(2)], axis=1))  # [128, 192]
    w["fc2bT_g3"] = bfw(np.concatenate(
        [f2b[cg * 96:(cg + 1) * 96, 192:256].T for cg in range(2)], axis=1))  # [64, 192]
    w["s3v"] = f32(np.stack([s3[:96], s3[96:]], axis=1))          # [96, 2]
    ob = s3 * cstv + t3
    w["out_bias"] = f32(np.stack([ob[:96], ob[96:]], axis=1))     # [96, 2]

    sg = np.where(s1 == 0, 1.0, s1)
    padv = -t1 / sg
    w["padv1"] = f32(np.concatenate([padv[64:128], padv[128:192]]).reshape(128, 1))
    w["padv2"] = f32(np.concatenate([padv[192:256], padv[192:256]]).reshape(128, 1))
    w["s1a"] = f32(s1[:128].reshape(128, 1))
    w["s1b"] = f32(s1[128:].reshape(128, 1))
    w["t1a"] = f32(t1[:128].reshape(128, 1))
    w["t1b"] = f32(t1[128:].reshape(128, 1))

    w["ones_st"] = bfw(np.full((96, 128), 1.0 / EMBED, np.float32))
    w["epsv"] = f32(np.full((128, 1), 1e-6, np.float32))
    vm = np.zeros((96, 96), np.float32)
    for h in range(HEADS):
        vm[h * CPH:(h + 1) * CPH, h * CPH:(h + 1) * CPH] = 1.0
    w["vmask"] = f32(vm)
    return w


WSPEC = {
    "ident": ([128, 128], F32), "identb": ([128, 128], BF16),
    "pos_diag": ([96, 18 * 96], BF16),
    "pos_b": ([96, 2], F32), "qkv_wT": ([96, 3 * 96], BF16),
    "qkv_bias": ([96, 3], F32), "qdw_diag": ([96, 27 * 96], BF16),
    "qdw8": ([96, 5 * 3 * 192], FP8), "pair8": ([128, 13 * 256], FP8),
    "dw38": ([128, 14 * 128], FP8),
    "gate_w1T": ([96, 192], BF16), "gate_b1": ([96, 1], F32),
    "gate_w2T": ([96, 1], BF16), "gate_b2": ([1, 1], F32),
    "proj1T": ([96, 192], BF16), "proj2T": ([96, 192], BF16),
    "proj_bias": ([96, 2], F32), "tempvec": ([96, 1], F32),
    "fc1T": ([96, 4 * 128], BF16), "fc1_bias": ([128, 2], F32),
    "pair_diag": ([128, 25 * 128], BF16), "pair_bias": ([128, 1], F32),
    "dw3_diag": ([128, 28 * 64], BF16), "dw3_bias": ([64, 1], F32),
    "g0_scale": ([64, 1], F32), "g0_bias": ([64, 1], F32),
    "fc2aT": ([128, 4 * 96], BF16), "fc2bT_g0": ([64, 192], BF16),
    "fc2bT_g12": ([128, 192], BF16), "fc2bT_g3": ([64, 192], BF16),
    "s3v": ([96, 2], F32), "out_bias": ([96, 2], F32),
    "padv1": ([128, 1], F32),
    "padv2": ([128, 1], F32),
    "s1a": ([128, 1], F32), "s1b": ([128, 1], F32),
    "t1a": ([128, 1], F32), "t1b": ([128, 1], F32),
    "ones_st": ([96, 128], BF16),
    "epsv": ([128, 1], F32),
    "vmask": ([96, 96], F32),
}


# ----------------------------------------------------------------------------
# device kernel
# ----------------------------------------------------------------------------

def build(nc, H, W, n_cores, attn_scale, dw3_passes, fc2b_zero=False):
    S = H * W
    Wp1 = W + 2
    P1B = (BR + 2) * Wp1 + 2 * SLOP   # band buffer (pad1)
    Wp3, Hp3 = W + 6, H + 6
    P3 = Hp3 * Wp3 + 2 * SLOP
    NCH = _ceil(H, RC)
    NB = _ceil(H, BR)
    NSC = _ceil(S, 512)
    GCH = 512 // W                    # gate chunk rows (512 cols)
    NGC_PER_BAND = _ceil(BR, GCH)

    # x and out are channel-major [EMBED, S]; host transposes NHWC<->CM
    x_t = nc.dram_tensor("x", [EMBED, S], BF16, kind="ExternalInput")
    out_t = nc.dram_tensor("out", [EMBED, S], F32, kind="ExternalOutput")
    wt = {k: nc.dram_tensor("w_" + k, shp, d, kind="ExternalInput")
          for k, (shp, d) in WSPEC.items()}

    def pd3(r):
        return SLOP + r * Wp3

    def dr_rhs(base2d, delta, n):
        # [P, 2, n] view with an overlapping middle dim of stride `delta`
        ap = [list(p) for p in base2d.ap]
        return bass.AP(base2d.tensor, base2d.offset,
                       [ap[0], [delta, 2], [1, n]])

    with tile.TileContext(nc) as tc:
        C_ONLY_W = ['pair8', 'dw38',
                    'fc1T', 'fc1_bias', 'pair_diag', 'pair_bias', 'dw3_diag',
                    'dw3_bias', 'g0_scale', 'g0_bias', 'fc2aT', 'fc2bT_g0',
                    'fc2bT_g12', 'fc2bT_g3', 's3v', 'out_bias', 's1a', 's1b',
                    't1a', 't1b', 'padv1', 'padv2']
        PERS_W = ['ones_st', 'epsv']
        with (
            tc.tile_pool(name="dram", bufs=1, space="DRAM") as dram,
            tc.tile_pool(name="persist", bufs=1) as pers,
        ):
            ws = {}

            def _load_w(pool, names):
                for k in names:
                    shp, d = WSPEC[k]
                    tl = pool.tile(shp, d, tag="w_" + k, name="w_" + k)
                    nc.sync.dma_start(out=tl[:], in_=wt[k][:])
                    ws[k] = tl

            yn1_sp = dram.tile([96, S], BF16)
            yn2_sp = dram.tile([96, S], BF16)
            xc_sp = [dram.tile([96, S], BF16, name=f"xc_sp{i}") for i in range(2)]
            xcp_sp = [dram.tile([96, S], BF16, name=f"xcp_sp{i}") for i in range(2)]
            zn_sp = [dram.tile([96, S], BF16, name=f"zn_sp{i}") for i in range(2)]
            cc_in = dram.tile([1, 1], F32)
            cc_out = dram.tile([1, 1], F32)

            gsum = pers.tile([1, NB * NGC_PER_BAND + 8], F32)
            nc.vector.memset(gsum[:], 0.0)
            dynk = pers.tile([96, 1], F32)
            probsT = pers.tile([96, 96], BF16)
            # persistent copies of LN helpers (used in phases A, B5 and C)
            ones_p = pers.tile([96, 128], BF16, name="p_ones")
            nc.sync.dma_start(out=ones_p[:], in_=wt["ones_st"][:])
            eps_p = pers.tile([128, 1], F32, name="p_eps")
            nc.sync.dma_start(out=eps_p[:], in_=wt["epsv"][:])
            # big memsets are pathologically slow; keep one zeroed band tile
            # and clear band buffers with fast engine copies instead
            zt = pers.tile([96, (BR + 2) * (W + 2) + 2 * SLOP], BF16, name="p_zero")
            nc.vector.memset(zt[:], 0.0)

            # ================= PHASE A =================
            _wpab_cm = tc.tile_pool(name="wpAB", bufs=1)
            wpab = _wpab_cm.__enter__()
            _load_w(wpab, [k for k in WSPEC
                           if k not in C_ONLY_W and k not in PERS_W])
            ident = ws["ident"]
            identb = ws["identb"]
            # v stays SBUF-resident through phase B5
            vres = wpab.tile([96, S], BF16, name="vres")
            with (
                tc.tile_pool(name="pa_band", bufs=3) as pab,
                tc.tile_pool(name="pa_rot", bufs=4) as par,
                tc.tile_pool(name="pa_ps", bufs=2, space="PSUM") as paps,
            ):
                for b in range(NB):
                    r0, r1 = b * BR, min((b + 1) * BR, H)
                    ylo, yhi = max(r0 - 1, 0), min(r1 + 1, H)
                    nr = yhi - ylo
                    boff = SLOP + (ylo - (r0 - 1)) * Wp1 + 1
                    xband = [pab.tile([96, P1B], BF16, tag=f"xb{cg}",
                                      name=f"xb{cg}") for cg in range(2)]
                    for cg in range(2):
                        # pad cells must be zero; buffers rotate with bufs=2 so
                        # zero each physical buffer once, then re-zero only the
                        # bottom halo row slot for the final band
                        if b < 3:
                            nc.scalar.copy(xband[cg][:], zt[:])
                        elif b == NB - 1:
                            ze = min(SLOP + (nr + 1) * Wp1 + SLOP, P1B)
                            zs = SLOP + nr * Wp1
                            nc.scalar.copy(xband[cg][:, zs:ze], zt[:, zs:ze])
                        dst = xband[cg][:, boff:boff + nr * Wp1] \
                            .rearrange("p (r w) -> p r w", w=Wp1)[:, :, 0:W]
                        src = x_t[cg * 96:(cg + 1) * 96, ylo * W:yhi * W] \
                            .rearrange("p (r w) -> p r w", w=W)
                        eng = nc.sync if cg == 0 else nc.scalar
                        eng.dma_start(out=dst, in_=src)
                    for c0 in range(r0, r1, RC):
                        nr_c = min(RC, H - c0)
                        N = nr_c * Wp1
                        NN = nr_c * W
                        sb0 = SLOP + (c0 - r0 + 1) * Wp1
                        xc_ch = [par.tile([96, RC * W], BF16, tag=f"xc{cg}",
                                          name=f"xc{cg}") for cg in range(2)]
                        xsq = [par.tile([96, RC * W], BF16, tag=f"xq{cg}",
                                        name=f"xq{cg}") for cg in range(2)]
                        for cg in range(2):
                            ps = paps.tile([96, RC * Wp1], F32, tag="posps")
                            for t in range(9):
                                dy, dx = t // 3 - 1, t % 3 - 1
                                o = sb0 + dy * Wp1 + dx
                                nc.tensor.matmul(
                                    ps[:, :N],
                                    ws["pos_diag"][:, (t * 2 + cg) * 96:(t * 2 + cg + 1) * 96],
                                    xband[cg][:, o:o + N],
                                    start=(t == 0), stop=(t == 8))
                            ps_int = ps[:, :N].rearrange("p (r w) -> p r w", w=Wp1)[:, :, 1:1 + W]
                            xb_int = xband[cg][:, sb0:sb0 + N] \
                                .rearrange("p (r w) -> p r w", w=Wp1)[:, :, 1:1 + W]
                            xcv = xc_ch[cg][:, :NN].rearrange("p (r w) -> p r w", w=W)
                            nc.vector.scalar_tensor_tensor(
                                out=xcv, in0=ps_int, scalar=ws["pos_b"][:, cg:cg + 1],
                                in1=xb_int, op0=Alu.add, op1=Alu.add)
                            nc.scalar.square(xsq[cg][:, :NN], xc_ch[cg][:, :NN])
                        mu_ps = paps.tile([128, RC * W], F32, tag="mups")
                        m2_ps = paps.tile([128, RC * W], F32, tag="m2ps")
                        for cg in range(2):
                            nc.tensor.matmul(mu_ps[:, :NN], ones_p, xc_ch[cg][:, :NN],
                                             start=(cg == 0), stop=(cg == 1))
                            nc.tensor.matmul(m2_ps[:, :NN], ones_p, xsq[cg][:, :NN],
                                             start=(cg == 0), stop=(cg == 1))
                        tmp = par.tile([128, RC * W], F32, tag="musq")
                        nc.scalar.square(tmp[:, :NN], mu_ps[:, :NN])
                        nc.vector.tensor_tensor(out=tmp[:, :NN], in0=m2_ps[:, :NN],
                                                in1=tmp[:, :NN], op=Alu.subtract)
                        nc.scalar.activation(tmp[:, :NN], tmp[:, :NN], Act.Sqrt,
                                             bias=eps_p)
                        rstd = par.tile([128, RC * W], F32, tag="rstd")
                        nc.vector.reciprocal(rstd[:, :NN], tmp[:, :NN])
                        for cg in range(2):
                            tdf = par.tile([96, RC * W], F32, tag=f"td{cg}")
                            nc.vector.tensor_tensor(out=tdf[:, :NN], in0=xc_ch[cg][:, :NN],
                                                    in1=mu_ps[:96, :NN], op=Alu.subtract)
                            ynch = par.tile([96, RC * W], BF16, tag=f"yn{cg}")
                            nc.vector.tensor_tensor(out=ynch[:, :NN], in0=tdf[:, :NN],
                                                    in1=rstd[:96, :NN], op=Alu.mult)
                            sp = yn1_sp if cg == 0 else yn2_sp
                            nc.sync.dma_start(out=sp[:, c0 * W:c0 * W + NN],
                                              in_=ynch[:, :NN])
                            nc.scalar.dma_start(out=xc_sp[cg][:, c0 * W:c0 * W + NN],
                                                in_=xc_ch[cg][:, :NN])

            # ================= PHASE B =================
            with (
                tc.tile_pool(name="pb_band", bufs=1) as pbb,
                tc.tile_pool(name="pb_rot", bufs=4) as pbr,
                tc.tile_pool(name="gram_ps", bufs=1, space="PSUM") as gpsp,
            ):
                g1_ps = gpsp.tile([96, 192], F32)
                g2_ps = gpsp.tile([96, 96], F32)
                with (
                    tc.tile_pool(name="pb_psg", bufs=1, space="PSUM") as pbpsg,
                    tc.tile_pool(name="pb_ps", bufs=2, space="PSUM") as pbps,
                ):
                    for b in range(NB):
                        r0, r1 = b * BR, min((b + 1) * BR, H)
                        ylo, yhi = max(r0 - 1, 0), min(r1 + 1, H)
                        ynb = [pbb.tile([96, (BR + 2) * W], BF16, tag=f"ynb{cg}",
                                        name=f"ynb{cg}") for cg in range(2)]
                        for cg in range(2):
                            sp = yn1_sp if cg == 0 else yn2_sp
                            eng = nc.sync if cg == 0 else nc.scalar
                            eng.dma_start(
                                out=ynb[cg][:, (ylo - r0 + 1) * W:(yhi - r0 + 1) * W],
                                in_=sp[:, ylo * W:yhi * W])
                        # gate (512-col chunks over rows [r0, r1))
                        for gi in range(NGC_PER_BAND):
                            gr0 = r0 + gi * GCH
                            if gr0 >= r1:
                                break
                            ngr = min(GCH, r1 - gr0)
                            NG = ngr * W
                            yo = (gr0 - r0 + 1) * W
                            gps = pbpsg.tile([96, 512], F32, tag="gps")
                            for cg in range(2):
                                nc.tensor.matmul(gps[:, :NG],
                                                 ws["gate_w1T"][:, cg * 96:(cg + 1) * 96],
                                                 ynb[cg][:, yo:yo + NG],
                                                 start=(cg == 0), stop=(cg == 1))
                            g1s = pbr.tile([96, 512], BF16, tag="g1s")
                            nc.scalar.activation(g1s[:, :NG], gps[:, :NG], Act.Relu,
                                                 bias=ws["gate_b1"])
                            g2ps = pbpsg.tile([96, 512], F32, tag="gps")
                            nc.tensor.matmul(g2ps[0:1, :NG], ws["gate_w2T"], g1s[:, :NG],
                                             start=True, stop=True)
                            sgt = pbr.tile([1, 512], F32, tag="sgt")
                            idx = b * NGC_PER_BAND + gi
                            nc.scalar.activation(sgt[:, :NG], g2ps[0:1, :NG], Act.Sigmoid,
                                                 bias=ws["gate_b2"],
                                                 accum_out=gsum[0:1, idx:idx + 1])
                        # qkv0 band
                        qkv0 = [pbb.tile([96, P1B], FP8 if FP8DW else BF16,
                                         tag=f"qk0{j}", name=f"qk0{j}")
                                for j in range(3)]
                        nrq = yhi - ylo
                        for j in range(3):
                            # single physical buffer (bufs=1): zero fully on the
                            # first band; re-zero only the stale bottom slots on
                            # the final band
                            if b == 0:
                                nc.scalar.copy(qkv0[j][:], zt[:])
                            elif b == NB - 1:
                                ze = min(SLOP + (nrq + 1) * Wp1 + SLOP, P1B)
                                zs = SLOP + nrq * Wp1
                                nc.scalar.copy(qkv0[j][:, zs:ze], zt[:, zs:ze])
                        for rr in range(ylo, yhi, 2):
                            nrw = min(2, yhi - rr)
                            NQ = nrw * W
                            for j in range(3):
                                qps = pbps.tile([96, 2 * W], F32, tag="qps")
                                nc.tensor.matmul(qps[:, :NQ],
                                                 ws["qkv_wT"][:, j * 96:(j + 1) * 96],
                                                 ynb[0][:, (rr - r0 + 1) * W:(rr - r0 + 1) * W + NQ],
                                                 start=True, stop=True)
                                dst = SLOP + (rr - r0 + 1) * Wp1 + 1
                                dview = qkv0[j][:, dst:dst + nrw * Wp1] \
                                    .rearrange("p (r w) -> p r w", w=Wp1)[:, :, 0:W]
                                nc.scalar.activation(
                                    dview, qps[:, :NQ].rearrange("p (r w) -> p r w", w=W),
                                    Act.Identity, bias=ws["qkv_bias"][:, j:j + 1])
                        # depthwise; q/k transposed into qkband via PE
                        qkband = pbr.tile([W, BR * 192], BF16, tag="qkband")
                        for c0 in range(r0, r1, RC):
                            nr_c = min(RC, H - c0)
                            N = nr_c * Wp1
                            NN = nr_c * W
                            sb0 = SLOP + (c0 - r0 + 1) * Wp1
                            qk = {}
                            for j in range(3):
                                ps = pbps.tile([96, RC * Wp1], F32, tag="dwps")
                                if FP8DW:
                                    toff = lambda t: (t // 3 - 1) * Wp1 + (t % 3 - 1)
                                    for pi, (ta, tb) in enumerate(QPAIRS):
                                        oa = sb0 + toff(ta)
                                        delta = (toff(tb) - toff(ta)) \
                                            if tb is not None else 1
                                        lhsT = ws["qdw8"][:, (pi * 3 + j) * 192:
                                                          (pi * 3 + j + 1) * 192] \
                                            .rearrange("p (two m) -> p two m", two=2)
                                        nc.tensor.matmul(
                                            ps[:, :N], lhsT,
                                            dr_rhs(qkv0[j][:, oa:oa + N], delta, N),
                                            start=(pi == 0),
                                            stop=(pi == len(QPAIRS) - 1),
                                            perf_mode=DR)
                                else:
                                    for t in range(9):
                                        dy, dx = t // 3 - 1, t % 3 - 1
                                        o = sb0 + dy * Wp1 + dx
                                        nc.tensor.matmul(
                                            ps[:, :N],
                                            ws["qdw_diag"][:, (t * 3 + j) * 96:(t * 3 + j + 1) * 96],
                                            qkv0[j][:, o:o + N],
                                            start=(t == 0), stop=(t == 8))
                                ps_int = ps[:, :N].rearrange("p (r w) -> p r w", w=Wp1)[:, :, 1:1 + W]
                                if j == 2:
                                    nc.scalar.copy(
                                        vres[:, c0 * W:c0 * W + NN]
                                        .rearrange("p (r w) -> p r w", w=W), ps_int)
                                else:
                                    qb = pbr.tile([96, RC * W], BF16, tag=f"qb{j}")
                                    nc.scalar.copy(
                                        qb[:, :NN].rearrange("p (r w) -> p r w", w=W),
                                        ps_int)
                                    qk[j] = qb
                            for rr in range(c0, c0 + nr_c):
                                ro = (rr - r0) * 192
                                rl = (rr - c0) * W
                                for j in range(2):
                                    tps = pbpsg.tile([128, 96], BF16, tag="tps")
                                    nc.tensor.transpose(tps[:], qk[j][:, rl:rl + W],
                                                        identb[:96, :96])
                                    nc.scalar.copy(
                                        qkband[:, ro + j * 96:ro + (j + 1) * 96],
                                        tps[:])
                        for rr in range(r0, r1):
                            ro = (rr - r0) * 192
                            nc.tensor.matmul(g1_ps[:], qkband[:, ro:ro + 96],
                                             qkband[:, ro:ro + 192],
                                             start=(rr == 0), stop=(rr == H - 1))
                            nc.tensor.matmul(g2_ps[:], qkband[:, ro + 96:ro + 192],
                                             qkband[:, ro + 96:ro + 192],
                                             start=(rr == 0), stop=(rr == H - 1))

                # ---- gate mean -> AllReduce -> dynk ----
                gred = pers.tile([1, 1], F32)
                nc.vector.reduce_sum(gred[:], gsum[0:1, 0:NB * NGC_PER_BAND], axis=AX)
                gsc = pers.tile([1, 1], F32)
                nc.vector.tensor_scalar_mul(gsc[:], gred[:], float(CPH) / (n_cores * S))
                nc.sync.dma_start(out=cc_in[:], in_=gsc[:])
                nc.gpsimd.collective_compute(
                    "AllReduce", Alu.add, replica_groups=[list(range(n_cores))],
                    ins=[cc_in.opt()], outs=[cc_out.opt()])
                nc.sync.dma_start(out=dynk[:], in_=cc_out[:].partition_broadcast(96))

                # ---- attn block ----
                with (
                    tc.tile_pool(name="at_ps", bufs=2, space="PSUM") as atps,
                    tc.tile_pool(name="at_sb", bufs=1) as ab,
                ):
                    g1sb = ab.tile([96, 192], F32)
                    nc.scalar.copy(g1sb[:], g1_ps[:])
                    g2sb = ab.tile([96, 96], F32)
                    nc.scalar.copy(g2sb[:], g2_ps[:])
                    idm = ident[:96, :96]
                    tq = ab.tile([96, 96], F32)
                    nc.vector.tensor_tensor(out=tq[:], in0=g1sb[:, 0:96], in1=idm,
                                            op=Alu.mult)
                    nq2 = ab.tile([96, 1], F32)
                    nc.vector.reduce_sum(nq2[:], tq[:], axis=AX)
                    tk = ab.tile([96, 96], F32)
                    nc.vector.tensor_tensor(out=tk[:], in0=g2sb[:], in1=idm,
                                            op=Alu.mult)
                    nk2 = ab.tile([96, 1], F32)
                    nc.vector.reduce_sum(nk2[:], tk[:], axis=AX)

                    def rsqrt_clamped(nm, src):
                        sq = ab.tile([96, 1], F32, tag=nm + "sq")
                        nc.scalar.sqrt(sq[:], src[:])
                        cl = ab.tile([96, 1], F32, tag=nm + "cl")
                        nc.vector.tensor_scalar_max(cl[:], sq[:], 1e-12)
                        rvv = ab.tile([96, 1], F32, tag=nm)
                        nc.vector.reciprocal(rvv[:], cl[:])
                        return rvv

                    rq = rsqrt_clamped("rq", nq2)
                    rk = rsqrt_clamped("rk", nk2)
                    rqt = ab.tile([96, 1], F32)
                    nc.vector.tensor_tensor(out=rqt[:], in0=rq[:], in1=ws["tempvec"][:],
                                            op=Alu.mult)
                    asr = ab.tile([96, 96], F32)
                    nc.vector.tensor_scalar_mul(asr[:], g1sb[:, 96:192], rqt[:])
                    as_ps = atps.tile([96, 96], F32, tag="atp")
                    nc.tensor.transpose(as_ps[:], asr[:], ident[:96, :96])
                    ast = ab.tile([96, 96], F32)
                    nc.vector.tensor_scalar_mul(ast[:], as_ps[:], rk[:])
                    as2_ps = atps.tile([96, 96], F32, tag="atp")
                    nc.tensor.transpose(as2_ps[:], ast[:], ident[:96, :96])
                    as2 = ab.tile([96, 96], F32)
                    nc.scalar.copy(as2[:], as2_ps[:])
                    # mask off-head-block entries to -60
                    t60 = ab.tile([96, 96], F32)
                    nc.vector.tensor_scalar_add(t60[:], as2[:], 60.0)
                    amf = ab.tile([96, 96], F32)
                    nc.vector.tensor_tensor(out=amf[:], in0=t60[:], in1=ws["vmask"][:],
                                            op=Alu.mult)
                    nc.vector.tensor_scalar_add(amf[:], amf[:], -60.0)
                    # rank+1 over full row via pairwise is_ge
                    rnk3 = ab.tile([96, 96 * 96], F32)
                    a_i = amf[:].unsqueeze(1).broadcast_to([96, 96, 96])
                    a_d = amf[:].unsqueeze(2).broadcast_to([96, 96, 96])
                    rvw = rnk3[:].rearrange("p (i d) -> p i d", d=96)
                    nc.vector.tensor_tensor(out=rvw, in0=a_i, in1=a_d, op=Alu.is_ge)
                    rank1 = ab.tile([96, 96], F32)
                    nc.vector.reduce_sum(rank1[:].unsqueeze(2), rvw, axis=AX)
                    sel = ab.tile([96, 96], F32)
                    nc.vector.tensor_tensor(out=sel[:], in0=rank1[:],
                                            in1=dynk[:].broadcast_to([96, 96]), op=Alu.is_le)
                    am = ab.tile([96, 96], F32)
                    t60b = ab.tile([96, 96], F32)
                    nc.vector.tensor_scalar_add(t60b[:], amf[:], 60.0)
                    nc.vector.tensor_tensor(out=am[:], in0=t60b[:], in1=sel[:], op=Alu.mult)
                    nc.vector.tensor_scalar_add(am[:], am[:], -60.0)
                    mx = ab.tile([96, 1], F32)
                    nc.vector.reduce_max(mx[:], am[:], axis=AX)
                    nmx = ab.tile([96, 1], F32)
                    nc.vector.tensor_scalar_mul(nmx[:], mx[:], -1.0)
                    ex = ab.tile([96, 96], F32)
                    nc.scalar.activation(ex[:], am[:], Act.Exp, bias=nmx[:])
                    sme = ab.tile([96, 1], F32)
                    nc.vector.reduce_sum(sme[:], ex[:], axis=AX)
                    rsm = ab.tile([96, 1], F32)
                    nc.vector.reciprocal(rsm[:], sme[:])
                    probs = ab.tile([96, 96], F32)
                    nc.vector.tensor_scalar_mul(probs[:], ex[:], rsm[:])
                    pt_ps = atps.tile([96, 96], F32, tag="atp2")
                    nc.tensor.transpose(pt_ps[:], probs[:], ident[:96, :96])
                    nc.scalar.copy(probsT[:], pt_ps[:])

            # ================= PHASE B5 =================
            with (
                tc.tile_pool(name="b5_rot", bufs=4) as b5r,
                tc.tile_pool(name="b5_ps", bufs=1, space="PSUM") as b5ps,
            ):
                for ci in range(NSC):
                    o0 = ci * 512
                    NN = min(512, S - o0)
                    av_ps = b5ps.tile([96, 512], F32, tag="avps")
                    nc.tensor.matmul(av_ps[:, :NN], probsT[:], vres[:, o0:o0 + NN],
                                     start=True, stop=True)
                    avs = b5r.tile([96, 512], BF16, tag="avs")
                    nc.scalar.activation(avs[:, :NN], av_ps[:, :NN], Act.Copy,
                                         scale=attn_scale)
                    x2ch = b5r.tile([96, 512], BF16, tag="x2ch")
                    nc.sync.dma_start(out=x2ch[:, :NN], in_=yn2_sp[:, o0:o0 + NN])
                    xpch = [b5r.tile([96, 512], BF16, tag=f"xp{cg}", name=f"xp{cg}")
                            for cg in range(2)]
                    xsq = [b5r.tile([96, 512], BF16, tag=f"xs{cg}", name=f"xs{cg}")
                           for cg in range(2)]
                    for cg in range(2):
                        xcch = b5r.tile([96, 512], BF16, tag=f"xcc{cg}")
                        nc.scalar.dma_start(out=xcch[:, :NN], in_=xc_sp[cg][:, o0:o0 + NN])
                        pj_ps = b5ps.tile([96, 512], F32, tag=f"pjps{cg}")
                        nc.tensor.matmul(pj_ps[:, :NN],
                                         ws["proj2T"][:, cg * 96:(cg + 1) * 96],
                                         x2ch[:, :NN], start=True, stop=False)
                        nc.tensor.matmul(pj_ps[:, :NN],
                                         ws["proj1T"][:, cg * 96:(cg + 1) * 96],
                                         avs[:, :NN], start=False, stop=True)
                        nc.vector.scalar_tensor_tensor(
                            out=xpch[cg][:, :NN], in0=pj_ps[:, :NN],
                            scalar=ws["proj_bias"][:, cg:cg + 1], in1=xcch[:, :NN],
                            op0=Alu.add, op1=Alu.add)
                        nc.sync.dma_start(out=xcp_sp[cg][:, o0:o0 + NN],
                                          in_=xpch[cg][:, :NN])
                        nc.scalar.square(xsq[cg][:, :NN], xpch[cg][:, :NN])
                    mu_ps = b5ps.tile([128, 512], F32, tag="mu2ps")
                    m2_ps = b5ps.tile([128, 512], F32, tag="m22ps")
                    for cg in range(2):
                        nc.tensor.matmul(mu_ps[:, :NN], ones_p, xpch[cg][:, :NN],
                                         start=(cg == 0), stop=(cg == 1))
                        nc.tensor.matmul(m2_ps[:, :NN], ones_p, xsq[cg][:, :NN],
                                         start=(cg == 0), stop=(cg == 1))
                    tmp = b5r.tile([128, 512], F32, tag="musq2")
                    nc.scalar.square(tmp[:, :NN], mu_ps[:, :NN])
                    nc.vector.tensor_tensor(out=tmp[:, :NN], in0=m2_ps[:, :NN],
                                            in1=tmp[:, :NN], op=Alu.subtract)
                    nc.scalar.activation(tmp[:, :NN], tmp[:, :NN], Act.Sqrt,
                                         bias=eps_p)
                    rstd = b5r.tile([128, 512], F32, tag="rstd2")
                    nc.vector.reciprocal(rstd[:, :NN], tmp[:, :NN])
                    for cg in range(2):
                        td2 = b5r.tile([96, 512], F32, tag=f"td2{cg}")
                        nc.vector.tensor_tensor(out=td2[:, :NN], in0=xpch[cg][:, :NN],
                                                in1=mu_ps[:96, :NN], op=Alu.subtract)
                        znt = b5r.tile([96, 512], BF16, tag=f"znt{cg}")
                        nc.vector.tensor_tensor(out=znt[:, :NN], in0=td2[:, :NN],
                                                in1=rstd[:96, :NN], op=Alu.mult)
                        nc.scalar.dma_start(out=zn_sp[cg][:, o0:o0 + NN],
                                            in_=znt[:, :NN])

            _wpab_cm.__exit__(None, None, None)
            # ================= PHASE C =================
            _wpc_cm = tc.tile_pool(name="wpC", bufs=1)
            wpc = _wpc_cm.__enter__()
            _load_w(wpc, C_ONLY_W)
            with tc.tile_pool(name="c_v0", bufs=1) as cv0:
                v0t1 = cv0.tile([128, P3], BF16)
                v0t2 = cv0.tile([128, P3], BF16)
                vgug = cv0.tile([128, P3], BF16)   # rows 0:64 = v-gelu0, 64:128 = u-gelu0
                with (
                    tc.tile_pool(name="c1_rot", bufs=2) as c1r,
                    tc.tile_pool(name="c1_ps", bufs=2, space="PSUM") as c1ps,
                ):
                    # pad cells must hold -t1/s1 so the bn-folded depthwise
                    # reads zeros in v0_bn space at image borders. Only the pad
                    # regions need initialization (interior is overwritten):
                    # top rows, bottom rows, and left/right columns per row.
                    def _pad_init(tl, padw, lcols, rcols):
                        for a, bnd in ((0, pd3(3)), (pd3(H + 3), P3)):
                            nc.vector.memset(tl[:, a:bnd], 0.0)
                            nc.vector.tensor_scalar_add(tl[:, a:bnd], tl[:, a:bnd],
                                                        padw)
                        for (p0, p1, c0_, c1_) in (lcols + rcols):
                            vv = tl[p0:p1, pd3(3):pd3(3 + H)] \
                                .rearrange("p (r w) -> p r w", w=Wp3)[:, :, c0_:c1_]
                            nc.vector.memset(vv, 0.0)
                            nc.vector.tensor_scalar_add(vv, vv, padw[p0:p1])

                    _pad_init(v0t1, ws["padv1"],
                              [(0, 128, 0, 3)], [(0, 128, 3 + W, Wp3)])
                    # v0t2 rows 64:128 are stored shifted +1 (interior written
                    # at cols [4, 4+W)), so their pads are cols [0,4) and
                    # [4+W, Wp3)
                    _pad_init(v0t2, ws["padv2"],
                              [(0, 64, 0, 3), (64, 128, 0, 4)],
                              [(0, 64, 3 + W, Wp3), (64, 128, 4 + W, Wp3)])
                    for ci in range(NCH):
                        c0 = ci * RC
                        nr_c = min(RC, H - c0)
                        NN = nr_c * W
                        o0 = c0 * W
                        d0 = pd3(3 + c0) + 3

                        def v0view(tl, lo, hi, shift=0):
                            return tl[lo:hi, d0 + shift:d0 + shift + nr_c * Wp3] \
                                .rearrange("p (r w) -> p r w", w=Wp3)[:, :, 0:W]

                        znch = [c1r.tile([96, RC * W], BF16, tag=f"cz{cg}",
                                         name=f"cz{cg}") for cg in range(2)]
                        for cg in range(2):
                            eng = nc.sync if cg == 0 else nc.scalar
                            eng.dma_start(out=znch[cg][:, :NN],
                                          in_=zn_sp[cg][:, o0:o0 + NN])
                        for mg in range(2):
                            fps = c1ps.tile([128, RC * W], F32, tag="fps")
                            for cg in range(2):
                                nc.tensor.matmul(
                                    fps[:, :NN],
                                    ws["fc1T"][:, (mg * 2 + cg) * 128:(mg * 2 + cg + 1) * 128],
                                    znch[cg][:, :NN], start=(cg == 0), stop=(cg == 1))
                            fv = lambda lo, hi: fps[lo:hi, :NN] \
                                .rearrange("p (r w) -> p r w", w=W)
                            if mg == 0:
                                nc.scalar.activation(
                                    v0view(vgug, 0, 64), fv(0, 64), Act.Gelu,
                                    bias=ws["fc1_bias"][0:64, 0:1])
                                nc.scalar.activation(
                                    v0view(vgug, 64, 128), v0view(vgug, 0, 64),
                                    Act.Gelu, bias=ws["g0_bias"], scale=ws["g0_scale"])
                                nc.scalar.activation(
                                    v0view(v0t1, 0, 64), fv(64, 128), Act.Gelu,
                                    bias=ws["fc1_bias"][64:128, 0:1])
                            else:
                                nc.scalar.activation(
                                    v0view(v0t1, 64, 128), fv(0, 64), Act.Gelu,
                                    bias=ws["fc1_bias"][0:64, 1:2])
                                nc.scalar.activation(
                                    v0view(v0t2, 0, 64), fv(64, 128), Act.Gelu,
                                    bias=ws["fc1_bias"][64:128, 1:2])
                                nc.scalar.activation(
                                    v0view(v0t2, 64, 128, shift=1), fv(64, 128),
                                    Act.Gelu, bias=ws["fc1_bias"][64:128, 1:2])

                if FP8DW_C:
                    v0t18 = cv0.tile([128, P3], FP8, name="v0t18")
                    v0t28 = cv0.tile([128, P3], FP8, name="v0t28")
                    nc.scalar.copy(v0t18[:], v0t1[:])
                    nc.vector.tensor_copy(out=v0t28[:], in_=v0t2[:])
                with (
                    tc.tile_pool(name="c2_rot", bufs=3) as c2r,
                    tc.tile_pool(name="c2_ps", bufs=2, space="PSUM") as c2ps,
                ):
                    for ci in range(NCH):
                        c0 = ci * RC
                        nr_c = min(RC, H - c0)
                        N = nr_c * Wp3
                        NN = nr_c * W
                        sb0 = pd3(3 + c0)
                        ps_a = c2ps.tile([128, RC * Wp3], F32, tag="psa")
                        if FP8DW_C:
                            poff = lambda t: (t // 5 - 2) * Wp3 + (t % 5 - 2)
                            for pi in range(13):
                                ta, tb = 2 * pi, 2 * pi + 1
                                oa = sb0 + poff(ta)
                                delta = (poff(tb) - poff(ta)) if tb < 25 else 1
                                lhsT = ws["pair8"][:, pi * 256:(pi + 1) * 256] \
                                    .rearrange("p (two m) -> p two m", two=2)
                                nc.tensor.matmul(
                                    ps_a[:, :N], lhsT,
                                    dr_rhs(v0t18[:, oa:oa + N], delta, N),
                                    start=(pi == 0), stop=(pi == 12),
                                    perf_mode=DR)
                        else:
                            for t in range(25):
                                dy, dx = t // 5 - 2, t % 5 - 2
                                o = sb0 + dy * Wp3 + dx
                                nc.tensor.matmul(ps_a[:, :N],
                                                 ws["pair_diag"][:, t * 128:(t + 1) * 128],
                                                 v0t1[:, o:o + N],
                                                 start=(t == 0), stop=(t == 24))
                        ps_b = c2ps.tile([64, RC * Wp3], F32, tag="psb")
                        if FP8DW_C:
                            nmeta = len(dw3_passes) // 2
                            for pi in range(nmeta):
                                (dya, dxaa, _), (dyb, dxab, _) = \
                                    dw3_passes[2 * pi], dw3_passes[2 * pi + 1]
                                oa = sb0 + dya * Wp3 + dxaa
                                delta = (dyb * Wp3 + dxab) - (dya * Wp3 + dxaa)
                                lhsT = ws["dw38"][:, pi * 128:(pi + 1) * 128] \
                                    .rearrange("p (two m) -> p two m", two=2)
                                nc.tensor.matmul(
                                    ps_b[:, :N], lhsT,
                                    dr_rhs(v0t28[:, oa:oa + N], delta, N),
                                    start=(pi == 0), stop=(pi == nmeta - 1),
                                    perf_mode=DR)
                        else:
                            for i, (dy, dxa, hasb) in enumerate(dw3_passes):
                                o = sb0 + dy * Wp3 + dxa
                                nc.tensor.matmul(ps_b[:, :N],
                                                 ws["dw3_diag"][:, i * 64:(i + 1) * 64],
                                                 v0t2[:, o:o + N],
                                                 start=(i == 0), stop=(i == len(dw3_passes) - 1))

                        def inner(ap_flat, lo, hi):
                            # interior view of a PSUM chunk (starts at free 0)
                            return ap_flat[lo:hi, :N].rearrange(
                                "p (r w) -> p r w", w=Wp3)[:, :, 3:3 + W]

                        def inner_v0(tl, lo, hi):
                            # interior view of the padded v0 buffers at this chunk
                            return tl[lo:hi, sb0:sb0 + N].rearrange(
                                "p (r w) -> p r w", w=Wp3)[:, :, 3:3 + W]

                        ug_a = c2r.tile([128, RC * W], BF16, tag="uga")
                        ug_b = c2r.tile([128, RC * W], BF16, tag="ugb")
                        vb_a = c2r.tile([128, RC * W], BF16, tag="vba")
                        vb_b = c2r.tile([128, RC * W], BF16, tag="vbb")
                        uv = lambda tl, lo, hi: tl[lo:hi, :NN] \
                            .rearrange("p (r w) -> p r w", w=W)
                        nc.scalar.copy(uv(ug_a, 0, 64), inner_v0(vgug, 64, 128))
                        nc.scalar.activation(uv(ug_a, 64, 128), inner(ps_a, 0, 64),
                                             Act.Gelu, bias=ws["pair_bias"][0:64])
                        nc.scalar.activation(uv(ug_b, 0, 64), inner(ps_a, 64, 128),
                                             Act.Gelu, bias=ws["pair_bias"][64:128])
                        nc.scalar.activation(uv(ug_b, 64, 128), inner(ps_b, 0, 64),
                                             Act.Gelu, bias=ws["dw3_bias"])
                        nc.gpsimd.tensor_scalar(out=uv(vb_a, 0, 64),
                                                in0=inner_v0(vgug, 0, 64),
                                                scalar1=ws["s1a"][0:64],
                                                scalar2=ws["t1a"][0:64],
                                                op0=Alu.mult, op1=Alu.add)
                        nc.gpsimd.tensor_scalar(out=uv(vb_a, 64, 128),
                                                in0=inner_v0(v0t1, 0, 64),
                                                scalar1=ws["s1a"][64:128],
                                                scalar2=ws["t1a"][64:128],
                                                op0=Alu.mult, op1=Alu.add)
                        nc.gpsimd.tensor_scalar(out=uv(vb_b, 0, 64),
                                                in0=inner_v0(v0t1, 64, 128),
                                                scalar1=ws["s1b"][0:64],
                                                scalar2=ws["t1b"][0:64],
                                                op0=Alu.mult, op1=Alu.add)
                        nc.gpsimd.tensor_scalar(out=uv(vb_b, 64, 128),
                                                in0=inner_v0(v0t2, 0, 64),
                                                scalar1=ws["s1b"][64:128],
                                                scalar2=ws["t1b"][64:128],
                                                op0=Alu.mult, op1=Alu.add)
                        z1a = c2r.tile([128, RC * W], BF16, tag="z1a")
                        z1b = c2r.tile([128, RC * W], BF16, tag="z1b")
                        nc.gpsimd.tensor_tensor(out=z1a[:, :NN], in0=ug_a[:, :NN],
                                                in1=vb_a[:, :NN], op=Alu.mult)
                        nc.vector.tensor_tensor(out=z1b[:, :NN], in0=ug_b[:, :NN],
                                                in1=vb_b[:, :NN], op=Alu.mult)
                        for cg in range(2):
                            ops = c2ps.tile([96, RC * W], F32, tag=f"ops{cg}")
                            nc.tensor.matmul(ops[:, :NN],
                                             ws["fc2aT"][:, (cg * 2) * 96:(cg * 2 + 1) * 96],
                                             z1a[:, :NN], start=True, stop=False)
                            nc.tensor.matmul(ops[:, :NN],
                                             ws["fc2aT"][:, (cg * 2 + 1) * 96:(cg * 2 + 2) * 96],
                                             z1b[:, :NN], start=False, stop=fc2b_zero)
                            if not fc2b_zero:
                                opsv = ops[:, :NN].rearrange("p (r w) -> p r w", w=W)
                                nc.tensor.matmul(opsv,
                                                 ws["fc2bT_g0"][:, cg * 96:(cg + 1) * 96],
                                                 inner_v0(vgug, 0, 64), start=False, stop=False)
                                nc.tensor.matmul(opsv,
                                                 ws["fc2bT_g12"][:, cg * 96:(cg + 1) * 96],
                                                 inner_v0(v0t1, 0, 128), start=False, stop=False)
                                nc.tensor.matmul(opsv,
                                                 ws["fc2bT_g3"][:, cg * 96:(cg + 1) * 96],
                                                 inner_v0(v0t2, 0, 64), start=False, stop=True)
                            xrch = c2r.tile([96, RC * W], BF16, tag=f"xr{cg}", bufs=1)
                            eng = nc.sync if cg == 0 else nc.scalar
                            eng.dma_start(out=xrch[:, :NN],
                                          in_=xcp_sp[cg][:, c0 * W:c0 * W + NN])
                            ob = c2r.tile([96, RC * W], F32, tag=f"ob{cg}", bufs=1)
                            nc.vector.tensor_scalar(out=ob[:, :NN], in0=ops[:, :NN],
                                                    scalar1=ws["s3v"][:, cg:cg + 1],
                                                    scalar2=ws["out_bias"][:, cg:cg + 1],
                                                    op0=Alu.mult, op1=Alu.add)
                            oc = c2r.tile([96, RC * W], F32, tag=f"oc{cg}", bufs=1)
                            nc.gpsimd.tensor_tensor(out=oc[:, :NN], in0=ob[:, :NN],
                                                    in1=xrch[:, :NN], op=Alu.add)
                            eng2 = nc.sync if cg == 0 else nc.scalar
                            eng2.dma_start(
                                out=out_t[cg * 96:(cg + 1) * 96,
                                          c0 * W:c0 * W + NN],
                                in_=oc[:, :NN])
            _wpc_cm.__exit__(None, None, None)
    return out_t.name


# ----------------------------------------------------------------------------
# host entry
# ----------------------------------------------------------------------------

_CACHE = {}
_FC2B_ZERO = {}


def make_program(H, W, n_cores, attn_scale, dw3_passes, fc2b_zero=None):
    if fc2b_zero is None:
        fc2b_zero = _FC2B_ZERO.get("v", False)
    key = (H, W, n_cores, round(attn_scale, 9), fc2b_zero)
    if key in _CACHE:
        return _CACHE[key]
    nc = bacc.Bacc("TRN2", target_bir_lowering=False, debug=False, num_devices=n_cores)
    out_name = build(nc, H, W, n_cores, attn_scale, dw3_passes, fc2b_zero=fc2b_zero)
    nc.compile()
    _CACHE[key] = (nc, out_name)
    return nc, out_name


def make_in_maps(inputs):
    x = np.asarray(inputs["x"], np.float32)
    B = x.shape[0]
    C = x.shape[-1]
    wdict = _prep_weights({k: np.asarray(v) for k, v in inputs.items()})
    _FC2B_ZERO["v"] = wdict["_fc2b_zero"][0]
    base = {}
    for k, (shp, d) in WSPEC.items():
        base["w_" + k] = wdict[k][0].reshape(shp)
    in_maps = []
    for b in range(B):
        m = dict(base)
        # channel-major [C, H*W] on device, bf16
        m["x"] = np.ascontiguousarray(x[b].reshape(-1, C).T).astype(BF16NP)
        in_maps.append(m)
    return in_maps, wdict


def kernel(**inputs):
    x = np.asarray(inputs["x"], np.float32)
    B, H, W, C = x.shape
    in_maps, wdict = make_in_maps(inputs)
    nc, out_name = make_program(H, W, B, wdict["_attn_scale"][0],
                                wdict["_dw3_passes"][0])
    res = bass_utils.run_bass_kernel_spmd(nc, in_maps, core_ids=list(range(B)))
    return np.stack([np.asarray(res.results[b][out_name], np.float32)
                     .reshape(C, H * W).T.reshape(H, W, C) for b in range(B)])


# revision 34
# speedup vs baseline: 1.2112x; 1.0043x over previous
"""Trainium2 Bass kernel for nn_Block_87351044866235 (sparse_attention).

Data-parallel over batch: 8 samples -> 8 NeuronCores. Channel-major
layout [C, H*W] on chip; depthwise convs as diagonal bf16 matmuls on
TensorE; 1x1 convs as bf16 matmuls; LN stats via ones-matmuls; q/k gram
via PE transposes + bf16 matmuls; dynamic-k gate mean via a scalar
AllReduce. Activation spills are bf16; v stays SBUF-resident.
"""
import sys, os

for _p in ("/opt/trn_rl_repo", "/root/.axon_site/_ro/trn_rl_repo"):
    if os.path.isdir(_p) and _p not in sys.path:
        sys.path.append(_p)

import numpy as np
import ml_dtypes
import concourse.bass as bass
import concourse.bacc as bacc
import concourse.tile as tile
from concourse import mybir
from concourse import bass_utils

try:
    from concourse import tile_utils as _tu
    _tu.max_sbuf_usage = 208 * 1024
except Exception:
    pass

dt = mybir.dt
Alu = mybir.AluOpType
Act = mybir.ActivationFunctionType
AX = mybir.AxisListType.X

EMBED, PDIM, HEADS, HID = 192, 96, 8, 256
CPH = PDIM // HEADS  # 12
SLOP = 8
RC = 3    # conv output rows per chunk
BR = 12   # rows per band

F32, F32R, BF16 = dt.float32, dt.float32r, dt.bfloat16
FP8 = dt.float8e4
BF16NP = ml_dtypes.bfloat16
FP8NP = ml_dtypes.float8_e4m3
FP8DW = False      # fp8 DoubleRow for the qkv_dw depthwise conv
FP8DW_C = False    # fp8 DoubleRow for the pair/dw3 FFN depthwise convs
DR = mybir.MatmulPerfMode.DoubleRow

# tap pairs for 3x3 depthwise as fp8 DoubleRow (2 taps per pass)
QPAIRS = [(0, 1), (2, 3), (4, 5), (6, 7), (8, None)]


def _ceil(a, b):
    return (a + b - 1) // b


# ----------------------------------------------------------------------------
# host-side weight prep: everything 2D [partitions, free]
# ----------------------------------------------------------------------------

def _prep_weights(p):
    w = {}
    f32 = lambda a: (np.ascontiguousarray(a, np.float32), F32)
    bfw = lambda a: (np.ascontiguousarray(np.asarray(a, np.float32)
                                          .astype(BF16NP)), BF16)
    eps_bn = 1e-5

    w["ident"] = f32(np.eye(128, dtype=np.float32))
    w["identb"] = bfw(np.eye(128, dtype=np.float32))

    # pos depthwise diag: [96, (t*2+cg)*96]
    pw = p["pos_w"][:, 0]  # [192,3,3]
    pos_d = np.zeros((96, 18 * 96), np.float32)
    for t in range(9):
        dy, dx = t // 3 - 1, t % 3 - 1
        for cg in range(2):
            pos_d[:, (t * 2 + cg) * 96:(t * 2 + cg + 1) * 96] = \
                np.diag(pw[cg * 96:(cg + 1) * 96, dy + 1, dx + 1])
    w["pos_diag"] = bfw(pos_d)
    w["pos_b"] = f32(p["pos_b"].reshape(2, 96).T)  # [96, 2]

    g1v, b1v = p["ln1_g"], p["ln1_b"]
    qw = p["qkv_w"][:, :, 0, 0]  # [288, 96]
    qw_eff = qw * g1v[None, :96]
    w["qkv_wT"] = bfw(np.concatenate(
        [qw_eff[j * 96:(j + 1) * 96].T for j in range(3)], axis=1))  # [96, 3*96]
    w["qkv_bias"] = f32((qw @ b1v[:96]).reshape(3, 96).T)  # [96, 3]

    qdw = p["qkv_dw_w"][:, 0]  # [288,3,3]
    qdw_d = np.zeros((96, 27 * 96), np.float32)
    for t in range(9):
        dy, dx = t // 3 - 1, t % 3 - 1
        for j in range(3):
            qdw_d[:, (t * 3 + j) * 96:(t * 3 + j + 1) * 96] = \
                np.diag(qdw[j * 96:(j + 1) * 96, dy + 1, dx + 1])
    w["qdw_diag"] = bfw(qdw_d)
    f8w = lambda a: (np.ascontiguousarray(
        np.clip(np.asarray(a, np.float32), -240, 240).astype(FP8NP)), FP8)
    qdw8 = np.zeros((96, len(QPAIRS) * 3 * 192), np.float32)
    for pi, (ta, tb) in enumerate(QPAIRS):
        for j in range(3):
            o = (pi * 3 + j) * 192
            qdw8[:, o:o + 96] = np.diag(qdw[j * 96:(j + 1) * 96,
                                            ta // 3, ta % 3])
            if tb is not None:
                qdw8[:, o + 96:o + 192] = np.diag(qdw[j * 96:(j + 1) * 96,
                                                      tb // 3, tb % 3])
    w["qdw8"] = f8w(qdw8)

    gw1 = p["gate_w1"][:, :, 0, 0]  # [96, 192]
    gw1_eff = gw1 * g1v[None, :]
    w["gate_w1T"] = bfw(np.concatenate(
        [gw1_eff[:, cg * 96:(cg + 1) * 96].T for cg in range(2)], axis=1))  # [96, 192]
    w["gate_b1"] = f32((p["gate_b1"] + gw1 @ b1v).reshape(96, 1))
    w["gate_w2T"] = bfw(p["gate_w2"][:, :, 0, 0].T.copy())  # [96,1]
    w["gate_b2"] = f32(p["gate_b2"].reshape(1, 1))

    pj = p["proj_w"][:, :, 0, 0]
    pj1, pj2 = pj[:, :96], pj[:, 96:] * g1v[None, 96:]
    w["proj1T"] = bfw(np.concatenate(
        [pj1[cg * 96:(cg + 1) * 96].T for cg in range(2)], axis=1))  # [96, 192]
    w["proj2T"] = bfw(np.concatenate(
        [pj2[cg * 96:(cg + 1) * 96].T for cg in range(2)], axis=1))
    w["proj_bias"] = f32((pj[:, 96:] @ b1v[96:]).reshape(2, 96).T)  # [96, 2]

    attn_scale = float(p["attn1"][0] + p["attn2"][0] + p["attn3"][0] + p["attn4"][0])
    w["_attn_scale"] = (attn_scale, None)
    w["tempvec"] = f32(np.repeat(p["temperature"].reshape(HEADS), CPH).reshape(96, 1))

    g2v, b2v = p["ln2_g"], p["ln2_b"]
    f1 = p["fc1_w"][:, :, 0, 0]  # [256, 192]
    f1_eff = f1 * g2v[None, :]
    fc1 = np.zeros((96, 4 * 128), np.float32)
    for mg in range(2):
        for cg in range(2):
            fc1[:, (mg * 2 + cg) * 128:(mg * 2 + cg + 1) * 128] = \
                f1_eff[mg * 128:(mg + 1) * 128, cg * 96:(cg + 1) * 96].T
    w["fc1T"] = bfw(fc1)
    w["fc1_bias"] = f32((f1 @ b2v).reshape(2, 128).T)  # [128, 2]

    s1 = p["bn1_g"] / np.sqrt(p["bn1_v"] + eps_bn)
    t1 = p["bn1_b"] - p["bn1_m"] * s1
    s2 = p["bn2_g"] / np.sqrt(p["bn2_v"] + eps_bn)
    t2 = p["bn2_b"] - p["bn2_m"] * s2
    s3 = p["bn3_g"] / np.sqrt(p["bn3_v"] + eps_bn)
    t3 = p["bn3_b"] - p["bn3_m"] * s3

    dw1w, dw2w, dw3w = p["dw1_w"][:, 0], p["dw2_w"][:, 0], p["dw3_w"][:, 0]
    dw1b, dw2b, dw3b = p["dw1_b"], p["dw2_b"], p["dw3_b"]
    s1g = [s1[i * 64:(i + 1) * 64] for i in range(4)]
    t1g = [t1[i * 64:(i + 1) * 64] for i in range(4)]

    pair_d = np.zeros((128, 25 * 128), np.float32)
    for t in range(25):
        dy, dx = t // 5 - 2, t % 5 - 2
        blk = np.zeros((128, 128), np.float32)
        d2 = dw2w[:, dy + 2, dx + 2] * s1g[2]
        if dy == 0 and dx == 0:
            d2 = d2 + s1g[2]
        blk[64:, 64:] = np.diag(d2)
        if -1 <= dy <= 1 and -1 <= dx <= 1:
            d1 = dw1w[:, dy + 1, dx + 1] * s1g[1]
            if dy == 0 and dx == 0:
                d1 = d1 + s1g[1]
            blk[:64, :64] = np.diag(d1)
        pair_d[:, t * 128:(t + 1) * 128] = blk
    w["pair_diag"] = bfw(pair_d)
    f8w = lambda a: (np.ascontiguousarray(
        np.clip(np.asarray(a, np.float32), -240, 240).astype(FP8NP)), FP8)
    npair8 = 13
    pair8 = np.zeros((128, npair8 * 256), np.float32)
    for pi in range(npair8):
        ta, tb = 2 * pi, 2 * pi + 1
        pair8[:, pi * 256:pi * 256 + 128] = pair_d[:, ta * 128:(ta + 1) * 128]
        if tb < 25:
            pair8[:, pi * 256 + 128:pi * 256 + 256] = \
                pair_d[:, tb * 128:(tb + 1) * 128]
    w["pair8"] = f8w(pair8)
    bc1 = t1g[1] * dw1w.sum((1, 2)) + dw1b + t1g[1]
    bc2 = t1g[2] * dw2w.sum((1, 2)) + dw2b + t1g[2]
    w["pair_bias"] = f32(np.concatenate([bc1, bc2]).reshape(128, 1))

    # rows 64:128 of v0t2 hold the same data stored shifted +1, so a read at
    # AP offset (dy, dxa) yields tap (dy, dxa-1) for those rows.
    dw3_passes = []
    for dy in range(-3, 4):
        for dxa in (-2, 0, 2):
            dw3_passes.append((dy, dxa, True))
        dw3_passes.append((dy, 3, False))
    dw3_d = np.zeros((128, len(dw3_passes) * 64), np.float32)
    for i, (dy, dxa, hasb) in enumerate(dw3_passes):
        wa = dw3w[:, dy + 3, dxa + 3] * s1g[3]
        if dy == 0 and dxa == 0:
            wa = wa + s1g[3]
        dw3_d[:64, i * 64:(i + 1) * 64] = np.diag(wa)
        if hasb:
            wb = dw3w[:, dy + 3, dxa - 1 + 3] * s1g[3]
            if dy == 0 and dxa - 1 == 0:
                wb = wb + s1g[3]
            dw3_d[64:, i * 64:(i + 1) * 64] = np.diag(wb)
    w["dw3_diag"] = bfw(dw3_d)
    ndw38 = len(dw3_passes) // 2
    dw38 = np.zeros((128, ndw38 * 128), np.float32)
    for pi in range(ndw38):
        dw38[:, pi * 128:pi * 128 + 64] = dw3_d[:, (2 * pi) * 64:(2 * pi + 1) * 64]
        dw38[:, pi * 128 + 64:pi * 128 + 128] = \
            dw3_d[:, (2 * pi + 1) * 64:(2 * pi + 2) * 64]
    w["dw38"] = f8w(dw38)
    w["_dw3_passes"] = (dw3_passes, None)
    w["dw3_bias"] = f32((t1g[3] * dw3w.sum((1, 2)) + dw3b + t1g[3]).reshape(64, 1))

    d0w, d0b = p["dw0_w"][:, 0, 0, 0], p["dw0_b"]
    w["g0_scale"] = f32(((d0w + 1.0) * s1g[0]).reshape(64, 1))
    w["g0_bias"] = f32(((d0w + 1.0) * t1g[0] + d0b).reshape(64, 1))

    f2 = p["fc2_w"][:, :, 0, 0]  # [192, 256]
    f2a = f2 * s2[None, :]
    f2b = f2 * (t2 * s1)[None, :]
    cstv = f2 @ (t2 * t1)
    fc2a = np.zeros((128, 4 * 96), np.float32)
    for cg in range(2):
        for kg in range(2):
            fc2a[:, (cg * 2 + kg) * 96:(cg * 2 + kg + 1) * 96] = \
                f2a[cg * 96:(cg + 1) * 96, kg * 128:(kg + 1) * 128].T
    w["fc2aT"] = bfw(fc2a)
    w["_fc2b_zero"] = (bool(np.all(f2b == 0.0)), None)
    w["fc2bT_g0"] = bfw(np.concatenate(
        [f2b[cg * 96:(cg + 1) * 96, 0:64].T for cg in range(2)], axis=1))    # [64, 192]
    w["fc2bT_g12"] = bfw(np.concatenate(
        [f2b[cg * 96:(cg + 1) * 96, 64:192].T for cg in range(2)], axis=1))  # [128, 192]
    w["fc2bT_g3"] = bfw(np.concatenate(
        [f2b[cg * 96:(cg + 1) * 96, 192:256].T for cg in range(2)], axis=1))  # [64, 192]
    w["s3v"] = f32(np.stack([s3[:96], s3[96:]], axis=1))          # [96, 2]
    ob = s3 * cstv + t3
    w["out_bias"] = f32(np.stack([ob[:96], ob[96:]], axis=1))     # [96, 2]

    sg = np.where(s1 == 0, 1.0, s1)
    padv = -t1 / sg
    w["padv1"] = f32(np.concatenate([padv[64:128], padv[128:192]]).reshape(128, 1))
    w["padv2"] = f32(np.concatenate([padv[192:256], padv[192:256]]).reshape(128, 1))
    w["s1a"] = f32(s1[:128].reshape(128, 1))
    w["s1b"] = f32(s1[128:].reshape(128, 1))
    w["t1a"] = f32(t1[:128].reshape(128, 1))
    w["t1b"] = f32(t1[128:].reshape(128, 1))

    w["ones_st"] = bfw(np.full((96, 128), 1.0 / EMBED, np.float32))
    w["epsv"] = f32(np.full((128, 1), 1e-6, np.float32))
    vm = np.zeros((96, 96), np.float32)
    for h in range(HEADS):
        vm[h * CPH:(h + 1) * CPH, h * CPH:(h + 1) * CPH] = 1.0
    w["vmask"] = f32(vm)
    return w


WSPEC = {
    "ident": ([128, 128], F32), "identb": ([128, 128], BF16),
    "pos_diag": ([96, 18 * 96], BF16),
    "pos_b": ([96, 2], F32), "qkv_wT": ([96, 3 * 96], BF16),
    "qkv_bias": ([96, 3], F32), "qdw_diag": ([96, 27 * 96], BF16),
    "qdw8": ([96, 5 * 3 * 192], FP8), "pair8": ([128, 13 * 256], FP8),
    "dw38": ([128, 14 * 128], FP8),
    "gate_w1T": ([96, 192], BF16), "gate_b1": ([96, 1], F32),
    "gate_w2T": ([96, 1], BF16), "gate_b2": ([1, 1], F32),
    "proj1T": ([96, 192], BF16), "proj2T": ([96, 192], BF16),
    "proj_bias": ([96, 2], F32), "tempvec": ([96, 1], F32),
    "fc1T": ([96, 4 * 128], BF16), "fc1_bias": ([128, 2], F32),
    "pair_diag": ([128, 25 * 128], BF16), "pair_bias": ([128, 1], F32),
    "dw3_diag": ([128, 28 * 64], BF16), "dw3_bias": ([64, 1], F32),
    "g0_scale": ([64, 1], F32), "g0_bias": ([64, 1], F32),
    "fc2aT": ([128, 4 * 96], BF16), "fc2bT_g0": ([64, 192], BF16),
    "fc2bT_g12": ([128, 192], BF16), "fc2bT_g3": ([64, 192], BF16),
    "s3v": ([96, 2], F32), "out_bias": ([96, 2], F32),
    "padv1": ([128, 1], F32),
    "padv2": ([128, 1], F32),
    "s1a": ([128, 1], F32), "s1b": ([128, 1], F32),
    "t1a": ([128, 1], F32), "t1b": ([128, 1], F32),
    "ones_st": ([96, 128], BF16),
    "epsv": ([128, 1], F32),
    "vmask": ([96, 96], F32),
}


# ----------------------------------------------------------------------------
# device kernel
# ----------------------------------------------------------------------------

def build(nc, H, W, n_cores, attn_scale, dw3_passes, fc2b_zero=False):
    S = H * W
    Wp1 = W + 2
    P1B = (BR + 2) * Wp1 + 2 * SLOP   # band buffer (pad1)
    Wp3, Hp3 = W + 6, H + 6
    P3 = Hp3 * Wp3 + 2 * SLOP
    NCH = _ceil(H, RC)
    NB = _ceil(H, BR)
    NSC = _ceil(S, 512)
    GCH = 512 // W                    # gate chunk rows (512 cols)
    NGC_PER_BAND = _ceil(BR, GCH)

    # x and out are channel-major [EMBED, S]; host transposes NHWC<->CM
    x_t = nc.dram_tensor("x", [EMBED, S], BF16, kind="ExternalInput")
    out_t = nc.dram_tensor("out", [EMBED, S], F32, kind="ExternalOutput")
    wt = {k: nc.dram_tensor("w_" + k, shp, d, kind="ExternalInput")
          for k, (shp, d) in WSPEC.items()}

    def pd3(r):
        return SLOP + r * Wp3

    def dr_rhs(base2d, delta, n):
        # [P, 2, n] view with an overlapping middle dim of stride `delta`
        ap = [list(p) for p in base2d.ap]
        return bass.AP(base2d.tensor, base2d.offset,
                       [ap[0], [delta, 2], [1, n]])

    with tile.TileContext(nc) as tc:
        C_ONLY_W = ['pair8', 'dw38',
                    'fc1T', 'fc1_bias', 'pair_diag', 'pair_bias', 'dw3_diag',
                    'dw3_bias', 'g0_scale', 'g0_bias', 'fc2aT', 'fc2bT_g0',
                    'fc2bT_g12', 'fc2bT_g3', 's3v', 'out_bias', 's1a', 's1b',
                    't1a', 't1b', 'padv1', 'padv2']
        PERS_W = ['ones_st', 'epsv']
        with (
            tc.tile_pool(name="dram", bufs=1, space="DRAM") as dram,
            tc.tile_pool(name="persist", bufs=1) as pers,
        ):
            ws = {}

            def _load_w(pool, names):
                for k in names:
                    shp, d = WSPEC[k]
                    tl = pool.tile(shp, d, tag="w_" + k, name="w_" + k)
                    nc.sync.dma_start(out=tl[:], in_=wt[k][:])
                    ws[k] = tl

            yn1_sp = dram.tile([96, S], BF16)
            yn2_sp = dram.tile([96, S], BF16)
            xc_sp = [dram.tile([96, S], BF16, name=f"xc_sp{i}") for i in range(2)]
            xcp_sp = [dram.tile([96, S], BF16, name=f"xcp_sp{i}") for i in range(2)]
            zn_sp = [dram.tile([96, S], BF16, name=f"zn_sp{i}") for i in range(2)]
            cc_in = dram.tile([1, 1], F32)
            cc_out = dram.tile([1, 1], F32)

            gsum = pers.tile([1, NB * NGC_PER_BAND + 8], F32)
            nc.vector.memset(gsum[:], 0.0)
            dynk = pers.tile([96, 1], F32)
            probsT = pers.tile([96, 96], BF16)
            # persistent copies of LN helpers (used in phases A, B5 and C)
            ones_p = pers.tile([96, 128], BF16, name="p_ones")
            nc.sync.dma_start(out=ones_p[:], in_=wt["ones_st"][:])
            eps_p = pers.tile([128, 1], F32, name="p_eps")
            nc.sync.dma_start(out=eps_p[:], in_=wt["epsv"][:])
            # big memsets are pathologically slow; keep one zeroed band tile
            # and clear band buffers with fast engine copies instead
            zt = pers.tile([96, (BR + 2) * (W + 2) + 2 * SLOP], BF16, name="p_zero")
            nc.vector.memset(zt[:], 0.0)

            # ================= PHASE A =================
            _wpab_cm = tc.tile_pool(name="wpAB", bufs=1)
            wpab = _wpab_cm.__enter__()
            _load_w(wpab, [k for k in WSPEC
                           if k not in C_ONLY_W and k not in PERS_W])
            ident = ws["ident"]
            identb = ws["identb"]
            # v stays SBUF-resident through phase B5
            vres = wpab.tile([96, S], BF16, name="vres")
            with (
                tc.tile_pool(name="pa_band", bufs=3) as pab,
                tc.tile_pool(name="pa_rot", bufs=4) as par,
                tc.tile_pool(name="pa_ps", bufs=2, space="PSUM") as paps,
            ):
                for b in range(NB):
                    r0, r1 = b * BR, min((b + 1) * BR, H)
                    ylo, yhi = max(r0 - 1, 0), min(r1 + 1, H)
                    nr = yhi - ylo
                    boff = SLOP + (ylo - (r0 - 1)) * Wp1 + 1
                    xband = [pab.tile([96, P1B], BF16, tag=f"xb{cg}",
                                      name=f"xb{cg}") for cg in range(2)]
                    for cg in range(2):
                        # pad cells must be zero; buffers rotate with bufs=2 so
                        # zero each physical buffer once, then re-zero only the
                        # bottom halo row slot for the final band
                        if b < 3:
                            nc.scalar.copy(xband[cg][:], zt[:])
                        elif b == NB - 1:
                            ze = min(SLOP + (nr + 1) * Wp1 + SLOP, P1B)
                            zs = SLOP + nr * Wp1
                            nc.scalar.copy(xband[cg][:, zs:ze], zt[:, zs:ze])
                        dst = xband[cg][:, boff:boff + nr * Wp1] \
                            .rearrange("p (r w) -> p r w", w=Wp1)[:, :, 0:W]
                        src = x_t[cg * 96:(cg + 1) * 96, ylo * W:yhi * W] \
                            .rearrange("p (r w) -> p r w", w=W)
                        eng = nc.sync if cg == 0 else nc.scalar
                        eng.dma_start(out=dst, in_=src)
                    for c0 in range(r0, r1, RC):
                        nr_c = min(RC, H - c0)
                        N = nr_c * Wp1
                        NN = nr_c * W
                        sb0 = SLOP + (c0 - r0 + 1) * Wp1
                        xc_ch = [par.tile([96, RC * W], BF16, tag=f"xc{cg}",
                                          name=f"xc{cg}") for cg in range(2)]
                        xsq = [par.tile([96, RC * W], BF16, tag=f"xq{cg}",
                                        name=f"xq{cg}") for cg in range(2)]
                        for cg in range(2):
                            ps = paps.tile([96, RC * Wp1], F32, tag="posps")
                            for t in range(9):
                                dy, dx = t // 3 - 1, t % 3 - 1
                                o = sb0 + dy * Wp1 + dx
                                nc.tensor.matmul(
                                    ps[:, :N],
                                    ws["pos_diag"][:, (t * 2 + cg) * 96:(t * 2 + cg + 1) * 96],
                                    xband[cg][:, o:o + N],
                                    start=(t == 0), stop=(t == 8))
                            ps_int = ps[:, :N].rearrange("p (r w) -> p r w", w=Wp1)[:, :, 1:1 + W]
                            xb_int = xband[cg][:, sb0:sb0 + N] \
                                .rearrange("p (r w) -> p r w", w=Wp1)[:, :, 1:1 + W]
                            xcv = xc_ch[cg][:, :NN].rearrange("p (r w) -> p r w", w=W)
                            nc.vector.scalar_tensor_tensor(
                                out=xcv, in0=ps_int, scalar=ws["pos_b"][:, cg:cg + 1],
                                in1=xb_int, op0=Alu.add, op1=Alu.add)
                            nc.scalar.square(xsq[cg][:, :NN], xc_ch[cg][:, :NN])
                        mu_ps = paps.tile([128, RC * W], F32, tag="mups")
                        m2_ps = paps.tile([128, RC * W], F32, tag="m2ps")
                        for cg in range(2):
                            nc.tensor.matmul(mu_ps[:, :NN], ones_p, xc_ch[cg][:, :NN],
                                             start=(cg == 0), stop=(cg == 1))
                            nc.tensor.matmul(m2_ps[:, :NN], ones_p, xsq[cg][:, :NN],
                                             start=(cg == 0), stop=(cg == 1))
                        tmp = par.tile([128, RC * W], F32, tag="musq")
                        nc.scalar.square(tmp[:, :NN], mu_ps[:, :NN])
                        nc.vector.tensor_tensor(out=tmp[:, :NN], in0=m2_ps[:, :NN],
                                                in1=tmp[:, :NN], op=Alu.subtract)
                        nc.scalar.activation(tmp[:, :NN], tmp[:, :NN], Act.Sqrt,
                                             bias=eps_p)
                        rstd = par.tile([128, RC * W], F32, tag="rstd")
                        nc.vector.reciprocal(rstd[:, :NN], tmp[:, :NN])
                        for cg in range(2):
                            tdf = par.tile([96, RC * W], F32, tag=f"td{cg}")
                            nc.vector.tensor_tensor(out=tdf[:, :NN], in0=xc_ch[cg][:, :NN],
                                                    in1=mu_ps[:96, :NN], op=Alu.subtract)
                            ynch = par.tile([96, RC * W], BF16, tag=f"yn{cg}")
                            nc.vector.tensor_tensor(out=ynch[:, :NN], in0=tdf[:, :NN],
                                                    in1=rstd[:96, :NN], op=Alu.mult)
                            sp = yn1_sp if cg == 0 else yn2_sp
                            nc.sync.dma_start(out=sp[:, c0 * W:c0 * W + NN],
                                              in_=ynch[:, :NN])
                            nc.scalar.dma_start(out=xc_sp[cg][:, c0 * W:c0 * W + NN],
                                                in_=xc_ch[cg][:, :NN])

            # ================= PHASE B =================
            with (
                tc.tile_pool(name="pb_band", bufs=1) as pbb,
                tc.tile_pool(name="pb_rot", bufs=4) as pbr,
                tc.tile_pool(name="gram_ps", bufs=1, space="PSUM") as gpsp,
            ):
                g1_ps = gpsp.tile([96, 192], F32)
                g2_ps = gpsp.tile([96, 96], F32)
                with (
                    tc.tile_pool(name="pb_psg", bufs=1, space="PSUM") as pbpsg,
                    tc.tile_pool(name="pb_ps", bufs=2, space="PSUM") as pbps,
                ):
                    for b in range(NB):
                        r0, r1 = b * BR, min((b + 1) * BR, H)
                        ylo, yhi = max(r0 - 1, 0), min(r1 + 1, H)
                        ynb = [pbb.tile([96, (BR + 2) * W], BF16, tag=f"ynb{cg}",
                                        name=f"ynb{cg}") for cg in range(2)]
                        for cg in range(2):
                            sp = yn1_sp if cg == 0 else yn2_sp
                            eng = nc.sync if cg == 0 else nc.scalar
                            eng.dma_start(
                                out=ynb[cg][:, (ylo - r0 + 1) * W:(yhi - r0 + 1) * W],
                                in_=sp[:, ylo * W:yhi * W])
                        # gate (512-col chunks over rows [r0, r1))
                        for gi in range(NGC_PER_BAND):
                            gr0 = r0 + gi * GCH
                            if gr0 >= r1:
                                break
                            ngr = min(GCH, r1 - gr0)
                            NG = ngr * W
                            yo = (gr0 - r0 + 1) * W
                            gps = pbpsg.tile([96, 512], F32, tag="gps")
                            for cg in range(2):
                                nc.tensor.matmul(gps[:, :NG],
                                                 ws["gate_w1T"][:, cg * 96:(cg + 1) * 96],
                                                 ynb[cg][:, yo:yo + NG],
                                                 start=(cg == 0), stop=(cg == 1))
                            g1s = pbr.tile([96, 512], BF16, tag="g1s")
                            nc.scalar.activation(g1s[:, :NG], gps[:, :NG], Act.Relu,
                                                 bias=ws["gate_b1"])
                            g2ps = pbpsg.tile([96, 512], F32, tag="gps")
                            nc.tensor.matmul(g2ps[0:1, :NG], ws["gate_w2T"], g1s[:, :NG],
                                             start=True, stop=True)
                            sgt = pbr.tile([1, 512], F32, tag="sgt")
                            idx = b * NGC_PER_BAND + gi
                            nc.scalar.activation(sgt[:, :NG], g2ps[0:1, :NG], Act.Sigmoid,
                                                 bias=ws["gate_b2"],
                                                 accum_out=gsum[0:1, idx:idx + 1])
                        # qkv0 band
                        qkv0 = [pbb.tile([96, P1B], FP8 if FP8DW else BF16,
                                         tag=f"qk0{j}", name=f"qk0{j}")
                                for j in range(3)]
                        nrq = yhi - ylo
                        for j in range(3):
                            # single physical buffer (bufs=1): zero fully on the
                            # first band; re-zero only the stale bottom slots on
                            # the final band
                            if b == 0:
                                nc.scalar.copy(qkv0[j][:], zt[:])
                            elif b == NB - 1:
                                ze = min(SLOP + (nrq + 1) * Wp1 + SLOP, P1B)
                                zs = SLOP + nrq * Wp1
                                nc.scalar.copy(qkv0[j][:, zs:ze], zt[:, zs:ze])
                        for rr in range(ylo, yhi, 2):
                            nrw = min(2, yhi - rr)
                            NQ = nrw * W
                            for j in range(3):
                                qps = pbps.tile([96, 2 * W], F32, tag="qps")
                                nc.tensor.matmul(qps[:, :NQ],
                                                 ws["qkv_wT"][:, j * 96:(j + 1) * 96],
                                                 ynb[0][:, (rr - r0 + 1) * W:(rr - r0 + 1) * W + NQ],
                                                 start=True, stop=True)
                                dst = SLOP + (rr - r0 + 1) * Wp1 + 1
                                dview = qkv0[j][:, dst:dst + nrw * Wp1] \
                                    .rearrange("p (r w) -> p r w", w=Wp1)[:, :, 0:W]
                                nc.scalar.activation(
                                    dview, qps[:, :NQ].rearrange("p (r w) -> p r w", w=W),
                                    Act.Identity, bias=ws["qkv_bias"][:, j:j + 1])
                        # depthwise; q/k transposed into qkband via PE
                        qkband = pbr.tile([W, BR * 192], BF16, tag="qkband")
                        for c0 in range(r0, r1, RC):
                            nr_c = min(RC, H - c0)
                            N = nr_c * Wp1
                            NN = nr_c * W
                            sb0 = SLOP + (c0 - r0 + 1) * Wp1
                            qk = {}
                            for j in range(3):
                                ps = pbps.tile([96, RC * Wp1], F32, tag="dwps")
                                if FP8DW:
                                    toff = lambda t: (t // 3 - 1) * Wp1 + (t % 3 - 1)
                                    for pi, (ta, tb) in enumerate(QPAIRS):
                                        oa = sb0 + toff(ta)
                                        delta = (toff(tb) - toff(ta)) \
                                            if tb is not None else 1
                                        lhsT = ws["qdw8"][:, (pi * 3 + j) * 192:
                                                          (pi * 3 + j + 1) * 192] \
                                            .rearrange("p (two m) -> p two m", two=2)
                                        nc.tensor.matmul(
                                            ps[:, :N], lhsT,
                                            dr_rhs(qkv0[j][:, oa:oa + N], delta, N),
                                            start=(pi == 0),
                                            stop=(pi == len(QPAIRS) - 1),
                                            perf_mode=DR)
                                else:
                                    for t in range(9):
                                        dy, dx = t // 3 - 1, t % 3 - 1
                                        o = sb0 + dy * Wp1 + dx
                                        nc.tensor.matmul(
                                            ps[:, :N],
                                            ws["qdw_diag"][:, (t * 3 + j) * 96:(t * 3 + j + 1) * 96],
                                            qkv0[j][:, o:o + N],
                                            start=(t == 0), stop=(t == 8))
                                ps_int = ps[:, :N].rearrange("p (r w) -> p r w", w=Wp1)[:, :, 1:1 + W]
                                if j == 2:
                                    nc.scalar.copy(
                                        vres[:, c0 * W:c0 * W + NN]
                                        .rearrange("p (r w) -> p r w", w=W), ps_int)
                                else:
                                    qb = pbr.tile([96, RC * W], BF16, tag=f"qb{j}")
                                    nc.scalar.copy(
                                        qb[:, :NN].rearrange("p (r w) -> p r w", w=W),
                                        ps_int)
                                    qk[j] = qb
                            for rr in range(c0, c0 + nr_c):
                                ro = (rr - r0) * 192
                                rl = (rr - c0) * W
                                for j in range(2):
                                    tps = pbpsg.tile([128, 96], BF16, tag="tps")
                                    nc.tensor.transpose(tps[:], qk[j][:, rl:rl + W],
                                                        identb[:96, :96])
                                    nc.scalar.copy(
                                        qkband[:, ro + j * 96:ro + (j + 1) * 96],
                                        tps[:])
                        for rr in range(r0, r1):
                            ro = (rr - r0) * 192
                            nc.tensor.matmul(g1_ps[:], qkband[:, ro:ro + 96],
                                             qkband[:, ro:ro + 192],
                                             start=(rr == 0), stop=(rr == H - 1))
                            nc.tensor.matmul(g2_ps[:], qkband[:, ro + 96:ro + 192],
                                             qkband[:, ro + 96:ro + 192],
                                             start=(rr == 0), stop=(rr == H - 1))

                # ---- gate mean -> AllReduce -> dynk ----
                gred = pers.tile([1, 1], F32)
                nc.vector.reduce_sum(gred[:], gsum[0:1, 0:NB * NGC_PER_BAND], axis=AX)
                gsc = pers.tile([1, 1], F32)
                nc.vector.tensor_scalar_mul(gsc[:], gred[:], float(CPH) / (n_cores * S))
                nc.sync.dma_start(out=cc_in[:], in_=gsc[:])
                nc.gpsimd.collective_compute(
                    "AllReduce", Alu.add, replica_groups=[list(range(n_cores))],
                    ins=[cc_in.opt()], outs=[cc_out.opt()])
                nc.sync.dma_start(out=dynk[:], in_=cc_out[:].partition_broadcast(96))

                # ---- attn block ----
                with (
                    tc.tile_pool(name="at_ps", bufs=2, space="PSUM") as atps,
                    tc.tile_pool(name="at_sb", bufs=1) as ab,
                ):
                    g1sb = ab.tile([96, 192], F32)
                    nc.scalar.copy(g1sb[:], g1_ps[:])
                    g2sb = ab.tile([96, 96], F32)
                    nc.scalar.copy(g2sb[:], g2_ps[:])
                    idm = ident[:96, :96]
                    tq = ab.tile([96, 96], F32)
                    nc.vector.tensor_tensor(out=tq[:], in0=g1sb[:, 0:96], in1=idm,
                                            op=Alu.mult)
                    nq2 = ab.tile([96, 1], F32)
                    nc.vector.reduce_sum(nq2[:], tq[:], axis=AX)
                    tk = ab.tile([96, 96], F32)
                    nc.vector.tensor_tensor(out=tk[:], in0=g2sb[:], in1=idm,
                                            op=Alu.mult)
                    nk2 = ab.tile([96, 1], F32)
                    nc.vector.reduce_sum(nk2[:], tk[:], axis=AX)

                    def rsqrt_clamped(nm, src):
                        sq = ab.tile([96, 1], F32, tag=nm + "sq")
                        nc.scalar.sqrt(sq[:], src[:])
                        cl = ab.tile([96, 1], F32, tag=nm + "cl")
                        nc.vector.tensor_scalar_max(cl[:], sq[:], 1e-12)
                        rvv = ab.tile([96, 1], F32, tag=nm)
                        nc.vector.reciprocal(rvv[:], cl[:])
                        return rvv

                    rq = rsqrt_clamped("rq", nq2)
                    rk = rsqrt_clamped("rk", nk2)
                    rqt = ab.tile([96, 1], F32)
                    nc.vector.tensor_tensor(out=rqt[:], in0=rq[:], in1=ws["tempvec"][:],
                                            op=Alu.mult)
                    asr = ab.tile([96, 96], F32)
                    nc.vector.tensor_scalar_mul(asr[:], g1sb[:, 96:192], rqt[:])
                    as_ps = atps.tile([96, 96], F32, tag="atp")
                    nc.tensor.transpose(as_ps[:], asr[:], ident[:96, :96])
                    ast = ab.tile([96, 96], F32)
                    nc.vector.tensor_scalar_mul(ast[:], as_ps[:], rk[:])
                    as2_ps = atps.tile([96, 96], F32, tag="atp")
                    nc.tensor.transpose(as2_ps[:], ast[:], ident[:96, :96])
                    as2 = ab.tile([96, 96], F32)
                    nc.scalar.copy(as2[:], as2_ps[:])
                    # mask off-head-block entries to -60
                    t60 = ab.tile([96, 96], F32)
                    nc.vector.tensor_scalar_add(t60[:], as2[:], 60.0)
                    amf = ab.tile([96, 96], F32)
                    nc.vector.tensor_tensor(out=amf[:], in0=t60[:], in1=ws["vmask"][:],
                                            op=Alu.mult)
                    nc.vector.tensor_scalar_add(amf[:], amf[:], -60.0)
                    # rank+1 over full row via pairwise is_ge
                    rnk3 = ab.tile([96, 96 * 96], F32)
                    a_i = amf[:].unsqueeze(1).broadcast_to([96, 96, 96])
                    a_d = amf[:].unsqueeze(2).broadcast_to([96, 96, 96])
                    rvw = rnk3[:].rearrange("p (i d) -> p i d", d=96)
                    nc.vector.tensor_tensor(out=rvw, in0=a_i, in1=a_d, op=Alu.is_ge)
                    rank1 = ab.tile([96, 96], F32)
                    nc.vector.reduce_sum(rank1[:].unsqueeze(2), rvw, axis=AX)
                    sel = ab.tile([96, 96], F32)
                    nc.vector.tensor_tensor(out=sel[:], in0=rank1[:],
                                            in1=dynk[:].broadcast_to([96, 96]), op=Alu.is_le)
                    am = ab.tile([96, 96], F32)
                    t60b = ab.tile([96, 96], F32)
                    nc.vector.tensor_scalar_add(t60b[:], amf[:], 60.0)
                    nc.vector.tensor_tensor(out=am[:], in0=t60b[:], in1=sel[:], op=Alu.mult)
                    nc.vector.tensor_scalar_add(am[:], am[:], -60.0)
                    mx = ab.tile([96, 1], F32)
                    nc.vector.reduce_max(mx[:], am[:], axis=AX)
                    nmx = ab.tile([96, 1], F32)
                    nc.vector.tensor_scalar_mul(nmx[:], mx[:], -1.0)
                    ex = ab.tile([96, 96], F32)
                    nc.scalar.activation(ex[:], am[:], Act.Exp, bias=nmx[:])
                    sme = ab.tile([96, 1], F32)
                    nc.vector.reduce_sum(sme[:], ex[:], axis=AX)
                    rsm = ab.tile([96, 1], F32)
                    nc.vector.reciprocal(rsm[:], sme[:])
                    probs = ab.tile([96, 96], F32)
                    nc.vector.tensor_scalar_mul(probs[:], ex[:], rsm[:])
                    pt_ps = atps.tile([96, 96], F32, tag="atp2")
                    nc.tensor.transpose(pt_ps[:], probs[:], ident[:96, :96])
                    nc.scalar.copy(probsT[:], pt_ps[:])

            # ================= PHASE B5 =================
            with (
                tc.tile_pool(name="b5_rot", bufs=4) as b5r,
                tc.tile_pool(name="b5_ps", bufs=1, space="PSUM") as b5ps,
            ):
                for ci in range(NSC):
                    o0 = ci * 512
                    NN = min(512, S - o0)
                    av_ps = b5ps.tile([96, 512], F32, tag="avps")
                    nc.tensor.matmul(av_ps[:, :NN], probsT[:], vres[:, o0:o0 + NN],
                                     start=True, stop=True)
                    avs = b5r.tile([96, 512], BF16, tag="avs")
                    nc.scalar.activation(avs[:, :NN], av_ps[:, :NN], Act.Copy,
                                         scale=attn_scale)
                    x2ch = b5r.tile([96, 512], BF16, tag="x2ch")
                    nc.sync.dma_start(out=x2ch[:, :NN], in_=yn2_sp[:, o0:o0 + NN])
                    xpch = [b5r.tile([96, 512], BF16, tag=f"xp{cg}", name=f"xp{cg}")
                            for cg in range(2)]
                    xsq = [b5r.tile([96, 512], BF16, tag=f"xs{cg}", name=f"xs{cg}")
                           for cg in range(2)]
                    for cg in range(2):
                        xcch = b5r.tile([96, 512], BF16, tag=f"xcc{cg}")
                        nc.scalar.dma_start(out=xcch[:, :NN], in_=xc_sp[cg][:, o0:o0 + NN])
                        pj_ps = b5ps.tile([96, 512], F32, tag=f"pjps{cg}")
                        nc.tensor.matmul(pj_ps[:, :NN],
                                         ws["proj2T"][:, cg * 96:(cg + 1) * 96],
                                         x2ch[:, :NN], start=True, stop=False)
                        nc.tensor.matmul(pj_ps[:, :NN],
                                         ws["proj1T"][:, cg * 96:(cg + 1) * 96],
                                         avs[:, :NN], start=False, stop=True)
                        nc.vector.scalar_tensor_tensor(
                            out=xpch[cg][:, :NN], in0=pj_ps[:, :NN],
                            scalar=ws["proj_bias"][:, cg:cg + 1], in1=xcch[:, :NN],
                            op0=Alu.add, op1=Alu.add)
                        nc.sync.dma_start(out=xcp_sp[cg][:, o0:o0 + NN],
                                          in_=xpch[cg][:, :NN])
                        nc.scalar.square(xsq[cg][:, :NN], xpch[cg][:, :NN])
                    mu_ps = b5ps.tile([128, 512], F32, tag="mu2ps")
                    m2_ps = b5ps.tile([128, 512], F32, tag="m22ps")
                    for cg in range(2):
                        nc.tensor.matmul(mu_ps[:, :NN], ones_p, xpch[cg][:, :NN],
                                         start=(cg == 0), stop=(cg == 1))
                        nc.tensor.matmul(m2_ps[:, :NN], ones_p, xsq[cg][:, :NN],
                                         start=(cg == 0), stop=(cg == 1))
                    tmp = b5r.tile([128, 512], F32, tag="musq2")
                    nc.scalar.square(tmp[:, :NN], mu_ps[:, :NN])
                    nc.vector.tensor_tensor(out=tmp[:, :NN], in0=m2_ps[:, :NN],
                                            in1=tmp[:, :NN], op=Alu.subtract)
                    nc.scalar.activation(tmp[:, :NN], tmp[:, :NN], Act.Sqrt,
                                         bias=eps_p)
                    rstd = b5r.tile([128, 512], F32, tag="rstd2")
                    nc.vector.reciprocal(rstd[:, :NN], tmp[:, :NN])
                    for cg in range(2):
                        td2 = b5r.tile([96, 512], F32, tag=f"td2{cg}")
                        nc.vector.tensor_tensor(out=td2[:, :NN], in0=xpch[cg][:, :NN],
                                                in1=mu_ps[:96, :NN], op=Alu.subtract)
                        znt = b5r.tile([96, 512], BF16, tag=f"znt{cg}")
                        nc.vector.tensor_tensor(out=znt[:, :NN], in0=td2[:, :NN],
                                                in1=rstd[:96, :NN], op=Alu.mult)
                        nc.scalar.dma_start(out=zn_sp[cg][:, o0:o0 + NN],
                                            in_=znt[:, :NN])

            _wpab_cm.__exit__(None, None, None)
            # ================= PHASE C =================
            _wpc_cm = tc.tile_pool(name="wpC", bufs=1)
            wpc = _wpc_cm.__enter__()
            _load_w(wpc, C_ONLY_W)
            with tc.tile_pool(name="c_v0", bufs=1) as cv0:
                v0t1 = cv0.tile([128, P3], BF16)
                v0t2 = cv0.tile([128, P3], BF16)
                vgug = cv0.tile([128, P3], BF16)   # rows 0:64 = v-gelu0, 64:128 = u-gelu0
                with (
                    tc.tile_pool(name="c1_rot", bufs=2) as c1r,
                    tc.tile_pool(name="c1_ps", bufs=2, space="PSUM") as c1ps,
                ):
                    # pad cells must hold -t1/s1 so the bn-folded depthwise
                    # reads zeros in v0_bn space at image borders. Only the pad
                    # regions need initialization (interior is overwritten):
                    # top rows, bottom rows, and left/right columns per row.
                    def _pad_init(tl, padw, lcols, rcols):
                        for a, bnd in ((0, pd3(3)), (pd3(H + 3), P3)):
                            nc.vector.memset(tl[:, a:bnd], 0.0)
                            nc.vector.tensor_scalar_add(tl[:, a:bnd], tl[:, a:bnd],
                                                        padw)
                        for (p0, p1, c0_, c1_) in (lcols + rcols):
                            vv = tl[p0:p1, pd3(3):pd3(3 + H)] \
                                .rearrange("p (r w) -> p r w", w=Wp3)[:, :, c0_:c1_]
                            nc.vector.memset(vv, 0.0)
                            nc.vector.tensor_scalar_add(vv, vv, padw[p0:p1])

                    _pad_init(v0t1, ws["padv1"],
                              [(0, 128, 0, 3)], [(0, 128, 3 + W, Wp3)])
                    # v0t2 rows 64:128 are stored shifted +1 (interior written
                    # at cols [4, 4+W)), so their pads are cols [0,4) and
                    # [4+W, Wp3)
                    _pad_init(v0t2, ws["padv2"],
                              [(0, 64, 0, 3), (64, 128, 0, 4)],
                              [(0, 64, 3 + W, Wp3), (64, 128, 4 + W, Wp3)])
                    for ci in range(NCH):
                        c0 = ci * RC
                        nr_c = min(RC, H - c0)
                        NN = nr_c * W
                        o0 = c0 * W
                        d0 = pd3(3 + c0) + 3

                        def v0view(tl, lo, hi, shift=0):
                            return tl[lo:hi, d0 + shift:d0 + shift + nr_c * Wp3] \
                                .rearrange("p (r w) -> p r w", w=Wp3)[:, :, 0:W]

                        znch = [c1r.tile([96, RC * W], BF16, tag=f"cz{cg}",
                                         name=f"cz{cg}") for cg in range(2)]
                        for cg in range(2):
                            eng = nc.sync if cg == 0 else nc.scalar
                            eng.dma_start(out=znch[cg][:, :NN],
                                          in_=zn_sp[cg][:, o0:o0 + NN])
                        for mg in range(2):
                            fps = c1ps.tile([128, RC * W], F32, tag="fps")
                            for cg in range(2):
                                nc.tensor.matmul(
                                    fps[:, :NN],
                                    ws["fc1T"][:, (mg * 2 + cg) * 128:(mg * 2 + cg + 1) * 128],
                                    znch[cg][:, :NN], start=(cg == 0), stop=(cg == 1))
                            fv = lambda lo, hi: fps[lo:hi, :NN] \
                                .rearrange("p (r w) -> p r w", w=W)
                            if mg == 0:
                                nc.scalar.activation(
                                    v0view(vgug, 0, 64), fv(0, 64), Act.Gelu,
                                    bias=ws["fc1_bias"][0:64, 0:1])
                                nc.scalar.activation(
                                    v0view(vgug, 64, 128), v0view(vgug, 0, 64),
                                    Act.Gelu, bias=ws["g0_bias"], scale=ws["g0_scale"])
                                nc.scalar.activation(
                                    v0view(v0t1, 0, 64), fv(64, 128), Act.Gelu,
                                    bias=ws["fc1_bias"][64:128, 0:1])
                            else:
                                nc.scalar.activation(
                                    v0view(v0t1, 64, 128), fv(0, 64), Act.Gelu,
                                    bias=ws["fc1_bias"][0:64, 1:2])
                                nc.scalar.activation(
                                    v0view(v0t2, 0, 64), fv(64, 128), Act.Gelu,
                                    bias=ws["fc1_bias"][64:128, 1:2])
                                nc.scalar.activation(
                                    v0view(v0t2, 64, 128, shift=1), fv(64, 128),
                                    Act.Gelu, bias=ws["fc1_bias"][64:128, 1:2])

                if FP8DW_C:
                    v0t18 = cv0.tile([128, P3], FP8, name="v0t18")
                    v0t28 = cv0.tile([128, P3], FP8, name="v0t28")
                    nc.scalar.copy(v0t18[:], v0t1[:])
                    nc.vector.tensor_copy(out=v0t28[:], in_=v0t2[:])
                with (
                    tc.tile_pool(name="c2_rot", bufs=3) as c2r,
                    tc.tile_pool(name="c2_ps", bufs=2, space="PSUM") as c2ps,
                ):
                    for ci in range(NCH):
                        c0 = ci * RC
                        nr_c = min(RC, H - c0)
                        N = nr_c * Wp3
                        NN = nr_c * W
                        sb0 = pd3(3 + c0)
                        ps_a = c2ps.tile([128, RC * Wp3], F32, tag="psa")
                        if FP8DW_C:
                            poff = lambda t: (t // 5 - 2) * Wp3 + (t % 5 - 2)
                            for pi in range(13):
                                ta, tb = 2 * pi, 2 * pi + 1
                                oa = sb0 + poff(ta)
                                delta = (poff(tb) - poff(ta)) if tb < 25 else 1
                                lhsT = ws["pair8"][:, pi * 256:(pi + 1) * 256] \
                                    .rearrange("p (two m) -> p two m", two=2)
                                nc.tensor.matmul(
                                    ps_a[:, :N], lhsT,
                                    dr_rhs(v0t18[:, oa:oa + N], delta, N),
                                    start=(pi == 0), stop=(pi == 12),
                                    perf_mode=DR)
                        else:
                            for t in range(25):
                                dy, dx = t // 5 - 2, t % 5 - 2
                                o = sb0 + dy * Wp3 + dx
                                nc.tensor.matmul(ps_a[:, :N],
                                                 ws["pair_diag"][:, t * 128:(t + 1) * 128],
                                                 v0t1[:, o:o + N],
                                                 start=(t == 0), stop=(t == 24))
                        ps_b = c2ps.tile([64, RC * Wp3], F32, tag="psb")
                        if FP8DW_C:
                            nmeta = len(dw3_passes) // 2
                            for pi in range(nmeta):
                                (dya, dxaa, _), (dyb, dxab, _) = \
                                    dw3_passes[2 * pi], dw3_passes[2 * pi + 1]
                                oa = sb0 + dya * Wp3 + dxaa
                                delta = (dyb * Wp3 + dxab) - (dya * Wp3 + dxaa)
                                lhsT = ws["dw38"][:, pi * 128:(pi + 1) * 128] \
                                    .rearrange("p (two m) -> p two m", two=2)
                                nc.tensor.matmul(
                                    ps_b[:, :N], lhsT,
                                    dr_rhs(v0t28[:, oa:oa + N], delta, N),
                                    start=(pi == 0), stop=(pi == nmeta - 1),
                                    perf_mode=DR)
                        else:
                            for i, (dy, dxa, hasb) in enumerate(dw3_passes):
                                o = sb0 + dy * Wp3 + dxa
                                nc.tensor.matmul(ps_b[:, :N],
                                                 ws["dw3_diag"][:, i * 64:(i + 1) * 64],
                                                 v0t2[:, o:o + N],
                                                 start=(i == 0), stop=(i == len(dw3_passes) - 1))

                        def inner(ap_flat, lo, hi):
                            # interior view of a PSUM chunk (starts at free 0)
                            return ap_flat[lo:hi, :N].rearrange(
                                "p (r w) -> p r w", w=Wp3)[:, :, 3:3 + W]

                        def inner_v0(tl, lo, hi):
                            # interior view of the padded v0 buffers at this chunk
                            return tl[lo:hi, sb0:sb0 + N].rearrange(
                                "p (r w) -> p r w", w=Wp3)[:, :, 3:3 + W]

                        ug_a = c2r.tile([128, RC * W], BF16, tag="uga")
                        ug_b = c2r.tile([128, RC * W], BF16, tag="ugb")
                        vb_a = c2r.tile([128, RC * W], BF16, tag="vba")
                        vb_b = c2r.tile([128, RC * W], BF16, tag="vbb")
                        uv = lambda tl, lo, hi: tl[lo:hi, :NN] \
                            .rearrange("p (r w) -> p r w", w=W)
                        nc.scalar.copy(uv(ug_a, 0, 64), inner_v0(vgug, 64, 128))
                        nc.scalar.activation(uv(ug_a, 64, 128), inner(ps_a, 0, 64),
                                             Act.Gelu, bias=ws["pair_bias"][0:64])
                        nc.scalar.activation(uv(ug_b, 0, 64), inner(ps_a, 64, 128),
                                             Act.Gelu, bias=ws["pair_bias"][64:128])
                        nc.scalar.activation(uv(ug_b, 64, 128), inner(ps_b, 0, 64),
                                             Act.Gelu, bias=ws["dw3_bias"])
                        nc.gpsimd.tensor_scalar(out=uv(vb_a, 0, 64),
                                                in0=inner_v0(vgug, 0, 64),
                                                scalar1=ws["s1a"][0:64],
                                                scalar2=ws["t1a"][0:64],
                                                op0=Alu.mult, op1=Alu.add)
                        nc.gpsimd.tensor_scalar(out=uv(vb_a, 64, 128),
                                                in0=inner_v0(v0t1, 0, 64),
                                                scalar1=ws["s1a"][64:128],
                                                scalar2=ws["t1a"][64:128],
                                                op0=Alu.mult, op1=Alu.add)
                        nc.gpsimd.tensor_scalar(out=uv(vb_b, 0, 64),
                                                in0=inner_v0(v0t1, 64, 128),
                                                scalar1=ws["s1b"][0:64],
                                                scalar2=ws["t1b"][0:64],
                                                op0=Alu.mult, op1=Alu.add)
                        nc.gpsimd.tensor_scalar(out=uv(vb_b, 64, 128),
                                                in0=inner_v0(v0t2, 0, 64),
                                                scalar1=ws["s1b"][64:128],
                                                scalar2=ws["t1b"][64:128],
                                                op0=Alu.mult, op1=Alu.add)
                        z1a = c2r.tile([128, RC * W], BF16, tag="z1a")
                        z1b = c2r.tile([128, RC * W], BF16, tag="z1b")
                        nc.gpsimd.tensor_tensor(out=z1a[:, :NN], in0=ug_a[:, :NN],
                                                in1=vb_a[:, :NN], op=Alu.mult)
                        nc.vector.tensor_tensor(out=z1b[:, :NN], in0=ug_b[:, :NN],
                                                in1=vb_b[:, :NN], op=Alu.mult)
                        for cg in range(2):
                            ops = c2ps.tile([96, RC * W], F32, tag=f"ops{cg}")
                            nc.tensor.matmul(ops[:, :NN],
                                             ws["fc2aT"][:, (cg * 2) * 96:(cg * 2 + 1) * 96],
                                             z1a[:, :NN], start=True, stop=False)
                            nc.tensor.matmul(ops[:, :NN],
                                             ws["fc2aT"][:, (cg * 2 + 1) * 96:(cg * 2 + 2) * 96],
                                             z1b[:, :NN], start=False, stop=fc2b_zero)
                            if not fc2b_zero:
                                opsv = ops[:, :NN].rearrange("p (r w) -> p r w", w=W)
                                nc.tensor.matmul(opsv,
                                                 ws["fc2bT_g0"][:, cg * 96:(cg + 1) * 96],
                                                 inner_v0(vgug, 0, 64), start=False, stop=False)
                                nc.tensor.matmul(opsv,
                                                 ws["fc2bT_g12"][:, cg * 96:(cg + 1) * 96],
                                                 inner_v0(v0t1, 0, 128), start=False, stop=False)
                                nc.tensor.matmul(opsv,
                                                 ws["fc2bT_g3"][:, cg * 96:(cg + 1) * 96],
                                                 inner_v0(v0t2, 0, 64), start=False, stop=True)
                            xrch = c2r.tile([96, RC * W], BF16, tag=f"xr{cg}", bufs=1)
                            eng = nc.sync if cg == 0 else nc.scalar
                            eng.dma_start(out=xrch[:, :NN],
                                          in_=xcp_sp[cg][:, c0 * W:c0 * W + NN])
                            ob = c2r.tile([96, RC * W], F32, tag=f"ob{cg}", bufs=1)
                            nc.vector.tensor_scalar(out=ob[:, :NN], in0=ops[:, :NN],
                                                    scalar1=ws["s3v"][:, cg:cg + 1],
                                                    scalar2=ws["out_bias"][:, cg:cg + 1],
                                                    op0=Alu.mult, op1=Alu.add)
                            oc = c2r.tile([96, RC * W], F32, tag=f"oc{cg}", bufs=1)
                            nc.gpsimd.tensor_tensor(out=oc[:, :NN], in0=ob[:, :NN],
                                                    in1=xrch[:, :NN], op=Alu.add)
                            eng2 = nc.sync if cg == 0 else nc.scalar
                            eng2.dma_start(
                                out=out_t[cg * 96:(cg + 1) * 96,
                                          c0 * W:c0 * W + NN],
                                in_=oc[:, :NN])
            _wpc_cm.__exit__(None, None, None)
    return out_t.name


# ----------------------------------------------------------------------------
# host entry
# ----------------------------------------------------------------------------

_CACHE = {}
_FC2B_ZERO = {}


def make_program(H, W, n_cores, attn_scale, dw3_passes, fc2b_zero=None):
    if fc2b_zero is None:
        fc2b_zero = _FC2B_ZERO.get("v", False)
    key = (H, W, n_cores, round(attn_scale, 9), fc2b_zero)
    if key in _CACHE:
        return _CACHE[key]
    nc = bacc.Bacc("TRN2", target_bir_lowering=False, debug=False, num_devices=n_cores)
    out_name = build(nc, H, W, n_cores, attn_scale, dw3_passes, fc2b_zero=fc2b_zero)
    nc.compile()
    _CACHE[key] = (nc, out_name)
    return nc, out_name


def make_in_maps(inputs):
    x = np.asarray(inputs["x"], np.float32)
    B = x.shape[0]
    C = x.shape[-1]
    wdict = _prep_weights({k: np.asarray(v) for k, v in inputs.items()})
    _FC2B_ZERO["v"] = wdict["_fc2b_zero"][0]
    base = {}
    for k, (shp, d) in WSPEC.items():
        base["w_" + k] = wdict[k][0].reshape(shp)
    in_maps = []
    for b in range(B):
        m = dict(base)
        # channel-major [C, H*W] on device, bf16
        m["x"] = np.ascontiguousarray(x[b].reshape(-1, C).T).astype(BF16NP)
        in_maps.append(m)
    return in_maps, wdict


def kernel(**inputs):
    x = np.asarray(inputs["x"], np.float32)
    B, H, W, C = x.shape
    in_maps, wdict = make_in_maps(inputs)
    nc, out_name = make_program(H, W, B, wdict["_attn_scale"][0],
                                wdict["_dw3_passes"][0])
    res = bass_utils.run_bass_kernel_spmd(nc, in_maps, core_ids=list(range(B)))
    return np.stack([np.asarray(res.results[b][out_name], np.float32)
                     .reshape(C, H * W).T.reshape(H, W, C) for b in range(B)])
